# revision 50
# baseline (speedup 1.0000x reference)
"""CGCNN message-passing kernel for 8 Trainium2 NeuronCores (Bass/Tile), v9.

Data-parallel by dst shard; gather-based edge pipeline:
- Host: nodes are dealt into 240 global windows (8 cores x 30 windows x 128
  slots) in descending-degree snake order, equalizing per-window edge counts
  so the uniform chunks-per-window pad cw is minimal (16). Edges go to the
  core owning their dst, grouped by dst window, chunk-padded to cw.
- lin0 is computed for ALL nodes redundantly on every core (from a replicated
  full xT) into a DRAM fp8 table, so layer 0 needs no h AllGather; layers
  1..L-1 AllGather h in fp8 (staged at the previous layer's BN boundary so
  the collective launches as early as possible).
- Per layer, per core:
  * Qd table (own shard, SBUF bf16 [128, 30, 128]) = h_own @ Wdst.
  * Full Qs table = h_full @ Wsrc into DRAM [30720, 128] bf16 in
    partition-major row order (node g -> row (g%128)*240 + g//128); PSUM->SBUF
    staging copies round-robin over DVE/ACT to minimize build latency.
  * Per 1024-edge tile: one dma_gather pulls per-edge Qs rows (1024 x 256B
    descriptors; 1024 = SWDGE ring capacity); Qe = ea(fp8) @ Wea by matmul
    (edge attrs streamed fp8, 4 tiles per DMA); the dst contribution expands
    via a host-precomputed one-hot (fp8, SBUF-resident, layer-invariant)
    matmul against the SBUF Qd table. All three accumulate in PSUM per
    128-edge chunk.
  * Nonlinearity: joint exp u = [e^-a | e^b] (f-gate weights pre-negated),
    v = ln(1+u); 1/3 of tiles compute sigmoid(a) = e^-v_f on ACT, 2/3 as
    1/(1+u_f) on DVE (bf16, engine balance). m = 2*sigmoid(a)*softplus(b);
    the factor 2 is absorbed exactly by BatchNorm using 4*EPS.
  * Aggregation one-hots (is_equal(iota, dst) * 1/cnt, bf16 on DVE) are
    pre-built per tile, and the aggregation matmuls are deferred by one tile
    so the in-order PE queue never stalls on the ACT/DVE nonlinearity chain.
  * Segment-mean accumulates per dst window in PSUM (agg PSUM shares banks
    with the build-phase staging, freeing a third pre-PSUM buffer); BatchNorm
    batch stats via a tiny stats AllGather + local sum; the residual
    (scalar_tensor_tensor + relu) is computed in halves, with the fp8
    AllGather payload produced on ACT in parallel with the f32/bf16 h copies
    on DVE/Pool.
- Global mean pool via one-hot matmul, bf16 partials AllGathered and summed
  locally, head MLP computed redundantly on every core.
"""
import numpy as np
import ml_dtypes

N = 30000
E = 480000
NF = 92
EF = 50
D1 = 64
D2 = 64
L = 3
FC = 2
G = 256
EPS = 1e-5
NCORES = 8
SHARD = N // NCORES            # 3750
SHARD_P = 3840                 # padded shard (30 windows of 128)
NWIN = SHARD_P // 128          # 30
NWING = NCORES * NWIN          # 240 global windows
TBL = NCORES * SHARD_P         # 30720 table rows

_CACHE = {}



def _build_nc(cw):
    """Build the SPMD bass module. cw = chunks per dst window (uniform)."""
    import concourse.mybir as mybir
    from concourse import bacc
    from concourse.tile import TileContext

    f32 = mybir.dt.float32
    bf16 = mybir.dt.bfloat16
    f8 = mybir.dt.float8e4
    i16 = mybir.dt.int16
    AF = mybir.ActivationFunctionType
    OP = mybir.AluOpType

    nchunk = NWIN * cw                 # chunks per layer per core
    etot = nchunk * 128                # padded edges per core
    ntile = (nchunk + 7) // 8          # 8-chunk (1024-edge) PSUM tiles

    import concourse.hw_specs as _hw
    import concourse.bacc as _bacc_mod
    _real_tables = _hw.get_activation_tables("gen3")
    _combined = "natural_log_exp_and_others"
    if _combined in _real_tables:
        _patched = {
            k: (v if k == _combined else (v - {AF.Exp, AF.Ln}))
            for k, v in _real_tables.items()
        }
        _bacc_mod.get_activation_tables = lambda arch: _patched

    nc = bacc.Bacc(None, target_bir_lowering=False)

    # ---- inputs (per core) ----
    xTF = nc.dram_tensor("xTF", [NF, TBL], bf16, kind="ExternalInput")
    xT = nc.dram_tensor("xT", [NF, SHARD_P], bf16, kind="ExternalInput")
    eaT = nc.dram_tensor("eaT", [64, etot], f8, kind="ExternalInput")
    qs_idxD = nc.dram_tensor("qs_idxD", [128, etot // 16], i16, kind="ExternalInput")
    ohTD = nc.dram_tensor("ohTD", [128, etot], f8, kind="ExternalInput")
    dstloc_p = nc.dram_tensor("dstloc_p", [128, nchunk], f32, kind="ExternalInput")
    rc_p = nc.dram_tensor("rc_p", [128, nchunk], f32, kind="ExternalInput")
    batchloc = nc.dram_tensor("batchloc", [128, NWIN], f32, kind="ExternalInput")
    rgc_pn = nc.dram_tensor("rgc_pn", [128, NWIN], f32, kind="ExternalInput")
    # weights (replicated; f-gate halves pre-negated)
    lin0w = nc.dram_tensor("lin0w", [NF, D1], bf16, kind="ExternalInput")
    lin0b = nc.dram_tensor("lin0b", [D1, 1], f32, kind="ExternalInput")
    wdst = nc.dram_tensor("wdst", [D1, L * 128], bf16, kind="ExternalInput")
    wsrc = nc.dram_tensor("wsrc", [D1, L * 128], bf16, kind="ExternalInput")
    wea = nc.dram_tensor("wea", [64, L * 128], bf16, kind="ExternalInput")
    bng = nc.dram_tensor("bng", [D1, L], f32, kind="ExternalInput")
    bnb = nc.dram_tensor("bnb", [D1, L], f32, kind="ExternalInput")
    lin1w = nc.dram_tensor("lin1w", [D1, D2], f32, kind="ExternalInput")
    lin1b = nc.dram_tensor("lin1b", [D2, 1], f32, kind="ExternalInput")
    fcw = nc.dram_tensor("fcw", [D2, FC * D2], f32, kind="ExternalInput")
    fcb = nc.dram_tensor("fcb", [D2, FC], f32, kind="ExternalInput")
    lin2w = nc.dram_tensor("lin2w", [D2, 1], f32, kind="ExternalInput")
    lin2b = nc.dram_tensor("lin2b", [1, 1], f32, kind="ExternalInput")
    iota128 = nc.dram_tensor("iota128", [128, 128], bf16, kind="ExternalInput")
    iota256 = nc.dram_tensor("iota256", [128, G], bf16, kind="ExternalInput")
    ident = nc.dram_tensor("ident", [128, 128], f32, kind="ExternalInput")
    identb = nc.dram_tensor("identb", [128, 128], bf16, kind="ExternalInput")

    yout = nc.dram_tensor("y", [1, G], f32, kind="ExternalOutput")

    # ---- DRAM scratch ----
    QsD = nc.dram_tensor("QsD", [TBL, 128], bf16)          # row p*NWING+W
    h1f8 = nc.dram_tensor("h1f8", [NCORES * D1, SHARD_P], f8)
    ag_in = nc.dram_tensor("ag_in", [D1, SHARD_P], f8)
    ag_out = nc.dram_tensor("ag_out", [NCORES * D1, SHARD_P], f8,
                            addr_space="Shared")
    ar_in = nc.dram_tensor("ar_in", [D1, 2], f32)
    ar_out = nc.dram_tensor("ar_out", [NCORES * D1, 2], f32, addr_space="Shared")
    pl_in = nc.dram_tensor("pl_in", [D1, G], bf16)
    pl_out = nc.dram_tensor("pl_out", [NCORES * D1, G], bf16, addr_space="Shared")

    rg = [list(range(NCORES))]
    QsD3 = QsD[:, :].rearrange("(p w) f -> p w f", p=128)   # [128, NWING, 128]

    from contextlib import ExitStack
    with TileContext(nc) as tc:
        with ExitStack() as _es:
            cp = _es.enter_context(tc.tile_pool(name="const", bufs=1))
            bigp = _es.enter_context(tc.tile_pool(name="big", bufs=1))
            wp = _es.enter_context(tc.tile_pool(name="work", bufs=3))
            tlp = _es.enter_context(tc.tile_pool(name="tail", bufs=1))
            gp = _es.enter_context(tc.tile_pool(name="gat", bufs=4))
            ep = _es.enter_context(tc.tile_pool(name="ea", bufs=3))
            nlp = _es.enter_context(tc.tile_pool(name="nl", bufs=3))
            ohp = _es.enter_context(tc.tile_pool(name="oh", bufs=13))
            ohgp = _es.enter_context(tc.tile_pool(name="ohg", bufs=5))
            stp = _es.enter_context(tc.tile_pool(name="st", bufs=2))
            sgp = _es.enter_context(tc.tile_pool(name="sgp", bufs=3))
            scp = _es.enter_context(tc.tile_pool(name="scr", bufs=1))
            pp = _es.enter_context(tc.tile_pool(name="pre", bufs=3, space="PSUM"))
            ppB = _es.enter_context(tc.tile_pool(name="psB", bufs=2, space="PSUM"))
            # ---------- constants ----------
            def load_const(t, dram, shape, dtype=f32):
                tt = cp.tile(shape, dtype, tag=t)
                nc.sync.dma_start(out=tt[:], in_=dram)
                return tt

            io128 = load_const("io128", iota128[:, :], [128, 128], bf16)
            io256 = load_const("io256", iota256[:, :], [128, G], bf16)
            idn = load_const("idn", ident[:, :], [128, 128])
            idnb = load_const("idnb", identb[:, :], [128, 128], bf16)
            l0w = load_const("l0w", lin0w[:, :], [NF, D1], bf16)
            l0b = load_const("l0b", lin0b[:, :], [D1, 1])
            wd = load_const("wd", wdst[:, :], [D1, L * 128], bf16)
            ws = load_const("ws", wsrc[:, :], [D1, L * 128], bf16)
            we = load_const("we", wea[:, :], [64, L * 128], bf16)
            gmt = load_const("gmt", bng[:, :], [D1, L])
            bbt = load_const("bbt", bnb[:, :], [D1, L])
            l1w = load_const("l1w", lin1w[:, :], [D1, D2])
            l1b = load_const("l1b", lin1b[:, :], [D2, 1])
            fw = load_const("fw", fcw[:, :], [D2, FC * D2])
            fb = load_const("fb", fcb[:, :], [D2, FC])
            l2w = load_const("l2w", lin2w[:, :], [D2, 1])
            l2b = load_const("l2b", lin2b[:, :], [1, 1])
            dlp = load_const("dlp", dstloc_p[:, :], [128, nchunk])
            rcp = load_const("rcp", rc_p[:, :], [128, nchunk])
            blc = load_const("blc", batchloc[:, :], [128, NWIN])
            rgp = load_const("rgp", rgc_pn[:, :], [128, NWIN])
            qsix = load_const("qsix", qs_idxD[:, :], [128, etot // 16], i16)
            # resident layer-invariant one-hot [slot, edge] (fp8)
            ohT_res = cp.tile([128, nchunk, 128], f8, tag="ohres")
            nc.sync.dma_start(
                out=ohT_res[:].rearrange("p a b -> p (a b)"), in_=ohTD[:, :])

            # ---------- resident state ----------
            hT_own = bigp.tile([D1, SHARD_P], f32, tag="hown")
            hb_own = bigp.tile([D1, SHARD_P], bf16, tag="hbown")
            aggr_sb = bigp.tile([D1, SHARD_P], bf16, tag="aggr")
            qd_sb = bigp.tile([128, NWIN, 128], bf16, tag="qdsb")
            asb = scp.tile([D1, SHARD_P], f32, tag="asb")

            # ---------- lin0 for ALL nodes (no AllGather for layer 0) ----
            # h1f8 holds relu(x @ lin0_w + b) for all 8 shards (global order),
            # computed redundantly on every core from the replicated xTF.
            HL = SHARD_P // 2
            for s_ in range(NCORES):
                h8s = stp.tile([D1, SHARD_P], f8, tag="h8")
                for hh in range(2):
                    xt = sgp.tile([NF, HL], bf16, tag="qsst")
                    o = s_ * SHARD_P + hh * HL
                    nc.sync.dma_start(out=xt[:], in_=xTF[:, o:o + HL])
                    for j in range(4):
                        sl = slice(j * 480, (j + 1) * 480)
                        ph = ppB.tile([D1, 512], f32, tag="bld")
                        nc.tensor.matmul(out=ph[:, :480], lhsT=l0w[:],
                                         rhs=xt[:, sl], start=True, stop=True)
                        osl = slice(hh * HL + j * 480, hh * HL + (j + 1) * 480)
                        if j % 2 == 0:
                            nc.scalar.activation(
                                out=h8s[:, osl],
                                in_=ph[:, :480], func=AF.Relu, bias=l0b[:],
                                scale=1.0)
                        else:
                            nc.vector.tensor_scalar(
                                out=h8s[:, osl], in0=ph[:, :480],
                                scalar1=l0b[:], scalar2=0.0,
                                op0=OP.add, op1=OP.max)
                nc.sync.dma_start(out=h1f8[s_ * D1:(s_ + 1) * D1, :], in_=h8s[:])

            # own-shard h in f32 from the per-core xT input
            for hh in range(2):
                xt0 = sgp.tile([NF, HL], bf16, tag="qsst")
                nc.sync.dma_start(out=xt0[:], in_=xT[:, hh * HL:(hh + 1) * HL])
                for j in range(4):
                    sl = slice(hh * HL + j * 480, hh * HL + (j + 1) * 480)
                    ph = ppB.tile([D1, 512], f32, tag="bld")
                    nc.tensor.matmul(out=ph[:, :480], lhsT=l0w[:],
                                     rhs=xt0[:, j * 480:(j + 1) * 480],
                                     start=True, stop=True)
                    nc.scalar.activation(out=hT_own[:, sl], in_=ph[:, :480],
                                         func=AF.Relu, bias=l0b[:], scale=1.0)
                    nc.vector.tensor_scalar(
                        out=hb_own[:, sl], in0=ph[:, :480],
                        scalar1=l0b[:], scalar2=0.0, op0=OP.add, op1=OP.max)

            # ---------- layers ----------
            for l in range(L):
                wd_l = wd[:, l * 128:(l + 1) * 128]
                ws_l = ws[:, l * 128:(l + 1) * 128]
                we_l = we[:, l * 128:(l + 1) * 128]

                if l == 0:
                    src_dram = h1f8
                else:
                    # --- AllGather h (fp8, staged into ag_in at layer end) ---
                    nc.gpsimd.collective_compute(
                        "AllGather", OP.bypass, replica_groups=rg,
                        ins=[ag_in.ap().opt()], outs=[ag_out.ap().opt()])
                    src_dram = ag_out

                # --- Qd table build (own shard) ---
                for w0 in range(0, NWIN, 4):
                    kk = min(4, NWIN - w0)
                    qp = ppB.tile([128, 512], f32, tag="bld")
                    for k in range(kk):
                        w = w0 + k
                        nc.tensor.matmul(
                            out=qp[:, k * 128:(k + 1) * 128],
                            lhsT=hb_own[:, w * 128:(w + 1) * 128],
                            rhs=wd_l, start=True, stop=True)
                    nc.vector.tensor_copy(
                        out=qd_sb[:, w0:w0 + kk, :].rearrange("p a b -> p (a b)"),
                        in_=qp[:, :kk * 128])

                # --- Qs table build (all nodes, per gathered shard) -> QsD ---
                ws8 = stp.tile([D1, 128], f8, tag="ws8")
                nc.scalar.activation(out=ws8[:], in_=ws_l,
                                     func=AF.Identity, scale=1.0)
                ncopy = 0
                for s_ in range(NCORES):
                    hb_sh = stp.tile([D1, SHARD_P], f8, tag="h8")
                    nc.sync.dma_start(out=hb_sh[:],
                                      in_=src_dram[s_ * D1:(s_ + 1) * D1, :])
                    for wB in range(0, NWIN, 16):
                        kB = min(16, NWIN - wB)
                        sg_t = sgp.tile([128, 16, 128], bf16, tag="qsst")
                        for w0 in range(wB, wB + kB, 4):
                            kk = min(4, wB + kB - w0)
                            qp = ppB.tile([128, 512], f32, tag="bld")
                            for k in range(kk):
                                w = w0 + k
                                nc.tensor.matmul(
                                    out=qp[:, k * 128:(k + 1) * 128],
                                    lhsT=hb_sh[:, w * 128:(w + 1) * 128],
                                    rhs=ws8[:], start=True, stop=True)
                            dst_ap = sg_t[:, w0 - wB:w0 - wB + kk, :] \
                                .rearrange("p a b -> p (a b)")
                            eng = ncopy % 5
                            ncopy += 1
                            if eng in (0, 2, 4):
                                nc.vector.tensor_copy(
                                    out=dst_ap, in_=qp[:, :kk * 128])
                            else:
                                nc.scalar.activation(
                                    out=dst_ap, in_=qp[:, :kk * 128],
                                    func=AF.Identity, scale=1.0)
                        W0 = s_ * NWIN + wB
                        nc.sync.dma_start(out=QsD3[:, W0:W0 + kB, :],
                                          in_=sg_t[:, :kB, :])

                # --- edge pipeline ---
                st1g = wp.tile([D1, 8], f32, tag="st1g")
                st2g = wp.tile([D1, 8], f32, tag="st2g")
                agg = None
                qs_g = None
                aggst = {"agg": None}

                def emit_agg(m, ohs_t, t, te):
                    # aggregation for tile t, deferred one tile so the PE
                    # queue never stalls waiting for m
                    for c in range(te):
                        gc = t * 8 + c
                        w = gc // cw
                        if gc % (4 * cw) == 0:
                            agg_new = ppB.tile([D1, 512], f32, tag="bld")
                            aggst["agg"] = agg_new
                        agg = aggst["agg"]
                        nc.tensor.matmul(
                            out=agg[:, (w % 4) * 128:(w % 4 + 1) * 128],
                            lhsT=m[:, c, :], rhs=ohs_t[c][:],
                            start=(gc % cw == 0), stop=(gc % cw == cw - 1))
                        if gc % (4 * cw) == 4 * cw - 1 or gc == nchunk - 1:
                            grp = w // 4
                            lo = grp * 512
                            hi = min(lo + 512, SHARD_P)
                            nc.scalar.activation(
                                out=aggr_sb[:, lo:hi], in_=agg[:, :hi - lo],
                                func=AF.Identity, scale=1.0)
                            nc.vector.reduce_sum(
                                out=st1g[:, grp:grp + 1],
                                in_=aggr_sb[:, lo:hi],
                                axis=mybir.AxisListType.X)
                            sqg = nlp.tile([D1, 512], bf16, tag="sqg")
                            nc.vector.tensor_tensor(
                                out=sqg[:, :hi - lo], in0=aggr_sb[:, lo:hi],
                                in1=aggr_sb[:, lo:hi], op=OP.mult)
                            nc.vector.reduce_sum(
                                out=st2g[:, grp:grp + 1],
                                in_=sqg[:, :hi - lo],
                                axis=mybir.AxisListType.X)

                pend = []
                for t in range(ntile):
                    te = min(8, nchunk - t * 8)          # chunks this tile
                    ne = te * 128                        # edges this tile
                    if t % 4 == 0:
                        tc32 = min(32, nchunk - t * 8)
                        et = ep.tile([64, 4096], f8, tag="et")
                        nc.sync.dma_start(
                            out=et[:, :tc32 * 128],
                            in_=eaT[:, t * 1024: t * 1024 + tc32 * 128])
                    qs_g = gp.tile([128, 8, 128], bf16, tag="qsg")
                    nc.gpsimd.dma_gather(
                        qs_g[:, :te, :], QsD[:, :],
                        qsix[:, t * 64: t * 64 + te * 8],
                        te * 128, te * 128, 128)
                    half = 0
                    qhalf = (t % 4) * 8

                    # one-hot aggregation matrices: const-only deps, built
                    # ahead so the agg matmuls never wait on DVE
                    ohs_t = []
                    for c in range(te):
                        gc = t * 8 + c
                        oh_ = ohp.tile([128, 128], bf16, tag="ohS")
                        nc.vector.tensor_scalar(
                            out=oh_[:], in0=io128[:],
                            scalar1=dlp[:, gc:gc + 1], scalar2=rcp[:, gc:gc + 1],
                            op0=OP.is_equal, op1=OP.mult)
                        ohs_t.append(oh_)

                    pre = pp.tile([128, 1024], f32, tag="pre")
                    qs_f = qs_g[:].rearrange("p a b -> p (a b)")
                    for c in range(te):
                        gc = t * 8 + c
                        w = gc // cw
                        csl = slice(c * 128, (c + 1) * 128)
                        csl2 = slice((half + c) * 128, (half + c + 1) * 128)
                        csl4 = slice((qhalf + c) * 128, (qhalf + c + 1) * 128)
                        nc.tensor.matmul(out=pre[:, csl], lhsT=et[:, csl4],
                                         rhs=we_l, start=True, stop=False)
                        nc.tensor.matmul(out=pre[:, csl], lhsT=idnb[:],
                                         rhs=qs_f[:, csl2], start=False, stop=False)
                        nc.tensor.matmul(out=pre[:, csl], lhsT=ohT_res[:, gc, :],
                                         rhs=qd_sb[:, w, :], start=False, stop=True)

                    # nonlinearity: m = (1+tanh(a/2)) * softplus(b)
                    #             = 2*sigmoid(a)*softplus(b)  (2 absorbed by BN)
                    # nonlinearity: u = [e^-a | e^b], v = ln(1+u) = [sp(-a)|sp(b)]
                    # even tiles (ACT): sigma = e^-sp(-a); odd tiles (DVE):
                    # sigma = 1/(1+e^-a). m = 2*sigma*sp(b) (2 absorbed by BN
                    # via 4*EPS).
                    uf = nlp.tile([128, 8, 128], bf16, tag="uf")
                    nc.scalar.activation(
                        out=uf[:, :te, :].rearrange("p a b -> p (a b)"),
                        in_=pre[:, :ne], func=AF.Exp, scale=1.0)
                    m = nlp.tile([128, 8, 64], bf16, tag="m")
                    if t % 3 == 0:
                        vf = nlp.tile([128, 8, 128], bf16, tag="vf")
                        nc.scalar.activation(
                            out=vf[:, :te, :].rearrange("p a b -> p (a b)"),
                            in_=uf[:, :te, :].rearrange("p a b -> p (a b)"),
                            func=AF.Ln, bias=1.0, scale=1.0)
                        sg = nlp.tile([128, 8, 64], bf16, tag="sg")
                        nc.scalar.activation(out=sg[:, :te, :],
                                             in_=vf[:, :te, 0:64],
                                             func=AF.Exp, scale=-1.0)
                        nc.vector.scalar_tensor_tensor(
                            out=m[:, :te, :], in0=sg[:, :te, :], scalar=2.0,
                            in1=vf[:, :te, 64:128], op0=OP.mult, op1=OP.mult)
                    else:
                        vs = nlp.tile([128, 8, 64], bf16, tag="vs")
                        nc.scalar.activation(out=vs[:, :te, :],
                                             in_=uf[:, :te, 64:128],
                                             func=AF.Ln, bias=1.0, scale=1.0)
                        w1 = nlp.tile([128, 8, 64], bf16, tag="sg")
                        with nc.allow_low_precision(reason="sigmoid in bf16"):
                            nc.vector.tensor_scalar(out=w1[:, :te, :],
                                                    in0=uf[:, :te, 0:64],
                                                    scalar1=1.0, scalar2=None,
                                                    op0=OP.add)
                            nc.vector.reciprocal(out=w1[:, :te, :],
                                                 in_=w1[:, :te, :])
                        nc.vector.scalar_tensor_tensor(
                            out=m[:, :te, :], in0=w1[:, :te, :], scalar=2.0,
                            in1=vs[:, :te, :], op0=OP.mult, op1=OP.mult)

                    pend.append((m, ohs_t, t, te))
                    if len(pend) > 1:
                        emit_agg(*pend.pop(0))
                while pend:
                    emit_agg(*pend.pop(0))

                # --- BN stats + AllReduce ---
                st = wp.tile([D1, 2], f32, tag="st")
                nc.vector.reduce_sum(out=st[:, 0:1], in_=st1g[:],
                                     axis=mybir.AxisListType.X)
                nc.vector.reduce_sum(out=st[:, 1:2], in_=st2g[:],
                                     axis=mybir.AxisListType.X)
                nc.sync.dma_start(out=ar_in[:, :], in_=st[:])
                nc.gpsimd.collective_compute(
                    "AllGather", OP.bypass, replica_groups=rg,
                    ins=[ar_in.ap().opt()], outs=[ar_out.ap().opt()])
                stga = wp.tile([D1, 2, NCORES], f32, tag="stga")
                nc.sync.dma_start(
                    out=stga[:],
                    in_=ar_out[:, :].rearrange("(c p) s -> p s c", p=D1))
                stg = wp.tile([D1, 2], f32, tag="stg")
                nc.vector.reduce_sum(
                    out=stg[:].rearrange("p (s o) -> p s o", o=1),
                    in_=stga[:], axis=mybir.AxisListType.X)
                mu = wp.tile([D1, 1], f32, tag="mu")
                nc.vector.tensor_scalar(out=mu[:], in0=stg[:, 0:1],
                                        scalar1=1.0 / N, scalar2=None, op0=OP.mult)
                ex2 = wp.tile([D1, 1], f32, tag="ex2")
                nc.vector.tensor_scalar(out=ex2[:], in0=stg[:, 1:2],
                                        scalar1=1.0 / N, scalar2=None, op0=OP.mult)
                mu2 = wp.tile([D1, 1], f32, tag="mu2")
                nc.vector.tensor_tensor(out=mu2[:], in0=mu[:], in1=mu[:], op=OP.mult)
                var = wp.tile([D1, 1], f32, tag="var")
                nc.vector.tensor_tensor(out=var[:], in0=ex2[:], in1=mu2[:],
                                        op=OP.subtract)
                ve = wp.tile([D1, 1], f32, tag="ve")
                # m carries a factor 2 -> aggr/mu scale by 2, var by 4; using
                # 4*EPS makes BN output exactly match the reference.
                nc.vector.tensor_scalar(out=ve[:], in0=var[:], scalar1=4.0 * EPS,
                                        scalar2=None, op0=OP.add)
                lv = wp.tile([D1, 1], f32, tag="lv")
                nc.scalar.activation(out=lv[:], in_=ve[:], func=AF.Ln, scale=1.0)
                isd = wp.tile([D1, 1], f32, tag="isd")
                nc.scalar.activation(out=isd[:], in_=lv[:], func=AF.Exp, scale=-0.5)
                scale = wp.tile([D1, 1], f32, tag="scale")
                nc.vector.tensor_tensor(out=scale[:], in0=isd[:],
                                        in1=gmt[:, l:l + 1], op=OP.mult)
                mshift = wp.tile([D1, 1], f32, tag="mshift")
                nc.vector.tensor_tensor(out=mshift[:], in0=mu[:], in1=scale[:],
                                        op=OP.mult)
                shift = wp.tile([D1, 1], f32, tag="shift")
                nc.vector.tensor_tensor(out=shift[:], in0=bbt[:, l:l + 1],
                                        in1=mshift[:], op=OP.subtract)
                # h = relu((aggr*scale + h) + shift); the three consumers
                # (f32 residual, f8 AllGather payload, bf16 matmul copy) are
                # produced from asb concurrently on DVE/ACT/Pool
                HB = SHARD_P // 2
                for hh in range(2):
                    hsl = slice(hh * HB, (hh + 1) * HB)
                    nc.vector.scalar_tensor_tensor(
                        out=asb[:, hsl], in0=aggr_sb[:, hsl], scalar=scale[:],
                        in1=hT_own[:, hsl], op0=OP.mult, op1=OP.add)
                if l + 1 < L:
                    h8n = stp.tile([D1, SHARD_P], f8, tag="h8")
                    for hh in range(2):
                        hsl = slice(hh * HB, (hh + 1) * HB)
                        nc.scalar.activation(out=h8n[:, hsl], in_=asb[:, hsl],
                                             func=AF.Relu, bias=shift[:],
                                             scale=1.0)
                        nc.sync.dma_start(out=ag_in[:, hsl], in_=h8n[:, hsl])
                nc.vector.tensor_scalar(out=hT_own[:], in0=asb[:],
                                        scalar1=shift[:], scalar2=0.0,
                                        op0=OP.add, op1=OP.max)
                nc.gpsimd.tensor_copy(out=hb_own[:], in_=hT_own[:])

            # ---------- global mean pool ----------
            pool_ps = pp.tile([D1, G], f32, tag="pre")
            for w in range(NWIN):
                tp = ppB.tile([128, D1], f32, tag="bld")
                nc.tensor.transpose(out=tp[:], in_=hT_own[:, w * 128:(w + 1) * 128],
                                    identity=idn[0:D1, 0:D1])
                rows = wp.tile([128, D1], bf16, tag="rows")
                nc.vector.tensor_copy(out=rows[:], in_=tp[:])
                ohg = ohgp.tile([128, G], bf16, tag="ohg")
                nc.vector.tensor_scalar(
                    out=ohg[:], in0=io256[:],
                    scalar1=blc[:, w:w + 1], scalar2=rgp[:, w:w + 1],
                    op0=OP.is_equal, op1=OP.mult)
                nc.tensor.matmul(out=pool_ps[:], lhsT=rows[:], rhs=ohg[:],
                                 start=(w == 0), stop=(w == NWIN - 1))
            poolT = tlp.tile([D1, G], bf16, tag="poolT")
            nc.vector.tensor_copy(out=poolT[:], in_=pool_ps[:])
            nc.sync.dma_start(out=pl_in[:, :], in_=poolT[:])
            nc.gpsimd.collective_compute(
                "AllGather", OP.bypass, replica_groups=rg,
                ins=[pl_in.ap().opt()], outs=[pl_out.ap().opt()])
            pga = tlp.tile([D1, NCORES, G], bf16, tag="pga")
            nc.sync.dma_start(
                out=pga[:],
                in_=pl_out[:, :].rearrange("(c p) g -> p c g", p=D1))
            pg = tlp.tile([D1, G], f32, tag="pg")
            nc.vector.reduce_sum(
                out=pg[:].rearrange("p (g o) -> p g o", o=1),
                in_=pga[:].rearrange("p c g -> p g c"),
                axis=mybir.AxisListType.X)

            # ---------- head ----------
            a = pg
            hw_ = [(l1w[:], l1b[:]), (fw[:, 0:D2], fb[:, 0:1]), (fw[:, D2:2 * D2], fb[:, 1:2])]
            for (wt, bt) in hw_:
                ps = ppB.tile([D2, G], f32, tag="bld")
                nc.tensor.matmul(out=ps[:, 0:G], lhsT=wt, rhs=a[:], start=True, stop=True)
                an = tlp.tile([D2, G], f32, tag="an")
                nc.scalar.activation(out=an[:], in_=ps[:, 0:G], func=AF.Relu,
                                     bias=bt, scale=1.0)
                a = an
            ps = ppB.tile([1, G], f32, tag="bld")
            nc.tensor.matmul(out=ps[:, 0:G], lhsT=l2w[:], rhs=a[:], start=True, stop=True)
            yt = tlp.tile([1, G], f32, tag="yt")
            nc.scalar.activation(out=yt[:], in_=ps[:, 0:G], func=AF.Identity,
                                 bias=l2b[:], scale=1.0)
            nc.sync.dma_start(out=yout[:, :], in_=yt[:])

    nc.compile()
    return nc


def _wrap16(idx):
    """Flat idx list -> [128, n/16] int16: slot i at [i%16, i//16], replicated
    across the 8 Q7 cores."""
    a = idx.reshape(-1, 16).T.astype(np.int16)
    return np.tile(a, (8, 1))


def _preprocess(inputs):
    x = np.asarray(inputs["x"], np.float32)
    ea = np.asarray(inputs["edge_attr"], np.float32)
    ei = np.asarray(inputs["edge_index"]).astype(np.int64)
    batch = np.asarray(inputs["batch"]).astype(np.int64)
    src, dst = ei[0], ei[1]

    cnt = np.bincount(dst, minlength=N).astype(np.float32)
    rc_node = 1.0 / np.maximum(cnt, 1.0)
    gcnt = np.bincount(batch, minlength=G).astype(np.float32)
    rgc = 1.0 / np.maximum(gcnt, 1.0)

    # Degree-balanced node -> (window, slot) assignment: snake-deal nodes in
    # descending-degree order across the 240 global windows, minimizing the
    # max per-window edge count (which sets the uniform chunk pad cw).
    deg_order = np.argsort(-cnt, kind="stable")       # node ids, deg desc
    nwin_g = NCORES * NWIN                            # 240
    perm_loc = np.empty(N, np.int64)                  # node -> global padded id
    for i0 in range(0, N, nwin_g):
        blk = deg_order[i0:i0 + nwin_g]
        j = i0 // nwin_g
        wins = np.arange(len(blk)) if j % 2 == 0 else (len(blk) - 1 - np.arange(len(blk)))
        w_ids = wins
        perm_loc[blk] = (w_ids // NWIN) * SHARD_P + (w_ids % NWIN) * 128 + j
    gperm = perm_loc
    srcg = gperm[src]
    dstg = gperm[dst]
    order = np.argsort(dstg, kind="stable")
    srcg_s, dstg_s, ea_idx = srcg[order], dstg[order], order

    bounds = []
    for c in range(NCORES):
        for w in range(NWIN):
            bounds.append(c * SHARD_P + w * 128)
    bounds.append(NCORES * SHARD_P)
    bpos = np.searchsorted(dstg_s, np.asarray(bounds), side="left")
    percw = {}
    maxcnt = 0
    k = 0
    for c in range(NCORES):
        for w in range(NWIN):
            lo, hi = bpos[k], bpos[k + 1]
            percw[(c, w)] = np.arange(lo, hi)
            maxcnt = max(maxcnt, hi - lo)
            k += 1
    cw = max(1, (maxcnt + 127) // 128)
    etot = NWIN * cw * 128

    # full padded x, rotated per core so block 0 is the own shard
    xfull = np.zeros((NF, NCORES * SHARD_P), np.float32)
    xfull[:, gperm] = x.T
    xfull = xfull.astype(ml_dtypes.bfloat16)

    per_core = []
    for c in range(NCORES):
        qs_idx = np.zeros(etot, np.int64)
        dl = np.full(etot, -1.0, np.float32)
        rc_e = np.ones(etot, np.float32)
        ea_e = np.zeros((etot, EF), np.float32)
        for w in range(NWIN):
            idxs = percw[(c, w)]
            o = w * cw * 128
            k = len(idxs)
            g = srcg_s[idxs]                           # padded global id
            qs_idx[o:o + k] = (g % 128) * NWING + (g // 128)
            loc = dstg_s[idxs] - c * SHARD_P           # 0..3839
            dl[o:o + k] = (loc - w * 128).astype(np.float32)
            rc_e[o:o + k] = rc_node[dst[ea_idx[idxs]]]
            ea_e[o:o + k] = ea[ea_idx[idxs]]
        eaT = np.zeros((64, etot), np.float32)
        eaT[1:EF + 1] = ea_e.T
        eaT[EF + 1] = 1.0
        eaT[EF + 1, dl < 0] = 0.0
        nch = etot // 128
        ohT = np.zeros((128, etot), np.float32)
        vv = dl >= 0
        ohT[dl[vv].astype(np.int64), np.nonzero(vv)[0]] = 1.0
        d = {
            "qs_idxD": _wrap16(qs_idx),
            "ohTD": ohT.astype(ml_dtypes.float8_e4m3),
            "dstloc_p": dl.reshape(nch, 128).T.copy(),
            "rc_p": rc_e.reshape(nch, 128).T.copy(),
            "eaT": eaT.astype(ml_dtypes.float8_e4m3),
        }
        d["xTF"] = xfull
        d["xT"] = xfull[:, c * SHARD_P:(c + 1) * SHARD_P].copy()
        nodes_c = np.nonzero((gperm // SHARD_P) == c)[0]
        locs_c = gperm[nodes_c] - c * SHARD_P
        bl = np.full(SHARD_P, -1.0, np.float32)
        bl[locs_c] = batch[nodes_c].astype(np.float32)
        rg_n = np.zeros(SHARD_P, np.float32)
        rg_n[locs_c] = rgc[batch[nodes_c]]
        d["batchloc"] = bl.reshape(NWIN, 128).T.copy()
        d["rgc_pn"] = rg_n.reshape(NWIN, 128).T.copy()
        per_core.append(d)

    # replicated weights; f-gate halves pre-negated
    wf = np.asarray(inputs["conv_wf"], np.float32)
    wsv = np.asarray(inputs["conv_ws"], np.float32)
    bf = np.asarray(inputs["conv_bf"], np.float32)
    bs = np.asarray(inputs["conv_bs"], np.float32)
    wdst = np.concatenate([-wf[:, 0:D1, :], wsv[:, 0:D1, :]], axis=2)
    wsrc = np.concatenate([-wf[:, D1:2 * D1, :], wsv[:, D1:2 * D1, :]], axis=2)
    wea = np.concatenate([-wf[:, 2 * D1:, :], wsv[:, 2 * D1:, :]], axis=2)
    bias = np.concatenate([-bf, bs], axis=1)[:, None, :]
    wea = np.concatenate([wea, bias], axis=1)
    shared = {
        "lin0w": np.asarray(inputs["lin0_w"], np.float32).astype(ml_dtypes.bfloat16),
        "lin0b": np.asarray(inputs["lin0_b"], np.float32).reshape(D1, 1),
        "wdst": np.transpose(wdst, (1, 0, 2)).reshape(D1, L * 128).astype(ml_dtypes.bfloat16),
        "wsrc": np.transpose(wsrc, (1, 0, 2)).reshape(D1, L * 128).astype(ml_dtypes.bfloat16),
        "wea": np.concatenate([
            np.zeros((1, L * 128), np.float32),
            np.transpose(wea, (1, 0, 2)).reshape(EF + 1, L * 128),
            np.zeros((64 - EF - 2, L * 128), np.float32),
        ], axis=0).astype(ml_dtypes.bfloat16),
        "bng": np.asarray(inputs["bn_gamma"], np.float32).T.copy(),
        "bnb": np.asarray(inputs["bn_beta"], np.float32).T.copy(),
        "lin1w": np.asarray(inputs["lin1_w"], np.float32),
        "lin1b": np.asarray(inputs["lin1_b"], np.float32).reshape(D2, 1),
        "fcw": np.transpose(np.asarray(inputs["fc_w"], np.float32), (1, 0, 2)).reshape(D2, FC * D2),
        "fcb": np.asarray(inputs["fc_b"], np.float32).T.copy(),
        "lin2w": np.asarray(inputs["lin2_w"], np.float32).reshape(D2, 1),
        "lin2b": np.asarray(inputs["lin2_b"], np.float32).reshape(1, 1),
        "iota128": np.broadcast_to(np.arange(128, dtype=np.float32)[None, :],
                                   (128, 128)).astype(ml_dtypes.bfloat16),
        "iota256": np.broadcast_to(np.arange(G, dtype=np.float32)[None, :],
                                   (128, G)).astype(ml_dtypes.bfloat16),
        "ident": np.eye(128, dtype=np.float32),
        "identb": np.eye(128, dtype=np.float32).astype(ml_dtypes.bfloat16),
    }
    in_maps = [dict(shared, **pc) for pc in per_core]
    return in_maps, cw


def kernel(**inputs):
    from concourse.bass_utils import run_bass_kernel_spmd

    in_maps, cw = _preprocess(inputs)
    key = ("nc", cw)
    if key not in _CACHE:
        _CACHE[key] = _build_nc(cw)
    nc = _CACHE[key]
    res = run_bass_kernel_spmd(nc, in_maps, core_ids=list(range(NCORES)))
    return res.results[0]["y"].reshape(G).astype(np.float32)


# revision 55
# speedup vs baseline: 1.0121x; 1.0121x over previous
"""CGCNN message-passing kernel for 8 Trainium2 NeuronCores (Bass/Tile), v9.

Data-parallel by dst shard; gather-based edge pipeline:
- Host: nodes are dealt into 240 global windows (8 cores x 30 windows x 128
  slots) in descending-degree snake order, equalizing per-window edge counts
  so the uniform chunks-per-window pad cw is minimal (16). Edges go to the
  core owning their dst, grouped by dst window, chunk-padded to cw.
- lin0 is computed for ALL nodes redundantly on every core (from a replicated
  full xT) into a DRAM fp8 table, so layer 0 needs no h AllGather; layers
  1..L-1 AllGather h in fp8 (staged at the previous layer's BN boundary so
  the collective launches as early as possible).
- Per layer, per core:
  * Qd table (own shard, SBUF bf16 [128, 30, 128]) = h_own @ Wdst.
  * Full Qs table = h_full @ Wsrc into DRAM [30720, 128] bf16 in
    partition-major row order (node g -> row (g%128)*240 + g//128); PSUM->SBUF
    staging copies round-robin over DVE/ACT to minimize build latency.
  * Per 1024-edge tile: one dma_gather pulls per-edge Qs rows (1024 x 256B
    descriptors; 1024 = SWDGE ring capacity); Qe = ea(fp8) @ Wea by matmul
    (edge attrs streamed fp8, 4 tiles per DMA); the dst contribution expands
    via a host-precomputed one-hot (fp8, SBUF-resident, layer-invariant)
    matmul against the SBUF Qd table. All three accumulate in PSUM per
    128-edge chunk.
  * Nonlinearity: joint exp u = [e^-a | e^b] (f-gate weights pre-negated),
    v = ln(1+u); 1/3 of tiles compute sigmoid(a) = e^-v_f on ACT, 2/3 as
    1/(1+u_f) on DVE (bf16, engine balance). m = 2*sigmoid(a)*softplus(b);
    the factor 2 is absorbed exactly by BatchNorm using 4*EPS.
  * Aggregation one-hots (is_equal(iota, dst) * 1/cnt, bf16 on DVE) are
    pre-built per tile, and the aggregation matmuls are deferred by one tile
    so the in-order PE queue never stalls on the ACT/DVE nonlinearity chain.
  * Segment-mean accumulates per dst window in PSUM (agg PSUM shares banks
    with the build-phase staging, freeing a third pre-PSUM buffer); BatchNorm
    batch stats via a tiny stats AllGather + local sum; the residual
    (scalar_tensor_tensor + relu) is computed in halves, with the fp8
    AllGather payload produced on ACT in parallel with the f32/bf16 h copies
    on DVE/Pool.
- Global mean pool via one-hot matmul, bf16 partials AllGathered and summed
  locally, head MLP computed redundantly on every core.
"""
import numpy as np
import ml_dtypes

N = 30000
E = 480000
NF = 92
EF = 50
D1 = 64
D2 = 64
L = 3
FC = 2
G = 256
EPS = 1e-5
NCORES = 8
SHARD = N // NCORES            # 3750
SHARD_P = 3840                 # padded shard (30 windows of 128)
NWIN = SHARD_P // 128          # 30
NWING = NCORES * NWIN          # 240 global windows
TBL = NCORES * SHARD_P         # 30720 table rows

_CACHE = {}



def _build_nc(cw):
    """Build the SPMD bass module. cw = chunks per dst window (uniform)."""
    import concourse.mybir as mybir
    from concourse import bacc
    from concourse.tile import TileContext

    f32 = mybir.dt.float32
    bf16 = mybir.dt.bfloat16
    f8 = mybir.dt.float8e4
    i16 = mybir.dt.int16
    AF = mybir.ActivationFunctionType
    OP = mybir.AluOpType

    nchunk = NWIN * cw                 # chunks per layer per core
    etot = nchunk * 128                # padded edges per core
    ntile = (nchunk + 7) // 8          # 8-chunk (1024-edge) PSUM tiles

    import concourse.hw_specs as _hw
    import concourse.bacc as _bacc_mod
    _real_tables = _hw.get_activation_tables("gen3")
    _combined = "natural_log_exp_and_others"
    if _combined in _real_tables:
        _patched = {
            k: (v if k == _combined else (v - {AF.Exp, AF.Ln}))
            for k, v in _real_tables.items()
        }
        _bacc_mod.get_activation_tables = lambda arch: _patched

    nc = bacc.Bacc(None, target_bir_lowering=False)

    # ---- inputs (per core) ----
    xTF = nc.dram_tensor("xTF", [NF, TBL], bf16, kind="ExternalInput")
    xT = nc.dram_tensor("xT", [NF, SHARD_P], bf16, kind="ExternalInput")
    eaT = nc.dram_tensor("eaT", [64, etot], f8, kind="ExternalInput")
    qs_idxD = nc.dram_tensor("qs_idxD", [128, etot // 16], i16, kind="ExternalInput")
    ohTD = nc.dram_tensor("ohTD", [128, etot], f8, kind="ExternalInput")
    dstloc_p = nc.dram_tensor("dstloc_p", [128, nchunk], f32, kind="ExternalInput")
    rc_p = nc.dram_tensor("rc_p", [128, nchunk], f32, kind="ExternalInput")
    batchloc = nc.dram_tensor("batchloc", [128, NWIN], f32, kind="ExternalInput")
    rgc_pn = nc.dram_tensor("rgc_pn", [128, NWIN], f32, kind="ExternalInput")
    # weights (replicated; f-gate halves pre-negated)
    lin0w = nc.dram_tensor("lin0w", [NF, D1], bf16, kind="ExternalInput")
    lin0b = nc.dram_tensor("lin0b", [D1, 1], f32, kind="ExternalInput")
    wdst = nc.dram_tensor("wdst", [D1, L * 128], bf16, kind="ExternalInput")
    wsrc = nc.dram_tensor("wsrc", [D1, L * 128], bf16, kind="ExternalInput")
    wea = nc.dram_tensor("wea", [64, L * 128], bf16, kind="ExternalInput")
    bng = nc.dram_tensor("bng", [D1, L], f32, kind="ExternalInput")
    bnb = nc.dram_tensor("bnb", [D1, L], f32, kind="ExternalInput")
    lin1w = nc.dram_tensor("lin1w", [D1, D2], f32, kind="ExternalInput")
    lin1b = nc.dram_tensor("lin1b", [D2, 1], f32, kind="ExternalInput")
    fcw = nc.dram_tensor("fcw", [D2, FC * D2], f32, kind="ExternalInput")
    fcb = nc.dram_tensor("fcb", [D2, FC], f32, kind="ExternalInput")
    lin2w = nc.dram_tensor("lin2w", [D2, 1], f32, kind="ExternalInput")
    lin2b = nc.dram_tensor("lin2b", [1, 1], f32, kind="ExternalInput")
    iota128 = nc.dram_tensor("iota128", [128, 128], bf16, kind="ExternalInput")
    iota256 = nc.dram_tensor("iota256", [128, G], bf16, kind="ExternalInput")
    ident = nc.dram_tensor("ident", [128, 128], f32, kind="ExternalInput")
    identb = nc.dram_tensor("identb", [128, 128], bf16, kind="ExternalInput")

    yout = nc.dram_tensor("y", [1, G], f32, kind="ExternalOutput")

    # ---- DRAM scratch ----
    QsD = nc.dram_tensor("QsD", [TBL, 128], bf16)          # row p*NWING+W
    h1f8 = nc.dram_tensor("h1f8", [NCORES * D1, SHARD_P], f8)
    ag_in = nc.dram_tensor("ag_in", [D1, SHARD_P], f8)
    ag_out = nc.dram_tensor("ag_out", [NCORES * D1, SHARD_P], f8,
                            addr_space="Shared")
    ar_in = nc.dram_tensor("ar_in", [D1, 2], f32)
    ar_out = nc.dram_tensor("ar_out", [NCORES * D1, 2], f32, addr_space="Shared")
    pl_in = nc.dram_tensor("pl_in", [D1, G], bf16)
    pl_out = nc.dram_tensor("pl_out", [NCORES * D1, G], bf16, addr_space="Shared")

    rg = [list(range(NCORES))]
    QsD3 = QsD[:, :].rearrange("(p w) f -> p w f", p=128)   # [128, NWING, 128]

    from contextlib import ExitStack
    with TileContext(nc) as tc:
        with ExitStack() as _es:
            cp = _es.enter_context(tc.tile_pool(name="const", bufs=1))
            bigp = _es.enter_context(tc.tile_pool(name="big", bufs=1))
            wp = _es.enter_context(tc.tile_pool(name="work", bufs=3))
            tlp = _es.enter_context(tc.tile_pool(name="tail", bufs=1))
            gp = _es.enter_context(tc.tile_pool(name="gat", bufs=4))
            ep = _es.enter_context(tc.tile_pool(name="ea", bufs=3))
            nlp = _es.enter_context(tc.tile_pool(name="nl", bufs=3))
            ohp = _es.enter_context(tc.tile_pool(name="oh", bufs=13))
            ohgp = _es.enter_context(tc.tile_pool(name="ohg", bufs=5))
            stp = _es.enter_context(tc.tile_pool(name="st", bufs=2))
            sgp = _es.enter_context(tc.tile_pool(name="sgp", bufs=3))
            scp = _es.enter_context(tc.tile_pool(name="scr", bufs=1))
            pp = _es.enter_context(tc.tile_pool(name="pre", bufs=3, space="PSUM"))
            ppB = _es.enter_context(tc.tile_pool(name="psB", bufs=2, space="PSUM"))
            # ---------- constants ----------
            def load_const(t, dram, shape, dtype=f32):
                tt = cp.tile(shape, dtype, tag=t)
                nc.sync.dma_start(out=tt[:], in_=dram)
                return tt

            io128 = load_const("io128", iota128[:, :], [128, 128], bf16)
            io256 = load_const("io256", iota256[:, :], [128, G], bf16)
            idn = load_const("idn", ident[:, :], [128, 128])
            idnb = load_const("idnb", identb[:, :], [128, 128], bf16)
            l0w = load_const("l0w", lin0w[:, :], [NF, D1], bf16)
            l0b = load_const("l0b", lin0b[:, :], [D1, 1])
            wd = load_const("wd", wdst[:, :], [D1, L * 128], bf16)
            ws = load_const("ws", wsrc[:, :], [D1, L * 128], bf16)
            we = load_const("we", wea[:, :], [64, L * 128], bf16)
            gmt = load_const("gmt", bng[:, :], [D1, L])
            bbt = load_const("bbt", bnb[:, :], [D1, L])
            l1w = load_const("l1w", lin1w[:, :], [D1, D2])
            l1b = load_const("l1b", lin1b[:, :], [D2, 1])
            fw = load_const("fw", fcw[:, :], [D2, FC * D2])
            fb = load_const("fb", fcb[:, :], [D2, FC])
            l2w = load_const("l2w", lin2w[:, :], [D2, 1])
            l2b = load_const("l2b", lin2b[:, :], [1, 1])
            dlp = load_const("dlp", dstloc_p[:, :], [128, nchunk])
            rcp = load_const("rcp", rc_p[:, :], [128, nchunk])
            blc = load_const("blc", batchloc[:, :], [128, NWIN])
            rgp = load_const("rgp", rgc_pn[:, :], [128, NWIN])

            # ---------- resident state ----------
            hT_own = bigp.tile([D1, SHARD_P], f32, tag="hown")
            hb_own = bigp.tile([D1, SHARD_P], bf16, tag="hbown")
            aggr_sb = bigp.tile([D1, SHARD_P], bf16, tag="aggr")
            qd_sb = bigp.tile([128, NWIN, 128], bf16, tag="qdsb")
            asb = scp.tile([D1, SHARD_P], f32, tag="asb")

            # ---------- lin0 for ALL nodes (no AllGather for layer 0) ----
            # h1f8 holds relu(x @ lin0_w + b) for all 8 shards (global order),
            # computed redundantly on every core from the replicated xTF.
            HL = SHARD_P // 2
            for s_ in range(NCORES):
                h8s = stp.tile([D1, SHARD_P], f8, tag="h8")
                for hh in range(2):
                    xt = sgp.tile([NF, HL], bf16, tag="qsst")
                    o = s_ * SHARD_P + hh * HL
                    nc.sync.dma_start(out=xt[:], in_=xTF[:, o:o + HL])
                    for j in range(4):
                        sl = slice(j * 480, (j + 1) * 480)
                        ph = ppB.tile([D1, 512], f32, tag="bld")
                        nc.tensor.matmul(out=ph[:, :480], lhsT=l0w[:],
                                         rhs=xt[:, sl], start=True, stop=True)
                        osl = slice(hh * HL + j * 480, hh * HL + (j + 1) * 480)
                        if j % 2 == 0:
                            nc.scalar.activation(
                                out=h8s[:, osl],
                                in_=ph[:, :480], func=AF.Relu, bias=l0b[:],
                                scale=1.0)
                        else:
                            nc.vector.tensor_scalar(
                                out=h8s[:, osl], in0=ph[:, :480],
                                scalar1=l0b[:], scalar2=0.0,
                                op0=OP.add, op1=OP.max)
                nc.sync.dma_start(out=h1f8[s_ * D1:(s_ + 1) * D1, :], in_=h8s[:])

            # own-shard h in f32 from the per-core xT input
            for hh in range(2):
                xt0 = sgp.tile([NF, HL], bf16, tag="qsst")
                nc.sync.dma_start(out=xt0[:], in_=xT[:, hh * HL:(hh + 1) * HL])
                for j in range(4):
                    sl = slice(hh * HL + j * 480, hh * HL + (j + 1) * 480)
                    ph = ppB.tile([D1, 512], f32, tag="bld")
                    nc.tensor.matmul(out=ph[:, :480], lhsT=l0w[:],
                                     rhs=xt0[:, j * 480:(j + 1) * 480],
                                     start=True, stop=True)
                    nc.scalar.activation(out=hT_own[:, sl], in_=ph[:, :480],
                                         func=AF.Relu, bias=l0b[:], scale=1.0)
                    nc.vector.tensor_scalar(
                        out=hb_own[:, sl], in0=ph[:, :480],
                        scalar1=l0b[:], scalar2=0.0, op0=OP.add, op1=OP.max)

            # edge-phase constants: emitted after lin0 so their ~25us of
            # DMA (60KB/partition one-hot + gather indices) doesn't serialize
            # ahead of the xTF streams in the SP/DMA queues
            qsix = load_const("qsix", qs_idxD[:, :], [128, etot // 16], i16)
            ohT_res = cp.tile([128, nchunk, 128], f8, tag="ohres")
            nc.sync.dma_start(
                out=ohT_res[:].rearrange("p a b -> p (a b)"), in_=ohTD[:, :])

            # ---------- layers ----------
            for l in range(L):
                wd_l = wd[:, l * 128:(l + 1) * 128]
                ws_l = ws[:, l * 128:(l + 1) * 128]
                we_l = we[:, l * 128:(l + 1) * 128]

                if l == 0:
                    src_dram = h1f8
                else:
                    # --- AllGather h (fp8, staged into ag_in at layer end) ---
                    nc.gpsimd.collective_compute(
                        "AllGather", OP.bypass, replica_groups=rg,
                        ins=[ag_in.ap().opt()], outs=[ag_out.ap().opt()])
                    src_dram = ag_out

                # --- Qd table build (own shard) ---
                for w0 in range(0, NWIN, 4):
                    kk = min(4, NWIN - w0)
                    qp = ppB.tile([128, 512], f32, tag="bld")
                    for k in range(kk):
                        w = w0 + k
                        nc.tensor.matmul(
                            out=qp[:, k * 128:(k + 1) * 128],
                            lhsT=hb_own[:, w * 128:(w + 1) * 128],
                            rhs=wd_l, start=True, stop=True)
                    nc.vector.tensor_copy(
                        out=qd_sb[:, w0:w0 + kk, :].rearrange("p a b -> p (a b)"),
                        in_=qp[:, :kk * 128])

                # --- Qs table build (all nodes, per gathered shard) -> QsD ---
                ws8 = stp.tile([D1, 128], f8, tag="ws8")
                nc.scalar.activation(out=ws8[:], in_=ws_l,
                                     func=AF.Identity, scale=1.0)
                ncopy = 0
                for s_ in range(NCORES):
                    hb_sh = stp.tile([D1, SHARD_P], f8, tag="h8")
                    nc.sync.dma_start(out=hb_sh[:],
                                      in_=src_dram[s_ * D1:(s_ + 1) * D1, :])
                    for wB in range(0, NWIN, 16):
                        kB = min(16, NWIN - wB)
                        sg_t = sgp.tile([128, 16, 128], bf16, tag="qsst")
                        for w0 in range(wB, wB + kB, 4):
                            kk = min(4, wB + kB - w0)
                            qp = ppB.tile([128, 512], f32, tag="bld")
                            for k in range(kk):
                                w = w0 + k
                                nc.tensor.matmul(
                                    out=qp[:, k * 128:(k + 1) * 128],
                                    lhsT=hb_sh[:, w * 128:(w + 1) * 128],
                                    rhs=ws8[:], start=True, stop=True)
                            dst_ap = sg_t[:, w0 - wB:w0 - wB + kk, :] \
                                .rearrange("p a b -> p (a b)")
                            eng = ncopy % 5
                            ncopy += 1
                            if eng in (0, 2, 4):
                                nc.vector.tensor_copy(
                                    out=dst_ap, in_=qp[:, :kk * 128])
                            else:
                                nc.scalar.activation(
                                    out=dst_ap, in_=qp[:, :kk * 128],
                                    func=AF.Identity, scale=1.0)
                        W0 = s_ * NWIN + wB
                        nc.sync.dma_start(out=QsD3[:, W0:W0 + kB, :],
                                          in_=sg_t[:, :kB, :])

                # --- edge pipeline ---
                st1g = wp.tile([D1, 8], f32, tag="st1g")
                st2g = wp.tile([D1, 8], f32, tag="st2g")
                agg = None
                qs_g = None
                aggst = {"agg": None}

                def emit_agg(m, ohs_t, t, te):
                    # aggregation for tile t, deferred one tile so the PE
                    # queue never stalls waiting for m
                    for c in range(te):
                        gc = t * 8 + c
                        w = gc // cw
                        if gc % (4 * cw) == 0:
                            agg_new = ppB.tile([D1, 512], f32, tag="bld")
                            aggst["agg"] = agg_new
                        agg = aggst["agg"]
                        nc.tensor.matmul(
                            out=agg[:, (w % 4) * 128:(w % 4 + 1) * 128],
                            lhsT=m[:, c, :], rhs=ohs_t[c][:],
                            start=(gc % cw == 0), stop=(gc % cw == cw - 1))
                        if gc % (4 * cw) == 4 * cw - 1 or gc == nchunk - 1:
                            grp = w // 4
                            lo = grp * 512
                            hi = min(lo + 512, SHARD_P)
                            nc.scalar.activation(
                                out=aggr_sb[:, lo:hi], in_=agg[:, :hi - lo],
                                func=AF.Identity, scale=1.0)
                            nc.vector.reduce_sum(
                                out=st1g[:, grp:grp + 1],
                                in_=aggr_sb[:, lo:hi],
                                axis=mybir.AxisListType.X)
                            sqg = nlp.tile([D1, 512], bf16, tag="sqg")
                            nc.vector.tensor_tensor(
                                out=sqg[:, :hi - lo], in0=aggr_sb[:, lo:hi],
                                in1=aggr_sb[:, lo:hi], op=OP.mult)
                            nc.vector.reduce_sum(
                                out=st2g[:, grp:grp + 1],
                                in_=sqg[:, :hi - lo],
                                axis=mybir.AxisListType.X)

                pend = []
                for t in range(ntile):
                    te = min(8, nchunk - t * 8)          # chunks this tile
                    ne = te * 128                        # edges this tile
                    if t % 4 == 0:
                        tc32 = min(32, nchunk - t * 8)
                        et = ep.tile([64, 4096], f8, tag="et")
                        nc.sync.dma_start(
                            out=et[:, :tc32 * 128],
                            in_=eaT[:, t * 1024: t * 1024 + tc32 * 128])
                    qs_g = gp.tile([128, 8, 128], bf16, tag="qsg")
                    nc.gpsimd.dma_gather(
                        qs_g[:, :te, :], QsD[:, :],
                        qsix[:, t * 64: t * 64 + te * 8],
                        te * 128, te * 128, 128)
                    half = 0
                    qhalf = (t % 4) * 8

                    # one-hot aggregation matrices: const-only deps, built
                    # ahead so the agg matmuls never wait on DVE
                    ohs_t = []
                    for c in range(te):
                        gc = t * 8 + c
                        oh_ = ohp.tile([128, 128], bf16, tag="ohS")
                        nc.vector.tensor_scalar(
                            out=oh_[:], in0=io128[:],
                            scalar1=dlp[:, gc:gc + 1], scalar2=rcp[:, gc:gc + 1],
                            op0=OP.is_equal, op1=OP.mult)
                        ohs_t.append(oh_)

                    pre = pp.tile([128, 1024], f32, tag="pre")
                    qs_f = qs_g[:].rearrange("p a b -> p (a b)")
                    for c in range(te):
                        gc = t * 8 + c
                        w = gc // cw
                        csl = slice(c * 128, (c + 1) * 128)
                        csl2 = slice((half + c) * 128, (half + c + 1) * 128)
                        csl4 = slice((qhalf + c) * 128, (qhalf + c + 1) * 128)
                        nc.tensor.matmul(out=pre[:, csl], lhsT=et[:, csl4],
                                         rhs=we_l, start=True, stop=False)
                        nc.tensor.matmul(out=pre[:, csl], lhsT=idnb[:],
                                         rhs=qs_f[:, csl2], start=False, stop=False)
                        nc.tensor.matmul(out=pre[:, csl], lhsT=ohT_res[:, gc, :],
                                         rhs=qd_sb[:, w, :], start=False, stop=True)

                    # nonlinearity: m = (1+tanh(a/2)) * softplus(b)
                    #             = 2*sigmoid(a)*softplus(b)  (2 absorbed by BN)
                    # nonlinearity: u = [e^-a | e^b], v = ln(1+u) = [sp(-a)|sp(b)]
                    # even tiles (ACT): sigma = e^-sp(-a); odd tiles (DVE):
                    # sigma = 1/(1+e^-a). m = 2*sigma*sp(b) (2 absorbed by BN
                    # via 4*EPS).
                    uf = nlp.tile([128, 8, 128], bf16, tag="uf")
                    nc.scalar.activation(
                        out=uf[:, :te, :].rearrange("p a b -> p (a b)"),
                        in_=pre[:, :ne], func=AF.Exp, scale=1.0)
                    m = nlp.tile([128, 8, 64], bf16, tag="m")
                    if t % 3 == 0:
                        vf = nlp.tile([128, 8, 128], bf16, tag="vf")
                        nc.scalar.activation(
                            out=vf[:, :te, :].rearrange("p a b -> p (a b)"),
                            in_=uf[:, :te, :].rearrange("p a b -> p (a b)"),
                            func=AF.Ln, bias=1.0, scale=1.0)
                        sg = nlp.tile([128, 8, 64], bf16, tag="sg")
                        nc.scalar.activation(out=sg[:, :te, :],
                                             in_=vf[:, :te, 0:64],
                                             func=AF.Exp, scale=-1.0)
                        nc.vector.scalar_tensor_tensor(
                            out=m[:, :te, :], in0=sg[:, :te, :], scalar=2.0,
                            in1=vf[:, :te, 64:128], op0=OP.mult, op1=OP.mult)
                    else:
                        vs = nlp.tile([128, 8, 64], bf16, tag="vs")
                        nc.scalar.activation(out=vs[:, :te, :],
                                             in_=uf[:, :te, 64:128],
                                             func=AF.Ln, bias=1.0, scale=1.0)
                        w1 = nlp.tile([128, 8, 64], bf16, tag="sg")
                        with nc.allow_low_precision(reason="sigmoid in bf16"):
                            nc.vector.tensor_scalar(out=w1[:, :te, :],
                                                    in0=uf[:, :te, 0:64],
                                                    scalar1=1.0, scalar2=None,
                                                    op0=OP.add)
                            nc.vector.reciprocal(out=w1[:, :te, :],
                                                 in_=w1[:, :te, :])
                        nc.vector.scalar_tensor_tensor(
                            out=m[:, :te, :], in0=w1[:, :te, :], scalar=2.0,
                            in1=vs[:, :te, :], op0=OP.mult, op1=OP.mult)

                    pend.append((m, ohs_t, t, te))
                    if len(pend) > 1:
                        emit_agg(*pend.pop(0))
                while pend:
                    emit_agg(*pend.pop(0))

                # --- BN stats + AllReduce ---
                st = wp.tile([D1, 2], f32, tag="st")
                nc.vector.reduce_sum(out=st[:, 0:1], in_=st1g[:],
                                     axis=mybir.AxisListType.X)
                nc.vector.reduce_sum(out=st[:, 1:2], in_=st2g[:],
                                     axis=mybir.AxisListType.X)
                nc.sync.dma_start(out=ar_in[:, :], in_=st[:])
                nc.gpsimd.collective_compute(
                    "AllGather", OP.bypass, replica_groups=rg,
                    ins=[ar_in.ap().opt()], outs=[ar_out.ap().opt()])
                stga = wp.tile([D1, 2, NCORES], f32, tag="stga")
                nc.sync.dma_start(
                    out=stga[:],
                    in_=ar_out[:, :].rearrange("(c p) s -> p s c", p=D1))
                stg = wp.tile([D1, 2], f32, tag="stg")
                nc.vector.reduce_sum(
                    out=stg[:].rearrange("p (s o) -> p s o", o=1),
                    in_=stga[:], axis=mybir.AxisListType.X)
                mu = wp.tile([D1, 1], f32, tag="mu")
                nc.vector.tensor_scalar(out=mu[:], in0=stg[:, 0:1],
                                        scalar1=1.0 / N, scalar2=None, op0=OP.mult)
                ex2 = wp.tile([D1, 1], f32, tag="ex2")
                nc.vector.tensor_scalar(out=ex2[:], in0=stg[:, 1:2],
                                        scalar1=1.0 / N, scalar2=None, op0=OP.mult)
                mu2 = wp.tile([D1, 1], f32, tag="mu2")
                nc.vector.tensor_tensor(out=mu2[:], in0=mu[:], in1=mu[:], op=OP.mult)
                var = wp.tile([D1, 1], f32, tag="var")
                nc.vector.tensor_tensor(out=var[:], in0=ex2[:], in1=mu2[:],
                                        op=OP.subtract)
                ve = wp.tile([D1, 1], f32, tag="ve")
                # m carries a factor 2 -> aggr/mu scale by 2, var by 4; using
                # 4*EPS makes BN output exactly match the reference.
                nc.vector.tensor_scalar(out=ve[:], in0=var[:], scalar1=4.0 * EPS,
                                        scalar2=None, op0=OP.add)
                lv = wp.tile([D1, 1], f32, tag="lv")
                nc.scalar.activation(out=lv[:], in_=ve[:], func=AF.Ln, scale=1.0)
                isd = wp.tile([D1, 1], f32, tag="isd")
                nc.scalar.activation(out=isd[:], in_=lv[:], func=AF.Exp, scale=-0.5)
                scale = wp.tile([D1, 1], f32, tag="scale")
                nc.vector.tensor_tensor(out=scale[:], in0=isd[:],
                                        in1=gmt[:, l:l + 1], op=OP.mult)
                mshift = wp.tile([D1, 1], f32, tag="mshift")
                nc.vector.tensor_tensor(out=mshift[:], in0=mu[:], in1=scale[:],
                                        op=OP.mult)
                shift = wp.tile([D1, 1], f32, tag="shift")
                nc.vector.tensor_tensor(out=shift[:], in0=bbt[:, l:l + 1],
                                        in1=mshift[:], op=OP.subtract)
                # h = relu((aggr*scale + h) + shift); the three consumers
                # (f32 residual, f8 AllGather payload, bf16 matmul copy) are
                # produced from asb concurrently on DVE/ACT/Pool
                HB = SHARD_P // 2
                for hh in range(2):
                    hsl = slice(hh * HB, (hh + 1) * HB)
                    nc.vector.scalar_tensor_tensor(
                        out=asb[:, hsl], in0=aggr_sb[:, hsl], scalar=scale[:],
                        in1=hT_own[:, hsl], op0=OP.mult, op1=OP.add)
                if l + 1 < L:
                    h8n = stp.tile([D1, SHARD_P], f8, tag="h8")
                    for hh in range(2):
                        hsl = slice(hh * HB, (hh + 1) * HB)
                        nc.scalar.activation(out=h8n[:, hsl], in_=asb[:, hsl],
                                             func=AF.Relu, bias=shift[:],
                                             scale=1.0)
                        nc.sync.dma_start(out=ag_in[:, hsl], in_=h8n[:, hsl])
                nc.vector.tensor_scalar(out=hT_own[:], in0=asb[:],
                                        scalar1=shift[:], scalar2=0.0,
                                        op0=OP.add, op1=OP.max)
                nc.gpsimd.tensor_copy(out=hb_own[:], in_=hT_own[:])

            # ---------- global mean pool ----------
            pool_ps = pp.tile([D1, G], f32, tag="pre")
            for w in range(NWIN):
                tp = ppB.tile([128, D1], f32, tag="bld")
                nc.tensor.transpose(out=tp[:], in_=hT_own[:, w * 128:(w + 1) * 128],
                                    identity=idn[0:D1, 0:D1])
                rows = wp.tile([128, D1], bf16, tag="rows")
                nc.vector.tensor_copy(out=rows[:], in_=tp[:])
                ohg = ohgp.tile([128, G], bf16, tag="ohg")
                nc.vector.tensor_scalar(
                    out=ohg[:], in0=io256[:],
                    scalar1=blc[:, w:w + 1], scalar2=rgp[:, w:w + 1],
                    op0=OP.is_equal, op1=OP.mult)
                nc.tensor.matmul(out=pool_ps[:], lhsT=rows[:], rhs=ohg[:],
                                 start=(w == 0), stop=(w == NWIN - 1))
            poolT = tlp.tile([D1, G], bf16, tag="poolT")
            nc.vector.tensor_copy(out=poolT[:], in_=pool_ps[:])
            nc.sync.dma_start(out=pl_in[:, :], in_=poolT[:])
            nc.gpsimd.collective_compute(
                "AllGather", OP.bypass, replica_groups=rg,
                ins=[pl_in.ap().opt()], outs=[pl_out.ap().opt()])
            pga = tlp.tile([D1, NCORES, G], bf16, tag="pga")
            nc.sync.dma_start(
                out=pga[:],
                in_=pl_out[:, :].rearrange("(c p) g -> p c g", p=D1))
            pg = tlp.tile([D1, G], f32, tag="pg")
            nc.vector.reduce_sum(
                out=pg[:].rearrange("p (g o) -> p g o", o=1),
                in_=pga[:].rearrange("p c g -> p g c"),
                axis=mybir.AxisListType.X)

            # ---------- head ----------
            a = pg
            hw_ = [(l1w[:], l1b[:]), (fw[:, 0:D2], fb[:, 0:1]), (fw[:, D2:2 * D2], fb[:, 1:2])]
            for (wt, bt) in hw_:
                ps = ppB.tile([D2, G], f32, tag="bld")
                nc.tensor.matmul(out=ps[:, 0:G], lhsT=wt, rhs=a[:], start=True, stop=True)
                an = tlp.tile([D2, G], f32, tag="an")
                nc.scalar.activation(out=an[:], in_=ps[:, 0:G], func=AF.Relu,
                                     bias=bt, scale=1.0)
                a = an
            ps = ppB.tile([1, G], f32, tag="bld")
            nc.tensor.matmul(out=ps[:, 0:G], lhsT=l2w[:], rhs=a[:], start=True, stop=True)
            yt = tlp.tile([1, G], f32, tag="yt")
            nc.scalar.activation(out=yt[:], in_=ps[:, 0:G], func=AF.Identity,
                                 bias=l2b[:], scale=1.0)
            nc.sync.dma_start(out=yout[:, :], in_=yt[:])

    nc.compile()
    return nc


def _wrap16(idx):
    """Flat idx list -> [128, n/16] int16: slot i at [i%16, i//16], replicated
    across the 8 Q7 cores."""
    a = idx.reshape(-1, 16).T.astype(np.int16)
    return np.tile(a, (8, 1))


def _preprocess(inputs):
    x = np.asarray(inputs["x"], np.float32)
    ea = np.asarray(inputs["edge_attr"], np.float32)
    ei = np.asarray(inputs["edge_index"]).astype(np.int64)
    batch = np.asarray(inputs["batch"]).astype(np.int64)
    src, dst = ei[0], ei[1]

    cnt = np.bincount(dst, minlength=N).astype(np.float32)
    rc_node = 1.0 / np.maximum(cnt, 1.0)
    gcnt = np.bincount(batch, minlength=G).astype(np.float32)
    rgc = 1.0 / np.maximum(gcnt, 1.0)

    # Degree-balanced node -> (window, slot) assignment: snake-deal nodes in
    # descending-degree order across the 240 global windows, minimizing the
    # max per-window edge count (which sets the uniform chunk pad cw).
    deg_order = np.argsort(-cnt, kind="stable")       # node ids, deg desc
    nwin_g = NCORES * NWIN                            # 240
    perm_loc = np.empty(N, np.int64)                  # node -> global padded id
    for i0 in range(0, N, nwin_g):
        blk = deg_order[i0:i0 + nwin_g]
        j = i0 // nwin_g
        wins = np.arange(len(blk)) if j % 2 == 0 else (len(blk) - 1 - np.arange(len(blk)))
        w_ids = wins
        perm_loc[blk] = (w_ids // NWIN) * SHARD_P + (w_ids % NWIN) * 128 + j
    gperm = perm_loc
    srcg = gperm[src]
    dstg = gperm[dst]
    order = np.argsort(dstg, kind="stable")
    srcg_s, dstg_s, ea_idx = srcg[order], dstg[order], order

    bounds = []
    for c in range(NCORES):
        for w in range(NWIN):
            bounds.append(c * SHARD_P + w * 128)
    bounds.append(NCORES * SHARD_P)
    bpos = np.searchsorted(dstg_s, np.asarray(bounds), side="left")
    percw = {}
    maxcnt = 0
    k = 0
    for c in range(NCORES):
        for w in range(NWIN):
            lo, hi = bpos[k], bpos[k + 1]
            percw[(c, w)] = np.arange(lo, hi)
            maxcnt = max(maxcnt, hi - lo)
            k += 1
    cw = max(1, (maxcnt + 127) // 128)
    etot = NWIN * cw * 128

    # full padded x, rotated per core so block 0 is the own shard
    xfull = np.zeros((NF, NCORES * SHARD_P), np.float32)
    xfull[:, gperm] = x.T
    xfull = xfull.astype(ml_dtypes.bfloat16)

    per_core = []
    for c in range(NCORES):
        qs_idx = np.zeros(etot, np.int64)
        dl = np.full(etot, -1.0, np.float32)
        rc_e = np.ones(etot, np.float32)
        ea_e = np.zeros((etot, EF), np.float32)
        for w in range(NWIN):
            idxs = percw[(c, w)]
            o = w * cw * 128
            k = len(idxs)
            g = srcg_s[idxs]                           # padded global id
            qs_idx[o:o + k] = (g % 128) * NWING + (g // 128)
            loc = dstg_s[idxs] - c * SHARD_P           # 0..3839
            dl[o:o + k] = (loc - w * 128).astype(np.float32)
            rc_e[o:o + k] = rc_node[dst[ea_idx[idxs]]]
            ea_e[o:o + k] = ea[ea_idx[idxs]]
        eaT = np.zeros((64, etot), np.float32)
        eaT[1:EF + 1] = ea_e.T
        eaT[EF + 1] = 1.0
        eaT[EF + 1, dl < 0] = 0.0
        nch = etot // 128
        ohT = np.zeros((128, etot), np.float32)
        vv = dl >= 0
        ohT[dl[vv].astype(np.int64), np.nonzero(vv)[0]] = 1.0
        d = {
            "qs_idxD": _wrap16(qs_idx),
            "ohTD": ohT.astype(ml_dtypes.float8_e4m3),
            "dstloc_p": dl.reshape(nch, 128).T.copy(),
            "rc_p": rc_e.reshape(nch, 128).T.copy(),
            "eaT": eaT.astype(ml_dtypes.float8_e4m3),
        }
        d["xTF"] = xfull
        d["xT"] = xfull[:, c * SHARD_P:(c + 1) * SHARD_P].copy()
        nodes_c = np.nonzero((gperm // SHARD_P) == c)[0]
        locs_c = gperm[nodes_c] - c * SHARD_P
        bl = np.full(SHARD_P, -1.0, np.float32)
        bl[locs_c] = batch[nodes_c].astype(np.float32)
        rg_n = np.zeros(SHARD_P, np.float32)
        rg_n[locs_c] = rgc[batch[nodes_c]]
        d["batchloc"] = bl.reshape(NWIN, 128).T.copy()
        d["rgc_pn"] = rg_n.reshape(NWIN, 128).T.copy()
        per_core.append(d)

    # replicated weights; f-gate halves pre-negated
    wf = np.asarray(inputs["conv_wf"], np.float32)
    wsv = np.asarray(inputs["conv_ws"], np.float32)
    bf = np.asarray(inputs["conv_bf"], np.float32)
    bs = np.asarray(inputs["conv_bs"], np.float32)
    wdst = np.concatenate([-wf[:, 0:D1, :], wsv[:, 0:D1, :]], axis=2)
    wsrc = np.concatenate([-wf[:, D1:2 * D1, :], wsv[:, D1:2 * D1, :]], axis=2)
    wea = np.concatenate([-wf[:, 2 * D1:, :], wsv[:, 2 * D1:, :]], axis=2)
    bias = np.concatenate([-bf, bs], axis=1)[:, None, :]
    wea = np.concatenate([wea, bias], axis=1)
    shared = {
        "lin0w": np.asarray(inputs["lin0_w"], np.float32).astype(ml_dtypes.bfloat16),
        "lin0b": np.asarray(inputs["lin0_b"], np.float32).reshape(D1, 1),
        "wdst": np.transpose(wdst, (1, 0, 2)).reshape(D1, L * 128).astype(ml_dtypes.bfloat16),
        "wsrc": np.transpose(wsrc, (1, 0, 2)).reshape(D1, L * 128).astype(ml_dtypes.bfloat16),
        "wea": np.concatenate([
            np.zeros((1, L * 128), np.float32),
            np.transpose(wea, (1, 0, 2)).reshape(EF + 1, L * 128),
            np.zeros((64 - EF - 2, L * 128), np.float32),
        ], axis=0).astype(ml_dtypes.bfloat16),
        "bng": np.asarray(inputs["bn_gamma"], np.float32).T.copy(),
        "bnb": np.asarray(inputs["bn_beta"], np.float32).T.copy(),
        "lin1w": np.asarray(inputs["lin1_w"], np.float32),
        "lin1b": np.asarray(inputs["lin1_b"], np.float32).reshape(D2, 1),
        "fcw": np.transpose(np.asarray(inputs["fc_w"], np.float32), (1, 0, 2)).reshape(D2, FC * D2),
        "fcb": np.asarray(inputs["fc_b"], np.float32).T.copy(),
        "lin2w": np.asarray(inputs["lin2_w"], np.float32).reshape(D2, 1),
        "lin2b": np.asarray(inputs["lin2_b"], np.float32).reshape(1, 1),
        "iota128": np.broadcast_to(np.arange(128, dtype=np.float32)[None, :],
                                   (128, 128)).astype(ml_dtypes.bfloat16),
        "iota256": np.broadcast_to(np.arange(G, dtype=np.float32)[None, :],
                                   (128, G)).astype(ml_dtypes.bfloat16),
        "ident": np.eye(128, dtype=np.float32),
        "identb": np.eye(128, dtype=np.float32).astype(ml_dtypes.bfloat16),
    }
    in_maps = [dict(shared, **pc) for pc in per_core]
    return in_maps, cw


def kernel(**inputs):
    from concourse.bass_utils import run_bass_kernel_spmd

    in_maps, cw = _preprocess(inputs)
    key = ("nc", cw)
    if key not in _CACHE:
        _CACHE[key] = _build_nc(cw)
    nc = _CACHE[key]
    res = run_bass_kernel_spmd(nc, in_maps, core_ids=list(range(NCORES)))
    return res.results[0]["y"].reshape(G).astype(np.float32)


# revision 56
# speedup vs baseline: 1.0126x; 1.0005x over previous
"""CGCNN message-passing kernel for 8 Trainium2 NeuronCores (Bass/Tile), v9.

Data-parallel by dst shard; gather-based edge pipeline:
- Host: nodes are dealt into 240 global windows (8 cores x 30 windows x 128
  slots) in descending-degree snake order, equalizing per-window edge counts
  so the uniform chunks-per-window pad cw is minimal (16). Edges go to the
  core owning their dst, grouped by dst window, chunk-padded to cw.
- lin0 is computed for ALL nodes redundantly on every core (from a replicated
  full xT) into a DRAM fp8 table, so layer 0 needs no h AllGather; layers
  1..L-1 AllGather h in fp8 (staged at the previous layer's BN boundary so
  the collective launches as early as possible).
- Per layer, per core:
  * Qd table (own shard, SBUF bf16 [128, 30, 128]) = h_own @ Wdst.
  * Full Qs table = h_full @ Wsrc into DRAM [30720, 128] bf16 in
    partition-major row order (node g -> row (g%128)*240 + g//128); PSUM->SBUF
    staging copies round-robin over DVE/ACT to minimize build latency.
  * Per 1024-edge tile: one dma_gather pulls per-edge Qs rows (1024 x 256B
    descriptors; 1024 = SWDGE ring capacity); Qe = ea(fp8) @ Wea by matmul
    (edge attrs streamed fp8, 4 tiles per DMA); the dst contribution expands
    via a host-precomputed one-hot (fp8, SBUF-resident, layer-invariant)
    matmul against the SBUF Qd table. All three accumulate in PSUM per
    128-edge chunk.
  * Nonlinearity: joint exp u = [e^-a | e^b] (f-gate weights pre-negated),
    v = ln(1+u); 1/3 of tiles compute sigmoid(a) = e^-v_f on ACT, 2/3 as
    1/(1+u_f) on DVE (bf16, engine balance). m = 2*sigmoid(a)*softplus(b);
    the factor 2 is absorbed exactly by BatchNorm using 4*EPS.
  * Aggregation one-hots (is_equal(iota, dst) * 1/cnt, bf16 on DVE) are
    pre-built per tile, and the aggregation matmuls are deferred by one tile
    so the in-order PE queue never stalls on the ACT/DVE nonlinearity chain.
  * Segment-mean accumulates per dst window in PSUM (agg PSUM shares banks
    with the build-phase staging, freeing a third pre-PSUM buffer); BatchNorm
    batch stats via a tiny stats AllGather + local sum; the residual
    (scalar_tensor_tensor + relu) is computed in halves, with the fp8
    AllGather payload produced on ACT in parallel with the f32/bf16 h copies
    on DVE/Pool.
- Global mean pool via one-hot matmul, bf16 partials AllGathered and summed
  locally, head MLP computed redundantly on every core.
"""
import numpy as np
import ml_dtypes

N = 30000
E = 480000
NF = 92
EF = 50
D1 = 64
D2 = 64
L = 3
FC = 2
G = 256
EPS = 1e-5
NCORES = 8
SHARD = N // NCORES            # 3750
SHARD_P = 3840                 # padded shard (30 windows of 128)
NWIN = SHARD_P // 128          # 30
NWING = NCORES * NWIN          # 240 global windows
TBL = NCORES * SHARD_P         # 30720 table rows

_CACHE = {}



def _build_nc(cw):
    """Build the SPMD bass module. cw = chunks per dst window (uniform)."""
    import concourse.mybir as mybir
    from concourse import bacc
    from concourse.tile import TileContext

    f32 = mybir.dt.float32
    bf16 = mybir.dt.bfloat16
    f8 = mybir.dt.float8e4
    i16 = mybir.dt.int16
    AF = mybir.ActivationFunctionType
    OP = mybir.AluOpType

    nchunk = NWIN * cw                 # chunks per layer per core
    etot = nchunk * 128                # padded edges per core
    ntile = (nchunk + 7) // 8          # 8-chunk (1024-edge) PSUM tiles

    import concourse.hw_specs as _hw
    import concourse.bacc as _bacc_mod
    _real_tables = _hw.get_activation_tables("gen3")
    _combined = "natural_log_exp_and_others"
    if _combined in _real_tables:
        _patched = {
            k: (v if k == _combined else (v - {AF.Exp, AF.Ln}))
            for k, v in _real_tables.items()
        }
        _bacc_mod.get_activation_tables = lambda arch: _patched

    nc = bacc.Bacc(None, target_bir_lowering=False)

    # ---- inputs (per core) ----
    xTF = nc.dram_tensor("xTF", [NF, TBL], bf16, kind="ExternalInput")
    xT = nc.dram_tensor("xT", [NF, SHARD_P], bf16, kind="ExternalInput")
    eaT = nc.dram_tensor("eaT", [64, etot], f8, kind="ExternalInput")
    qs_idxD = nc.dram_tensor("qs_idxD", [128, etot // 16], i16, kind="ExternalInput")
    ohTD = nc.dram_tensor("ohTD", [128, etot], f8, kind="ExternalInput")
    dstloc_p = nc.dram_tensor("dstloc_p", [128, nchunk], f32, kind="ExternalInput")
    rc_p = nc.dram_tensor("rc_p", [128, nchunk], f32, kind="ExternalInput")
    batchloc = nc.dram_tensor("batchloc", [128, NWIN], f32, kind="ExternalInput")
    rgc_pn = nc.dram_tensor("rgc_pn", [128, NWIN], f32, kind="ExternalInput")
    # weights (replicated; f-gate halves pre-negated)
    lin0w = nc.dram_tensor("lin0w", [NF, D1], bf16, kind="ExternalInput")
    lin0b = nc.dram_tensor("lin0b", [D1, 1], f32, kind="ExternalInput")
    wdst = nc.dram_tensor("wdst", [D1, L * 128], bf16, kind="ExternalInput")
    wsrc = nc.dram_tensor("wsrc", [D1, L * 128], bf16, kind="ExternalInput")
    wea = nc.dram_tensor("wea", [64, L * 128], bf16, kind="ExternalInput")
    bng = nc.dram_tensor("bng", [D1, L], f32, kind="ExternalInput")
    bnb = nc.dram_tensor("bnb", [D1, L], f32, kind="ExternalInput")
    lin1w = nc.dram_tensor("lin1w", [D1, D2], f32, kind="ExternalInput")
    lin1b = nc.dram_tensor("lin1b", [D2, 1], f32, kind="ExternalInput")
    fcw = nc.dram_tensor("fcw", [D2, FC * D2], f32, kind="ExternalInput")
    fcb = nc.dram_tensor("fcb", [D2, FC], f32, kind="ExternalInput")
    lin2w = nc.dram_tensor("lin2w", [D2, 1], f32, kind="ExternalInput")
    lin2b = nc.dram_tensor("lin2b", [1, 1], f32, kind="ExternalInput")
    iota128 = nc.dram_tensor("iota128", [128, 128], bf16, kind="ExternalInput")
    iota256 = nc.dram_tensor("iota256", [128, G], bf16, kind="ExternalInput")
    ident = nc.dram_tensor("ident", [128, 128], f32, kind="ExternalInput")
    identb = nc.dram_tensor("identb", [128, 128], bf16, kind="ExternalInput")

    yout = nc.dram_tensor("y", [1, G], f32, kind="ExternalOutput")

    # ---- DRAM scratch ----
    QsD = nc.dram_tensor("QsD", [TBL, 128], bf16)          # row p*NWING+W
    h1f8 = nc.dram_tensor("h1f8", [NCORES * D1, SHARD_P], f8)
    ag_in = nc.dram_tensor("ag_in", [D1, SHARD_P], f8)
    ag_out = nc.dram_tensor("ag_out", [NCORES * D1, SHARD_P], f8,
                            addr_space="Shared")
    ar_in = nc.dram_tensor("ar_in", [D1, 2], f32)
    ar_out = nc.dram_tensor("ar_out", [NCORES * D1, 2], f32, addr_space="Shared")
    pl_in = nc.dram_tensor("pl_in", [D1, G], bf16)
    pl_out = nc.dram_tensor("pl_out", [NCORES * D1, G], bf16, addr_space="Shared")

    rg = [list(range(NCORES))]
    QsD3 = QsD[:, :].rearrange("(p w) f -> p w f", p=128)   # [128, NWING, 128]

    from contextlib import ExitStack
    with TileContext(nc) as tc:
        with ExitStack() as _es:
            cp = _es.enter_context(tc.tile_pool(name="const", bufs=1))
            bigp = _es.enter_context(tc.tile_pool(name="big", bufs=1))
            wp = _es.enter_context(tc.tile_pool(name="work", bufs=3))
            tlp = _es.enter_context(tc.tile_pool(name="tail", bufs=1))
            gp = _es.enter_context(tc.tile_pool(name="gat", bufs=4))
            ep = _es.enter_context(tc.tile_pool(name="ea", bufs=3))
            nlp = _es.enter_context(tc.tile_pool(name="nl", bufs=3))
            ohp = _es.enter_context(tc.tile_pool(name="oh", bufs=13))
            ohgp = _es.enter_context(tc.tile_pool(name="ohg", bufs=5))
            stp = _es.enter_context(tc.tile_pool(name="st", bufs=2))
            sgp = _es.enter_context(tc.tile_pool(name="sgp", bufs=3))
            scp = _es.enter_context(tc.tile_pool(name="scr", bufs=1))
            pp = _es.enter_context(tc.tile_pool(name="pre", bufs=3, space="PSUM"))
            ppB = _es.enter_context(tc.tile_pool(name="psB", bufs=2, space="PSUM"))
            # ---------- constants ----------
            def load_const(t, dram, shape, dtype=f32):
                tt = cp.tile(shape, dtype, tag=t)
                nc.sync.dma_start(out=tt[:], in_=dram)
                return tt

            l0w = load_const("l0w", lin0w[:, :], [NF, D1], bf16)
            l0b = load_const("l0b", lin0b[:, :], [D1, 1])

            # ---------- resident state ----------
            hT_own = bigp.tile([D1, SHARD_P], f32, tag="hown")
            hb_own = bigp.tile([D1, SHARD_P], bf16, tag="hbown")
            aggr_sb = bigp.tile([D1, SHARD_P], bf16, tag="aggr")
            qd_sb = bigp.tile([128, NWIN, 128], bf16, tag="qdsb")
            asb = scp.tile([D1, SHARD_P], f32, tag="asb")

            # ---------- lin0 for ALL nodes (no AllGather for layer 0) ----
            # h1f8 holds relu(x @ lin0_w + b) for all 8 shards (global order),
            # computed redundantly on every core from the replicated xTF.
            HL = SHARD_P // 2
            for s_ in range(NCORES):
                h8s = stp.tile([D1, SHARD_P], f8, tag="h8")
                for hh in range(2):
                    xt = sgp.tile([NF, HL], bf16, tag="qsst")
                    o = s_ * SHARD_P + hh * HL
                    nc.sync.dma_start(out=xt[:], in_=xTF[:, o:o + HL])
                    for j in range(4):
                        sl = slice(j * 480, (j + 1) * 480)
                        ph = ppB.tile([D1, 512], f32, tag="bld")
                        nc.tensor.matmul(out=ph[:, :480], lhsT=l0w[:],
                                         rhs=xt[:, sl], start=True, stop=True)
                        osl = slice(hh * HL + j * 480, hh * HL + (j + 1) * 480)
                        if j % 2 == 0:
                            nc.scalar.activation(
                                out=h8s[:, osl],
                                in_=ph[:, :480], func=AF.Relu, bias=l0b[:],
                                scale=1.0)
                        else:
                            nc.vector.tensor_scalar(
                                out=h8s[:, osl], in0=ph[:, :480],
                                scalar1=l0b[:], scalar2=0.0,
                                op0=OP.add, op1=OP.max)
                nc.sync.dma_start(out=h1f8[s_ * D1:(s_ + 1) * D1, :], in_=h8s[:])

            # own-shard h in f32 from the per-core xT input
            for hh in range(2):
                xt0 = sgp.tile([NF, HL], bf16, tag="qsst")
                nc.sync.dma_start(out=xt0[:], in_=xT[:, hh * HL:(hh + 1) * HL])
                for j in range(4):
                    sl = slice(hh * HL + j * 480, hh * HL + (j + 1) * 480)
                    ph = ppB.tile([D1, 512], f32, tag="bld")
                    nc.tensor.matmul(out=ph[:, :480], lhsT=l0w[:],
                                     rhs=xt0[:, j * 480:(j + 1) * 480],
                                     start=True, stop=True)
                    nc.scalar.activation(out=hT_own[:, sl], in_=ph[:, :480],
                                         func=AF.Relu, bias=l0b[:], scale=1.0)
                    nc.vector.tensor_scalar(
                        out=hb_own[:, sl], in0=ph[:, :480],
                        scalar1=l0b[:], scalar2=0.0, op0=OP.add, op1=OP.max)

            # remaining constants: emitted after lin0 so their DMA (notably
            # the 60KB/partition one-hot + gather indices) doesn't serialize
            # ahead of the xTF streams in the SP/DMA queues
            ws = load_const("ws", wsrc[:, :], [D1, L * 128], bf16)
            wd = load_const("wd", wdst[:, :], [D1, L * 128], bf16)
            we = load_const("we", wea[:, :], [64, L * 128], bf16)
            io128 = load_const("io128", iota128[:, :], [128, 128], bf16)
            idnb = load_const("idnb", identb[:, :], [128, 128], bf16)
            dlp = load_const("dlp", dstloc_p[:, :], [128, nchunk])
            rcp = load_const("rcp", rc_p[:, :], [128, nchunk])
            gmt = load_const("gmt", bng[:, :], [D1, L])
            bbt = load_const("bbt", bnb[:, :], [D1, L])
            io256 = load_const("io256", iota256[:, :], [128, G], bf16)
            idn = load_const("idn", ident[:, :], [128, 128])
            blc = load_const("blc", batchloc[:, :], [128, NWIN])
            rgp = load_const("rgp", rgc_pn[:, :], [128, NWIN])
            l1w = load_const("l1w", lin1w[:, :], [D1, D2])
            l1b = load_const("l1b", lin1b[:, :], [D2, 1])
            fw = load_const("fw", fcw[:, :], [D2, FC * D2])
            fb = load_const("fb", fcb[:, :], [D2, FC])
            l2w = load_const("l2w", lin2w[:, :], [D2, 1])
            l2b = load_const("l2b", lin2b[:, :], [1, 1])
            qsix = load_const("qsix", qs_idxD[:, :], [128, etot // 16], i16)
            ohT_res = cp.tile([128, nchunk, 128], f8, tag="ohres")
            nc.sync.dma_start(
                out=ohT_res[:].rearrange("p a b -> p (a b)"), in_=ohTD[:, :])

            # ---------- layers ----------
            for l in range(L):
                wd_l = wd[:, l * 128:(l + 1) * 128]
                ws_l = ws[:, l * 128:(l + 1) * 128]
                we_l = we[:, l * 128:(l + 1) * 128]

                if l == 0:
                    src_dram = h1f8
                else:
                    # --- AllGather h (fp8, staged into ag_in at layer end) ---
                    nc.gpsimd.collective_compute(
                        "AllGather", OP.bypass, replica_groups=rg,
                        ins=[ag_in.ap().opt()], outs=[ag_out.ap().opt()])
                    src_dram = ag_out

                # --- Qd table build (own shard) ---
                for w0 in range(0, NWIN, 4):
                    kk = min(4, NWIN - w0)
                    qp = ppB.tile([128, 512], f32, tag="bld")
                    for k in range(kk):
                        w = w0 + k
                        nc.tensor.matmul(
                            out=qp[:, k * 128:(k + 1) * 128],
                            lhsT=hb_own[:, w * 128:(w + 1) * 128],
                            rhs=wd_l, start=True, stop=True)
                    nc.vector.tensor_copy(
                        out=qd_sb[:, w0:w0 + kk, :].rearrange("p a b -> p (a b)"),
                        in_=qp[:, :kk * 128])

                # --- Qs table build (all nodes, per gathered shard) -> QsD ---
                ws8 = stp.tile([D1, 128], f8, tag="ws8")
                nc.scalar.activation(out=ws8[:], in_=ws_l,
                                     func=AF.Identity, scale=1.0)
                ncopy = 0
                for s_ in range(NCORES):
                    hb_sh = stp.tile([D1, SHARD_P], f8, tag="h8")
                    nc.sync.dma_start(out=hb_sh[:],
                                      in_=src_dram[s_ * D1:(s_ + 1) * D1, :])
                    for wB in range(0, NWIN, 16):
                        kB = min(16, NWIN - wB)
                        sg_t = sgp.tile([128, 16, 128], bf16, tag="qsst")
                        for w0 in range(wB, wB + kB, 4):
                            kk = min(4, wB + kB - w0)
                            qp = ppB.tile([128, 512], f32, tag="bld")
                            for k in range(kk):
                                w = w0 + k
                                nc.tensor.matmul(
                                    out=qp[:, k * 128:(k + 1) * 128],
                                    lhsT=hb_sh[:, w * 128:(w + 1) * 128],
                                    rhs=ws8[:], start=True, stop=True)
                            dst_ap = sg_t[:, w0 - wB:w0 - wB + kk, :] \
                                .rearrange("p a b -> p (a b)")
                            eng = ncopy % 5
                            ncopy += 1
                            if eng in (0, 2, 4):
                                nc.vector.tensor_copy(
                                    out=dst_ap, in_=qp[:, :kk * 128])
                            else:
                                nc.scalar.activation(
                                    out=dst_ap, in_=qp[:, :kk * 128],
                                    func=AF.Identity, scale=1.0)
                        W0 = s_ * NWIN + wB
                        nc.sync.dma_start(out=QsD3[:, W0:W0 + kB, :],
                                          in_=sg_t[:, :kB, :])

                # --- edge pipeline ---
                st1g = wp.tile([D1, 8], f32, tag="st1g")
                st2g = wp.tile([D1, 8], f32, tag="st2g")
                agg = None
                qs_g = None
                aggst = {"agg": None}

                def emit_agg(m, ohs_t, t, te):
                    # aggregation for tile t, deferred one tile so the PE
                    # queue never stalls waiting for m
                    for c in range(te):
                        gc = t * 8 + c
                        w = gc // cw
                        if gc % (4 * cw) == 0:
                            agg_new = ppB.tile([D1, 512], f32, tag="bld")
                            aggst["agg"] = agg_new
                        agg = aggst["agg"]
                        nc.tensor.matmul(
                            out=agg[:, (w % 4) * 128:(w % 4 + 1) * 128],
                            lhsT=m[:, c, :], rhs=ohs_t[c][:],
                            start=(gc % cw == 0), stop=(gc % cw == cw - 1))
                        if gc % (4 * cw) == 4 * cw - 1 or gc == nchunk - 1:
                            grp = w // 4
                            lo = grp * 512
                            hi = min(lo + 512, SHARD_P)
                            nc.scalar.activation(
                                out=aggr_sb[:, lo:hi], in_=agg[:, :hi - lo],
                                func=AF.Identity, scale=1.0)
                            nc.vector.reduce_sum(
                                out=st1g[:, grp:grp + 1],
                                in_=aggr_sb[:, lo:hi],
                                axis=mybir.AxisListType.X)
                            sqg = nlp.tile([D1, 512], bf16, tag="sqg")
                            nc.vector.tensor_tensor(
                                out=sqg[:, :hi - lo], in0=aggr_sb[:, lo:hi],
                                in1=aggr_sb[:, lo:hi], op=OP.mult)
                            nc.vector.reduce_sum(
                                out=st2g[:, grp:grp + 1],
                                in_=sqg[:, :hi - lo],
                                axis=mybir.AxisListType.X)

                pend = []
                for t in range(ntile):
                    te = min(8, nchunk - t * 8)          # chunks this tile
                    ne = te * 128                        # edges this tile
                    if t % 4 == 0:
                        tc32 = min(32, nchunk - t * 8)
                        et = ep.tile([64, 4096], f8, tag="et")
                        nc.sync.dma_start(
                            out=et[:, :tc32 * 128],
                            in_=eaT[:, t * 1024: t * 1024 + tc32 * 128])
                    qs_g = gp.tile([128, 8, 128], bf16, tag="qsg")
                    nc.gpsimd.dma_gather(
                        qs_g[:, :te, :], QsD[:, :],
                        qsix[:, t * 64: t * 64 + te * 8],
                        te * 128, te * 128, 128)
                    half = 0
                    qhalf = (t % 4) * 8

                    # one-hot aggregation matrices: const-only deps, built
                    # ahead so the agg matmuls never wait on DVE
                    ohs_t = []
                    for c in range(te):
                        gc = t * 8 + c
                        oh_ = ohp.tile([128, 128], bf16, tag="ohS")
                        nc.vector.tensor_scalar(
                            out=oh_[:], in0=io128[:],
                            scalar1=dlp[:, gc:gc + 1], scalar2=rcp[:, gc:gc + 1],
                            op0=OP.is_equal, op1=OP.mult)
                        ohs_t.append(oh_)

                    pre = pp.tile([128, 1024], f32, tag="pre")
                    qs_f = qs_g[:].rearrange("p a b -> p (a b)")
                    for c in range(te):
                        gc = t * 8 + c
                        w = gc // cw
                        csl = slice(c * 128, (c + 1) * 128)
                        csl2 = slice((half + c) * 128, (half + c + 1) * 128)
                        csl4 = slice((qhalf + c) * 128, (qhalf + c + 1) * 128)
                        nc.tensor.matmul(out=pre[:, csl], lhsT=et[:, csl4],
                                         rhs=we_l, start=True, stop=False)
                        nc.tensor.matmul(out=pre[:, csl], lhsT=idnb[:],
                                         rhs=qs_f[:, csl2], start=False, stop=False)
                        nc.tensor.matmul(out=pre[:, csl], lhsT=ohT_res[:, gc, :],
                                         rhs=qd_sb[:, w, :], start=False, stop=True)

                    # nonlinearity: m = (1+tanh(a/2)) * softplus(b)
                    #             = 2*sigmoid(a)*softplus(b)  (2 absorbed by BN)
                    # nonlinearity: u = [e^-a | e^b], v = ln(1+u) = [sp(-a)|sp(b)]
                    # even tiles (ACT): sigma = e^-sp(-a); odd tiles (DVE):
                    # sigma = 1/(1+e^-a). m = 2*sigma*sp(b) (2 absorbed by BN
                    # via 4*EPS).
                    uf = nlp.tile([128, 8, 128], bf16, tag="uf")
                    nc.scalar.activation(
                        out=uf[:, :te, :].rearrange("p a b -> p (a b)"),
                        in_=pre[:, :ne], func=AF.Exp, scale=1.0)
                    m = nlp.tile([128, 8, 64], bf16, tag="m")
                    if t % 3 == 0:
                        vf = nlp.tile([128, 8, 128], bf16, tag="vf")
                        nc.scalar.activation(
                            out=vf[:, :te, :].rearrange("p a b -> p (a b)"),
                            in_=uf[:, :te, :].rearrange("p a b -> p (a b)"),
                            func=AF.Ln, bias=1.0, scale=1.0)
                        sg = nlp.tile([128, 8, 64], bf16, tag="sg")
                        nc.scalar.activation(out=sg[:, :te, :],
                                             in_=vf[:, :te, 0:64],
                                             func=AF.Exp, scale=-1.0)
                        nc.vector.scalar_tensor_tensor(
                            out=m[:, :te, :], in0=sg[:, :te, :], scalar=2.0,
                            in1=vf[:, :te, 64:128], op0=OP.mult, op1=OP.mult)
                    else:
                        vs = nlp.tile([128, 8, 64], bf16, tag="vs")
                        nc.scalar.activation(out=vs[:, :te, :],
                                             in_=uf[:, :te, 64:128],
                                             func=AF.Ln, bias=1.0, scale=1.0)
                        w1 = nlp.tile([128, 8, 64], bf16, tag="sg")
                        with nc.allow_low_precision(reason="sigmoid in bf16"):
                            nc.vector.tensor_scalar(out=w1[:, :te, :],
                                                    in0=uf[:, :te, 0:64],
                                                    scalar1=1.0, scalar2=None,
                                                    op0=OP.add)
                            nc.vector.reciprocal(out=w1[:, :te, :],
                                                 in_=w1[:, :te, :])
                        nc.vector.scalar_tensor_tensor(
                            out=m[:, :te, :], in0=w1[:, :te, :], scalar=2.0,
                            in1=vs[:, :te, :], op0=OP.mult, op1=OP.mult)

                    pend.append((m, ohs_t, t, te))
                    if len(pend) > 1:
                        emit_agg(*pend.pop(0))
                while pend:
                    emit_agg(*pend.pop(0))

                # --- BN stats + AllReduce ---
                st = wp.tile([D1, 2], f32, tag="st")
                nc.vector.reduce_sum(out=st[:, 0:1], in_=st1g[:],
                                     axis=mybir.AxisListType.X)
                nc.vector.reduce_sum(out=st[:, 1:2], in_=st2g[:],
                                     axis=mybir.AxisListType.X)
                nc.sync.dma_start(out=ar_in[:, :], in_=st[:])
                nc.gpsimd.collective_compute(
                    "AllGather", OP.bypass, replica_groups=rg,
                    ins=[ar_in.ap().opt()], outs=[ar_out.ap().opt()])
                stga = wp.tile([D1, 2, NCORES], f32, tag="stga")
                nc.sync.dma_start(
                    out=stga[:],
                    in_=ar_out[:, :].rearrange("(c p) s -> p s c", p=D1))
                stg = wp.tile([D1, 2], f32, tag="stg")
                nc.vector.reduce_sum(
                    out=stg[:].rearrange("p (s o) -> p s o", o=1),
                    in_=stga[:], axis=mybir.AxisListType.X)
                mu = wp.tile([D1, 1], f32, tag="mu")
                nc.vector.tensor_scalar(out=mu[:], in0=stg[:, 0:1],
                                        scalar1=1.0 / N, scalar2=None, op0=OP.mult)
                ex2 = wp.tile([D1, 1], f32, tag="ex2")
                nc.vector.tensor_scalar(out=ex2[:], in0=stg[:, 1:2],
                                        scalar1=1.0 / N, scalar2=None, op0=OP.mult)
                mu2 = wp.tile([D1, 1], f32, tag="mu2")
                nc.vector.tensor_tensor(out=mu2[:], in0=mu[:], in1=mu[:], op=OP.mult)
                var = wp.tile([D1, 1], f32, tag="var")
                nc.vector.tensor_tensor(out=var[:], in0=ex2[:], in1=mu2[:],
                                        op=OP.subtract)
                ve = wp.tile([D1, 1], f32, tag="ve")
                # m carries a factor 2 -> aggr/mu scale by 2, var by 4; using
                # 4*EPS makes BN output exactly match the reference.
                nc.vector.tensor_scalar(out=ve[:], in0=var[:], scalar1=4.0 * EPS,
                                        scalar2=None, op0=OP.add)
                lv = wp.tile([D1, 1], f32, tag="lv")
                nc.scalar.activation(out=lv[:], in_=ve[:], func=AF.Ln, scale=1.0)
                isd = wp.tile([D1, 1], f32, tag="isd")
                nc.scalar.activation(out=isd[:], in_=lv[:], func=AF.Exp, scale=-0.5)
                scale = wp.tile([D1, 1], f32, tag="scale")
                nc.vector.tensor_tensor(out=scale[:], in0=isd[:],
                                        in1=gmt[:, l:l + 1], op=OP.mult)
                mshift = wp.tile([D1, 1], f32, tag="mshift")
                nc.vector.tensor_tensor(out=mshift[:], in0=mu[:], in1=scale[:],
                                        op=OP.mult)
                shift = wp.tile([D1, 1], f32, tag="shift")
                nc.vector.tensor_tensor(out=shift[:], in0=bbt[:, l:l + 1],
                                        in1=mshift[:], op=OP.subtract)
                # h = relu((aggr*scale + h) + shift); the three consumers
                # (f32 residual, f8 AllGather payload, bf16 matmul copy) are
                # produced from asb concurrently on DVE/ACT/Pool
                HB = SHARD_P // 2
                for hh in range(2):
                    hsl = slice(hh * HB, (hh + 1) * HB)
                    nc.vector.scalar_tensor_tensor(
                        out=asb[:, hsl], in0=aggr_sb[:, hsl], scalar=scale[:],
                        in1=hT_own[:, hsl], op0=OP.mult, op1=OP.add)
                if l + 1 < L:
                    h8n = stp.tile([D1, SHARD_P], f8, tag="h8")
                    for hh in range(2):
                        hsl = slice(hh * HB, (hh + 1) * HB)
                        nc.scalar.activation(out=h8n[:, hsl], in_=asb[:, hsl],
                                             func=AF.Relu, bias=shift[:],
                                             scale=1.0)
                        nc.sync.dma_start(out=ag_in[:, hsl], in_=h8n[:, hsl])
                nc.vector.tensor_scalar(out=hT_own[:], in0=asb[:],
                                        scalar1=shift[:], scalar2=0.0,
                                        op0=OP.add, op1=OP.max)
                nc.gpsimd.tensor_copy(out=hb_own[:], in_=hT_own[:])

            # ---------- global mean pool ----------
            pool_ps = pp.tile([D1, G], f32, tag="pre")
            for w in range(NWIN):
                tp = ppB.tile([128, D1], f32, tag="bld")
                nc.tensor.transpose(out=tp[:], in_=hT_own[:, w * 128:(w + 1) * 128],
                                    identity=idn[0:D1, 0:D1])
                rows = wp.tile([128, D1], bf16, tag="rows")
                nc.vector.tensor_copy(out=rows[:], in_=tp[:])
                ohg = ohgp.tile([128, G], bf16, tag="ohg")
                nc.vector.tensor_scalar(
                    out=ohg[:], in0=io256[:],
                    scalar1=blc[:, w:w + 1], scalar2=rgp[:, w:w + 1],
                    op0=OP.is_equal, op1=OP.mult)
                nc.tensor.matmul(out=pool_ps[:], lhsT=rows[:], rhs=ohg[:],
                                 start=(w == 0), stop=(w == NWIN - 1))
            poolT = tlp.tile([D1, G], bf16, tag="poolT")
            nc.vector.tensor_copy(out=poolT[:], in_=pool_ps[:])
            nc.sync.dma_start(out=pl_in[:, :], in_=poolT[:])
            nc.gpsimd.collective_compute(
                "AllGather", OP.bypass, replica_groups=rg,
                ins=[pl_in.ap().opt()], outs=[pl_out.ap().opt()])
            pga = tlp.tile([D1, NCORES, G], bf16, tag="pga")
            nc.sync.dma_start(
                out=pga[:],
                in_=pl_out[:, :].rearrange("(c p) g -> p c g", p=D1))
            pg = tlp.tile([D1, G], f32, tag="pg")
            nc.vector.reduce_sum(
                out=pg[:].rearrange("p (g o) -> p g o", o=1),
                in_=pga[:].rearrange("p c g -> p g c"),
                axis=mybir.AxisListType.X)

            # ---------- head ----------
            a = pg
            hw_ = [(l1w[:], l1b[:]), (fw[:, 0:D2], fb[:, 0:1]), (fw[:, D2:2 * D2], fb[:, 1:2])]
            for (wt, bt) in hw_:
                ps = ppB.tile([D2, G], f32, tag="bld")
                nc.tensor.matmul(out=ps[:, 0:G], lhsT=wt, rhs=a[:], start=True, stop=True)
                an = tlp.tile([D2, G], f32, tag="an")
                nc.scalar.activation(out=an[:], in_=ps[:, 0:G], func=AF.Relu,
                                     bias=bt, scale=1.0)
                a = an
            ps = ppB.tile([1, G], f32, tag="bld")
            nc.tensor.matmul(out=ps[:, 0:G], lhsT=l2w[:], rhs=a[:], start=True, stop=True)
            yt = tlp.tile([1, G], f32, tag="yt")
            nc.scalar.activation(out=yt[:], in_=ps[:, 0:G], func=AF.Identity,
                                 bias=l2b[:], scale=1.0)
            nc.sync.dma_start(out=yout[:, :], in_=yt[:])

    nc.compile()
    return nc


def _wrap16(idx):
    """Flat idx list -> [128, n/16] int16: slot i at [i%16, i//16], replicated
    across the 8 Q7 cores."""
    a = idx.reshape(-1, 16).T.astype(np.int16)
    return np.tile(a, (8, 1))


def _preprocess(inputs):
    x = np.asarray(inputs["x"], np.float32)
    ea = np.asarray(inputs["edge_attr"], np.float32)
    ei = np.asarray(inputs["edge_index"]).astype(np.int64)
    batch = np.asarray(inputs["batch"]).astype(np.int64)
    src, dst = ei[0], ei[1]

    cnt = np.bincount(dst, minlength=N).astype(np.float32)
    rc_node = 1.0 / np.maximum(cnt, 1.0)
    gcnt = np.bincount(batch, minlength=G).astype(np.float32)
    rgc = 1.0 / np.maximum(gcnt, 1.0)

    # Degree-balanced node -> (window, slot) assignment: snake-deal nodes in
    # descending-degree order across the 240 global windows, minimizing the
    # max per-window edge count (which sets the uniform chunk pad cw).
    deg_order = np.argsort(-cnt, kind="stable")       # node ids, deg desc
    nwin_g = NCORES * NWIN                            # 240
    perm_loc = np.empty(N, np.int64)                  # node -> global padded id
    for i0 in range(0, N, nwin_g):
        blk = deg_order[i0:i0 + nwin_g]
        j = i0 // nwin_g
        wins = np.arange(len(blk)) if j % 2 == 0 else (len(blk) - 1 - np.arange(len(blk)))
        w_ids = wins
        perm_loc[blk] = (w_ids // NWIN) * SHARD_P + (w_ids % NWIN) * 128 + j
    gperm = perm_loc
    srcg = gperm[src]
    dstg = gperm[dst]
    order = np.argsort(dstg, kind="stable")
    srcg_s, dstg_s, ea_idx = srcg[order], dstg[order], order

    bounds = []
    for c in range(NCORES):
        for w in range(NWIN):
            bounds.append(c * SHARD_P + w * 128)
    bounds.append(NCORES * SHARD_P)
    bpos = np.searchsorted(dstg_s, np.asarray(bounds), side="left")
    percw = {}
    maxcnt = 0
    k = 0
    for c in range(NCORES):
        for w in range(NWIN):
            lo, hi = bpos[k], bpos[k + 1]
            percw[(c, w)] = np.arange(lo, hi)
            maxcnt = max(maxcnt, hi - lo)
            k += 1
    cw = max(1, (maxcnt + 127) // 128)
    etot = NWIN * cw * 128

    # full padded x, rotated per core so block 0 is the own shard
    xfull = np.zeros((NF, NCORES * SHARD_P), np.float32)
    xfull[:, gperm] = x.T
    xfull = xfull.astype(ml_dtypes.bfloat16)

    per_core = []
    for c in range(NCORES):
        qs_idx = np.zeros(etot, np.int64)
        dl = np.full(etot, -1.0, np.float32)
        rc_e = np.ones(etot, np.float32)
        ea_e = np.zeros((etot, EF), np.float32)
        for w in range(NWIN):
            idxs = percw[(c, w)]
            o = w * cw * 128
            k = len(idxs)
            g = srcg_s[idxs]                           # padded global id
            qs_idx[o:o + k] = (g % 128) * NWING + (g // 128)
            loc = dstg_s[idxs] - c * SHARD_P           # 0..3839
            dl[o:o + k] = (loc - w * 128).astype(np.float32)
            rc_e[o:o + k] = rc_node[dst[ea_idx[idxs]]]
            ea_e[o:o + k] = ea[ea_idx[idxs]]
        eaT = np.zeros((64, etot), np.float32)
        eaT[1:EF + 1] = ea_e.T
        eaT[EF + 1] = 1.0
        eaT[EF + 1, dl < 0] = 0.0
        nch = etot // 128
        ohT = np.zeros((128, etot), np.float32)
        vv = dl >= 0
        ohT[dl[vv].astype(np.int64), np.nonzero(vv)[0]] = 1.0
        d = {
            "qs_idxD": _wrap16(qs_idx),
            "ohTD": ohT.astype(ml_dtypes.float8_e4m3),
            "dstloc_p": dl.reshape(nch, 128).T.copy(),
            "rc_p": rc_e.reshape(nch, 128).T.copy(),
            "eaT": eaT.astype(ml_dtypes.float8_e4m3),
        }
        d["xTF"] = xfull
        d["xT"] = xfull[:, c * SHARD_P:(c + 1) * SHARD_P].copy()
        nodes_c = np.nonzero((gperm // SHARD_P) == c)[0]
        locs_c = gperm[nodes_c] - c * SHARD_P
        bl = np.full(SHARD_P, -1.0, np.float32)
        bl[locs_c] = batch[nodes_c].astype(np.float32)
        rg_n = np.zeros(SHARD_P, np.float32)
        rg_n[locs_c] = rgc[batch[nodes_c]]
        d["batchloc"] = bl.reshape(NWIN, 128).T.copy()
        d["rgc_pn"] = rg_n.reshape(NWIN, 128).T.copy()
        per_core.append(d)

    # replicated weights; f-gate halves pre-negated
    wf = np.asarray(inputs["conv_wf"], np.float32)
    wsv = np.asarray(inputs["conv_ws"], np.float32)
    bf = np.asarray(inputs["conv_bf"], np.float32)
    bs = np.asarray(inputs["conv_bs"], np.float32)
    wdst = np.concatenate([-wf[:, 0:D1, :], wsv[:, 0:D1, :]], axis=2)
    wsrc = np.concatenate([-wf[:, D1:2 * D1, :], wsv[:, D1:2 * D1, :]], axis=2)
    wea = np.concatenate([-wf[:, 2 * D1:, :], wsv[:, 2 * D1:, :]], axis=2)
    bias = np.concatenate([-bf, bs], axis=1)[:, None, :]
    wea = np.concatenate([wea, bias], axis=1)
    shared = {
        "lin0w": np.asarray(inputs["lin0_w"], np.float32).astype(ml_dtypes.bfloat16),
        "lin0b": np.asarray(inputs["lin0_b"], np.float32).reshape(D1, 1),
        "wdst": np.transpose(wdst, (1, 0, 2)).reshape(D1, L * 128).astype(ml_dtypes.bfloat16),
        "wsrc": np.transpose(wsrc, (1, 0, 2)).reshape(D1, L * 128).astype(ml_dtypes.bfloat16),
        "wea": np.concatenate([
            np.zeros((1, L * 128), np.float32),
            np.transpose(wea, (1, 0, 2)).reshape(EF + 1, L * 128),
            np.zeros((64 - EF - 2, L * 128), np.float32),
        ], axis=0).astype(ml_dtypes.bfloat16),
        "bng": np.asarray(inputs["bn_gamma"], np.float32).T.copy(),
        "bnb": np.asarray(inputs["bn_beta"], np.float32).T.copy(),
        "lin1w": np.asarray(inputs["lin1_w"], np.float32),
        "lin1b": np.asarray(inputs["lin1_b"], np.float32).reshape(D2, 1),
        "fcw": np.transpose(np.asarray(inputs["fc_w"], np.float32), (1, 0, 2)).reshape(D2, FC * D2),
        "fcb": np.asarray(inputs["fc_b"], np.float32).T.copy(),
        "lin2w": np.asarray(inputs["lin2_w"], np.float32).reshape(D2, 1),
        "lin2b": np.asarray(inputs["lin2_b"], np.float32).reshape(1, 1),
        "iota128": np.broadcast_to(np.arange(128, dtype=np.float32)[None, :],
                                   (128, 128)).astype(ml_dtypes.bfloat16),
        "iota256": np.broadcast_to(np.arange(G, dtype=np.float32)[None, :],
                                   (128, G)).astype(ml_dtypes.bfloat16),
        "ident": np.eye(128, dtype=np.float32),
        "identb": np.eye(128, dtype=np.float32).astype(ml_dtypes.bfloat16),
    }
    in_maps = [dict(shared, **pc) for pc in per_core]
    return in_maps, cw


def kernel(**inputs):
    from concourse.bass_utils import run_bass_kernel_spmd

    in_maps, cw = _preprocess(inputs)
    key = ("nc", cw)
    if key not in _CACHE:
        _CACHE[key] = _build_nc(cw)
    nc = _CACHE[key]
    res = run_bass_kernel_spmd(nc, in_maps, core_ids=list(range(NCORES)))
    return res.results[0]["y"].reshape(G).astype(np.float32)


# revision 57
# speedup vs baseline: 1.0149x; 1.0023x over previous
"""CGCNN message-passing kernel for 8 Trainium2 NeuronCores (Bass/Tile), v9.

Data-parallel by dst shard; gather-based edge pipeline:
- Host: nodes are dealt into 240 global windows (8 cores x 30 windows x 128
  slots) in descending-degree snake order, equalizing per-window edge counts
  so the uniform chunks-per-window pad cw is minimal (16). Edges go to the
  core owning their dst, grouped by dst window, chunk-padded to cw.
- lin0 is computed for ALL nodes redundantly on every core (from a replicated
  full xT) into a DRAM fp8 table, so layer 0 needs no h AllGather; layers
  1..L-1 AllGather h in fp8 (staged at the previous layer's BN boundary so
  the collective launches as early as possible).
- Per layer, per core:
  * Qd table (own shard, SBUF bf16 [128, 30, 128]) = h_own @ Wdst.
  * Full Qs table = h_full @ Wsrc into DRAM [30720, 128] bf16 in
    partition-major row order (node g -> row (g%128)*240 + g//128); PSUM->SBUF
    staging copies round-robin over DVE/ACT to minimize build latency.
  * Per 1024-edge tile: one dma_gather pulls per-edge Qs rows (1024 x 256B
    descriptors; 1024 = SWDGE ring capacity); Qe = ea(fp8) @ Wea by matmul
    (edge attrs streamed fp8, 4 tiles per DMA); the dst contribution expands
    via a host-precomputed one-hot (fp8, SBUF-resident, layer-invariant)
    matmul against the SBUF Qd table. All three accumulate in PSUM per
    128-edge chunk.
  * Nonlinearity: joint exp u = [e^-a | e^b] (f-gate weights pre-negated),
    v = ln(1+u); 1/3 of tiles compute sigmoid(a) = e^-v_f on ACT, 2/3 as
    1/(1+u_f) on DVE (bf16, engine balance). m = 2*sigmoid(a)*softplus(b);
    the factor 2 is absorbed exactly by BatchNorm using 4*EPS.
  * Aggregation one-hots (is_equal(iota, dst) * 1/cnt, bf16 on DVE) are
    pre-built per tile, and the aggregation matmuls are deferred by one tile
    so the in-order PE queue never stalls on the ACT/DVE nonlinearity chain.
  * Segment-mean accumulates per dst window in PSUM (agg PSUM shares banks
    with the build-phase staging, freeing a third pre-PSUM buffer); BatchNorm
    batch stats via a tiny stats AllGather + local sum; the residual
    (scalar_tensor_tensor + relu) is computed in halves, with the fp8
    AllGather payload produced on ACT in parallel with the f32/bf16 h copies
    on DVE/Pool.
- Global mean pool via one-hot matmul, bf16 partials AllGathered and summed
  locally, head MLP computed redundantly on every core.
"""
import numpy as np
import ml_dtypes

N = 30000
E = 480000
NF = 92
EF = 50
D1 = 64
D2 = 64
L = 3
FC = 2
G = 256
EPS = 1e-5
NCORES = 8
SHARD = N // NCORES            # 3750
SHARD_P = 3840                 # padded shard (30 windows of 128)
NWIN = SHARD_P // 128          # 30
NWING = NCORES * NWIN          # 240 global windows
TBL = NCORES * SHARD_P         # 30720 table rows

_CACHE = {}



def _build_nc(cw):
    """Build the SPMD bass module. cw = chunks per dst window (uniform)."""
    import concourse.mybir as mybir
    from concourse import bacc
    from concourse.tile import TileContext

    f32 = mybir.dt.float32
    bf16 = mybir.dt.bfloat16
    f8 = mybir.dt.float8e4
    i16 = mybir.dt.int16
    AF = mybir.ActivationFunctionType
    OP = mybir.AluOpType

    nchunk = NWIN * cw                 # chunks per layer per core
    etot = nchunk * 128                # padded edges per core
    ntile = (nchunk + 7) // 8          # 8-chunk (1024-edge) PSUM tiles

    import concourse.hw_specs as _hw
    import concourse.bacc as _bacc_mod
    _real_tables = _hw.get_activation_tables("gen3")
    _combined = "natural_log_exp_and_others"
    if _combined in _real_tables:
        _patched = {
            k: (v if k == _combined else (v - {AF.Exp, AF.Ln}))
            for k, v in _real_tables.items()
        }
        _bacc_mod.get_activation_tables = lambda arch: _patched

    nc = bacc.Bacc(None, target_bir_lowering=False)

    # ---- inputs (per core) ----
    xTF = nc.dram_tensor("xTF", [NF, TBL], bf16, kind="ExternalInput")
    xT = nc.dram_tensor("xT", [NF, SHARD_P], bf16, kind="ExternalInput")
    eaT = nc.dram_tensor("eaT", [64, etot], f8, kind="ExternalInput")
    qs_idxD = nc.dram_tensor("qs_idxD", [128, etot // 16], i16, kind="ExternalInput")
    ohTD = nc.dram_tensor("ohTD", [128, etot], f8, kind="ExternalInput")
    dstloc_p = nc.dram_tensor("dstloc_p", [128, nchunk], f32, kind="ExternalInput")
    rc_p = nc.dram_tensor("rc_p", [128, nchunk], f32, kind="ExternalInput")
    batchloc = nc.dram_tensor("batchloc", [128, NWIN], f32, kind="ExternalInput")
    rgc_pn = nc.dram_tensor("rgc_pn", [128, NWIN], f32, kind="ExternalInput")
    # weights (replicated; f-gate halves pre-negated)
    lin0w = nc.dram_tensor("lin0w", [NF, D1], bf16, kind="ExternalInput")
    lin0b = nc.dram_tensor("lin0b", [D1, 1], f32, kind="ExternalInput")
    wdst = nc.dram_tensor("wdst", [D1, L * 128], bf16, kind="ExternalInput")
    wsrc = nc.dram_tensor("wsrc", [D1, L * 128], bf16, kind="ExternalInput")
    wea = nc.dram_tensor("wea", [64, L * 128], bf16, kind="ExternalInput")
    bng = nc.dram_tensor("bng", [D1, L], f32, kind="ExternalInput")
    bnb = nc.dram_tensor("bnb", [D1, L], f32, kind="ExternalInput")
    lin1w = nc.dram_tensor("lin1w", [D1, D2], f32, kind="ExternalInput")
    lin1b = nc.dram_tensor("lin1b", [D2, 1], f32, kind="ExternalInput")
    fcw = nc.dram_tensor("fcw", [D2, FC * D2], f32, kind="ExternalInput")
    fcb = nc.dram_tensor("fcb", [D2, FC], f32, kind="ExternalInput")
    lin2w = nc.dram_tensor("lin2w", [D2, 1], f32, kind="ExternalInput")
    lin2b = nc.dram_tensor("lin2b", [1, 1], f32, kind="ExternalInput")
    iota128 = nc.dram_tensor("iota128", [128, 128], bf16, kind="ExternalInput")
    iota256 = nc.dram_tensor("iota256", [128, G], bf16, kind="ExternalInput")
    ident = nc.dram_tensor("ident", [128, 128], f32, kind="ExternalInput")
    identb = nc.dram_tensor("identb", [128, 128], bf16, kind="ExternalInput")

    yout = nc.dram_tensor("y", [1, G], f32, kind="ExternalOutput")

    # ---- DRAM scratch ----
    QsD = nc.dram_tensor("QsD", [TBL, 128], bf16)          # row p*NWING+W
    h1f8 = nc.dram_tensor("h1f8", [NCORES * D1, SHARD_P], f8)
    ag_in = nc.dram_tensor("ag_in", [D1, SHARD_P], f8)
    ag_out = nc.dram_tensor("ag_out", [NCORES * D1, SHARD_P], f8,
                            addr_space="Shared")
    ar_in = nc.dram_tensor("ar_in", [D1, 2], f32)
    ar_out = nc.dram_tensor("ar_out", [NCORES * D1, 2], f32, addr_space="Shared")
    pl_in = nc.dram_tensor("pl_in", [D1, G], bf16)
    pl_out = nc.dram_tensor("pl_out", [NCORES * D1, G], bf16, addr_space="Shared")

    rg = [list(range(NCORES))]
    QsD3 = QsD[:, :].rearrange("(p w) f -> p w f", p=128)   # [128, NWING, 128]

    from contextlib import ExitStack
    with TileContext(nc) as tc:
        with ExitStack() as _es:
            cp = _es.enter_context(tc.tile_pool(name="const", bufs=1))
            bigp = _es.enter_context(tc.tile_pool(name="big", bufs=1))
            wp = _es.enter_context(tc.tile_pool(name="work", bufs=3))
            tlp = _es.enter_context(tc.tile_pool(name="tail", bufs=1))
            gp = _es.enter_context(tc.tile_pool(name="gat", bufs=4))
            ep = _es.enter_context(tc.tile_pool(name="ea", bufs=3))
            nlp = _es.enter_context(tc.tile_pool(name="nl", bufs=3))
            ohp = _es.enter_context(tc.tile_pool(name="oh", bufs=13))
            ohgp = _es.enter_context(tc.tile_pool(name="ohg", bufs=5))
            stp = _es.enter_context(tc.tile_pool(name="st", bufs=2))
            sgp = _es.enter_context(tc.tile_pool(name="sgp", bufs=3))
            scp = _es.enter_context(tc.tile_pool(name="scr", bufs=1))
            pp = _es.enter_context(tc.tile_pool(name="pre", bufs=3, space="PSUM"))
            ppB = _es.enter_context(tc.tile_pool(name="psB", bufs=2, space="PSUM"))
            # ---------- constants ----------
            def load_const(t, dram, shape, dtype=f32):
                tt = cp.tile(shape, dtype, tag=t)
                nc.sync.dma_start(out=tt[:], in_=dram)
                return tt

            l0w = load_const("l0w", lin0w[:, :], [NF, D1], bf16)
            l0b = load_const("l0b", lin0b[:, :], [D1, 1])

            # ---------- resident state ----------
            hT_own = bigp.tile([D1, SHARD_P], f32, tag="hown")
            hb_own = bigp.tile([D1, SHARD_P], bf16, tag="hbown")
            aggr_sb = bigp.tile([D1, SHARD_P], bf16, tag="aggr")
            qd_sb = bigp.tile([128, NWIN, 128], bf16, tag="qdsb")
            asb = scp.tile([D1, SHARD_P], f32, tag="asb")

            # ---------- lin0 for ALL nodes (no AllGather for layer 0) ----
            # h1f8 holds relu(x @ lin0_w + b) for all 8 shards (global order),
            # computed redundantly on every core from the replicated xTF.
            HL = SHARD_P // 2
            for s_ in range(NCORES):
                h8s = stp.tile([D1, SHARD_P], f8, tag="h8")
                for hh in range(2):
                    xt = sgp.tile([NF, HL], bf16, tag="qsst")
                    o = s_ * SHARD_P + hh * HL
                    nc.sync.dma_start(out=xt[:], in_=xTF[:, o:o + HL])
                    for j in range(4):
                        sl = slice(j * 480, (j + 1) * 480)
                        ph = ppB.tile([D1, 512], f32, tag="bld")
                        nc.tensor.matmul(out=ph[:, :480], lhsT=l0w[:],
                                         rhs=xt[:, sl], start=True, stop=True)
                        osl = slice(hh * HL + j * 480, hh * HL + (j + 1) * 480)
                        if j % 2 == 0:
                            nc.scalar.activation(
                                out=h8s[:, osl],
                                in_=ph[:, :480], func=AF.Relu, bias=l0b[:],
                                scale=1.0)
                        else:
                            nc.vector.tensor_scalar(
                                out=h8s[:, osl], in0=ph[:, :480],
                                scalar1=l0b[:], scalar2=0.0,
                                op0=OP.add, op1=OP.max)
                nc.sync.dma_start(out=h1f8[s_ * D1:(s_ + 1) * D1, :], in_=h8s[:])

            # own-shard h in f32 from the per-core xT input
            for hh in range(2):
                xt0 = sgp.tile([NF, HL], bf16, tag="qsst")
                nc.sync.dma_start(out=xt0[:], in_=xT[:, hh * HL:(hh + 1) * HL])
                for j in range(4):
                    sl = slice(hh * HL + j * 480, hh * HL + (j + 1) * 480)
                    ph = ppB.tile([D1, 512], f32, tag="bld")
                    nc.tensor.matmul(out=ph[:, :480], lhsT=l0w[:],
                                     rhs=xt0[:, j * 480:(j + 1) * 480],
                                     start=True, stop=True)
                    nc.scalar.activation(out=hT_own[:, sl], in_=ph[:, :480],
                                         func=AF.Relu, bias=l0b[:], scale=1.0)
                    nc.vector.tensor_scalar(
                        out=hb_own[:, sl], in0=ph[:, :480],
                        scalar1=l0b[:], scalar2=0.0, op0=OP.add, op1=OP.max)

            # remaining constants: emitted after lin0 so their DMA (notably
            # the 60KB/partition one-hot + gather indices) doesn't serialize
            # ahead of the xTF streams in the SP/DMA queues
            ws = load_const("ws", wsrc[:, :], [D1, L * 128], bf16)
            wd = load_const("wd", wdst[:, :], [D1, L * 128], bf16)
            we = load_const("we", wea[:, :], [64, L * 128], bf16)
            io128 = load_const("io128", iota128[:, :], [128, 128], bf16)
            idnb = load_const("idnb", identb[:, :], [128, 128], bf16)
            dlp = load_const("dlp", dstloc_p[:, :], [128, nchunk])
            rcp = load_const("rcp", rc_p[:, :], [128, nchunk])
            gmt = load_const("gmt", bng[:, :], [D1, L])
            bbt = load_const("bbt", bnb[:, :], [D1, L])
            io256 = load_const("io256", iota256[:, :], [128, G], bf16)
            idn = load_const("idn", ident[:, :], [128, 128])
            blc = load_const("blc", batchloc[:, :], [128, NWIN])
            rgp = load_const("rgp", rgc_pn[:, :], [128, NWIN])
            l1w = load_const("l1w", lin1w[:, :], [D1, D2])
            l1b = load_const("l1b", lin1b[:, :], [D2, 1])
            fw = load_const("fw", fcw[:, :], [D2, FC * D2])
            fb = load_const("fb", fcb[:, :], [D2, FC])
            l2w = load_const("l2w", lin2w[:, :], [D2, 1])
            l2b = load_const("l2b", lin2b[:, :], [1, 1])
            qsix = load_const("qsix", qs_idxD[:, :], [128, etot // 16], i16)
            ohT_res = cp.tile([128, nchunk, 128], f8, tag="ohres")
            nc.sync.dma_start(
                out=ohT_res[:].rearrange("p a b -> p (a b)"), in_=ohTD[:, :])

            # ---------- layers ----------
            for l in range(L):
                wd_l = wd[:, l * 128:(l + 1) * 128]
                ws_l = ws[:, l * 128:(l + 1) * 128]
                we_l = we[:, l * 128:(l + 1) * 128]

                if l == 0:
                    src_dram = h1f8
                else:
                    # --- AllGather h (fp8, staged into ag_in at layer end) ---
                    nc.gpsimd.collective_compute(
                        "AllGather", OP.bypass, replica_groups=rg,
                        ins=[ag_in.ap().opt()], outs=[ag_out.ap().opt()])
                    src_dram = ag_out

                # --- Qd table build (own shard) ---
                for w0 in range(0, NWIN, 4):
                    kk = min(4, NWIN - w0)
                    qp = ppB.tile([128, 512], f32, tag="bld")
                    for k in range(kk):
                        w = w0 + k
                        nc.tensor.matmul(
                            out=qp[:, k * 128:(k + 1) * 128],
                            lhsT=hb_own[:, w * 128:(w + 1) * 128],
                            rhs=wd_l, start=True, stop=True)
                    nc.vector.tensor_copy(
                        out=qd_sb[:, w0:w0 + kk, :].rearrange("p a b -> p (a b)"),
                        in_=qp[:, :kk * 128])

                # --- Qs table build (all nodes, per gathered shard) -> QsD ---
                ws8 = stp.tile([D1, 128], f8, tag="ws8")
                nc.scalar.activation(out=ws8[:], in_=ws_l,
                                     func=AF.Identity, scale=1.0)
                ncopy = 0
                for s_ in range(NCORES):
                    hb_sh = stp.tile([D1, SHARD_P], f8, tag="h8")
                    nc.sync.dma_start(out=hb_sh[:],
                                      in_=src_dram[s_ * D1:(s_ + 1) * D1, :])
                    for wB in range(0, NWIN, 16):
                        kB = min(16, NWIN - wB)
                        sg_t = sgp.tile([128, 16, 128], bf16, tag="qsst")
                        for w0 in range(wB, wB + kB, 4):
                            kk = min(4, wB + kB - w0)
                            qp = ppB.tile([128, 512], f32, tag="bld")
                            for k in range(kk):
                                w = w0 + k
                                nc.tensor.matmul(
                                    out=qp[:, k * 128:(k + 1) * 128],
                                    lhsT=hb_sh[:, w * 128:(w + 1) * 128],
                                    rhs=ws8[:], start=True, stop=True)
                            dst_ap = sg_t[:, w0 - wB:w0 - wB + kk, :] \
                                .rearrange("p a b -> p (a b)")
                            eng = ncopy % 5
                            ncopy += 1
                            if eng in (0, 2, 4):
                                nc.vector.tensor_copy(
                                    out=dst_ap, in_=qp[:, :kk * 128])
                            else:
                                nc.scalar.activation(
                                    out=dst_ap, in_=qp[:, :kk * 128],
                                    func=AF.Identity, scale=1.0)
                        W0 = s_ * NWIN + wB
                        nc.sync.dma_start(out=QsD3[:, W0:W0 + kB, :],
                                          in_=sg_t[:, :kB, :])

                # --- edge pipeline ---
                st1g = wp.tile([D1, 8], f32, tag="st1g")
                st2g = wp.tile([D1, 8], f32, tag="st2g")
                agg = None
                qs_g = None
                aggst = {"agg": None}

                def emit_agg(m, ohs_t, t, te):
                    # aggregation for tile t, deferred one tile so the PE
                    # queue never stalls waiting for m
                    for c in range(te):
                        gc = t * 8 + c
                        w = gc // cw
                        if gc % (4 * cw) == 0:
                            agg_new = ppB.tile([D1, 512], f32, tag="bld")
                            aggst["agg"] = agg_new
                        agg = aggst["agg"]
                        nc.tensor.matmul(
                            out=agg[:, (w % 4) * 128:(w % 4 + 1) * 128],
                            lhsT=m[:, c, :], rhs=ohs_t[c][:],
                            start=(gc % cw == 0), stop=(gc % cw == cw - 1))
                        if gc % (4 * cw) == 4 * cw - 1 or gc == nchunk - 1:
                            grp = w // 4
                            lo = grp * 512
                            hi = min(lo + 512, SHARD_P)
                            nc.scalar.activation(
                                out=aggr_sb[:, lo:hi], in_=agg[:, :hi - lo],
                                func=AF.Identity, scale=1.0)
                            nc.vector.reduce_sum(
                                out=st1g[:, grp:grp + 1],
                                in_=aggr_sb[:, lo:hi],
                                axis=mybir.AxisListType.X)
                            sqg = nlp.tile([D1, 512], bf16, tag="sqg")
                            nc.vector.tensor_tensor(
                                out=sqg[:, :hi - lo], in0=aggr_sb[:, lo:hi],
                                in1=aggr_sb[:, lo:hi], op=OP.mult)
                            nc.vector.reduce_sum(
                                out=st2g[:, grp:grp + 1],
                                in_=sqg[:, :hi - lo],
                                axis=mybir.AxisListType.X)

                pend = []
                for t in range(ntile):
                    te = min(8, nchunk - t * 8)          # chunks this tile
                    ne = te * 128                        # edges this tile
                    if t % 4 == 0:
                        tc32 = min(32, nchunk - t * 8)
                        et = ep.tile([64, 4096], f8, tag="et")
                        nc.sync.dma_start(
                            out=et[:, :tc32 * 128],
                            in_=eaT[:, t * 1024: t * 1024 + tc32 * 128])
                    qs_g = gp.tile([128, 8, 128], bf16, tag="qsg")
                    nc.gpsimd.dma_gather(
                        qs_g[:, :te, :], QsD[:, :],
                        qsix[:, t * 64: t * 64 + te * 8],
                        te * 128, te * 128, 128)
                    half = 0
                    qhalf = (t % 4) * 8

                    # one-hot aggregation matrices: const-only deps, built
                    # ahead so the agg matmuls never wait on DVE
                    ohs_t = []
                    for c in range(te):
                        gc = t * 8 + c
                        oh_ = ohp.tile([128, 128], bf16, tag="ohS")
                        nc.vector.tensor_scalar(
                            out=oh_[:], in0=io128[:],
                            scalar1=dlp[:, gc:gc + 1], scalar2=rcp[:, gc:gc + 1],
                            op0=OP.is_equal, op1=OP.mult)
                        ohs_t.append(oh_)

                    pre = pp.tile([128, 1024], f32, tag="pre")
                    qs_f = qs_g[:].rearrange("p a b -> p (a b)")
                    for c in range(te):
                        gc = t * 8 + c
                        w = gc // cw
                        csl = slice(c * 128, (c + 1) * 128)
                        csl2 = slice((half + c) * 128, (half + c + 1) * 128)
                        csl4 = slice((qhalf + c) * 128, (qhalf + c + 1) * 128)
                        nc.tensor.matmul(out=pre[:, csl], lhsT=et[:, csl4],
                                         rhs=we_l, start=True, stop=False)
                        nc.tensor.matmul(out=pre[:, csl], lhsT=idnb[:],
                                         rhs=qs_f[:, csl2], start=False, stop=False)
                        nc.tensor.matmul(out=pre[:, csl], lhsT=ohT_res[:, gc, :],
                                         rhs=qd_sb[:, w, :], start=False, stop=True)

                    # nonlinearity: m = (1+tanh(a/2)) * softplus(b)
                    #             = 2*sigmoid(a)*softplus(b)  (2 absorbed by BN)
                    # nonlinearity: u = [e^-a | e^b], v = ln(1+u) = [sp(-a)|sp(b)]
                    # even tiles (ACT): sigma = e^-sp(-a); odd tiles (DVE):
                    # sigma = 1/(1+e^-a). m = 2*sigma*sp(b) (2 absorbed by BN
                    # via 4*EPS).
                    uf = nlp.tile([128, 8, 128], bf16, tag="uf")
                    nc.scalar.activation(
                        out=uf[:, :te, :].rearrange("p a b -> p (a b)"),
                        in_=pre[:, :ne], func=AF.Exp, scale=1.0)
                    m = nlp.tile([128, 8, 64], bf16, tag="m")
                    if t % 3 == 0:
                        vf = nlp.tile([128, 8, 128], bf16, tag="vf")
                        nc.scalar.activation(
                            out=vf[:, :te, :].rearrange("p a b -> p (a b)"),
                            in_=uf[:, :te, :].rearrange("p a b -> p (a b)"),
                            func=AF.Ln, bias=1.0, scale=1.0)
                        sg = nlp.tile([128, 8, 64], bf16, tag="sg")
                        nc.scalar.activation(out=sg[:, :te, :],
                                             in_=vf[:, :te, 0:64],
                                             func=AF.Exp, scale=-1.0)
                        nc.vector.scalar_tensor_tensor(
                            out=m[:, :te, :], in0=sg[:, :te, :], scalar=2.0,
                            in1=vf[:, :te, 64:128], op0=OP.mult, op1=OP.mult)
                    else:
                        vs = nlp.tile([128, 8, 64], bf16, tag="vs")
                        nc.scalar.activation(out=vs[:, :te, :],
                                             in_=uf[:, :te, 64:128],
                                             func=AF.Ln, bias=1.0, scale=1.0)
                        w1 = nlp.tile([128, 8, 64], bf16, tag="sg")
                        with nc.allow_low_precision(reason="sigmoid in bf16"):
                            nc.vector.tensor_scalar(out=w1[:, :te, :],
                                                    in0=uf[:, :te, 0:64],
                                                    scalar1=1.0, scalar2=None,
                                                    op0=OP.add)
                            nc.vector.reciprocal(out=w1[:, :te, :],
                                                 in_=w1[:, :te, :])
                        nc.vector.scalar_tensor_tensor(
                            out=m[:, :te, :], in0=w1[:, :te, :], scalar=2.0,
                            in1=vs[:, :te, :], op0=OP.mult, op1=OP.mult)

                    pend.append((m, ohs_t, t, te))
                    if len(pend) > 1:
                        emit_agg(*pend.pop(0))
                while pend:
                    emit_agg(*pend.pop(0))

                # --- BN stats + AllReduce ---
                st = wp.tile([D1, 2], f32, tag="st")
                nc.vector.reduce_sum(out=st[:, 0:1], in_=st1g[:],
                                     axis=mybir.AxisListType.X)
                nc.vector.reduce_sum(out=st[:, 1:2], in_=st2g[:],
                                     axis=mybir.AxisListType.X)
                nc.sync.dma_start(out=ar_in[:, :], in_=st[:])
                nc.gpsimd.collective_compute(
                    "AllGather", OP.bypass, replica_groups=rg,
                    ins=[ar_in.ap().opt()], outs=[ar_out.ap().opt()])
                stga = wp.tile([D1, 2, NCORES], f32, tag="stga")
                nc.sync.dma_start(
                    out=stga[:],
                    in_=ar_out[:, :].rearrange("(c p) s -> p s c", p=D1))
                stg = wp.tile([D1, 2], f32, tag="stg")
                nc.vector.reduce_sum(
                    out=stg[:].rearrange("p (s o) -> p s o", o=1),
                    in_=stga[:], axis=mybir.AxisListType.X)
                mu = wp.tile([D1, 1], f32, tag="mu")
                nc.vector.tensor_scalar(out=mu[:], in0=stg[:, 0:1],
                                        scalar1=1.0 / N, scalar2=None, op0=OP.mult)
                ex2 = wp.tile([D1, 1], f32, tag="ex2")
                nc.vector.tensor_scalar(out=ex2[:], in0=stg[:, 1:2],
                                        scalar1=1.0 / N, scalar2=None, op0=OP.mult)
                mu2 = wp.tile([D1, 1], f32, tag="mu2")
                nc.vector.tensor_tensor(out=mu2[:], in0=mu[:], in1=mu[:], op=OP.mult)
                var = wp.tile([D1, 1], f32, tag="var")
                nc.vector.tensor_tensor(out=var[:], in0=ex2[:], in1=mu2[:],
                                        op=OP.subtract)
                ve = wp.tile([D1, 1], f32, tag="ve")
                # m carries a factor 2 -> aggr/mu scale by 2, var by 4; using
                # 4*EPS makes BN output exactly match the reference.
                nc.vector.tensor_scalar(out=ve[:], in0=var[:], scalar1=4.0 * EPS,
                                        scalar2=None, op0=OP.add)
                lv = wp.tile([D1, 1], f32, tag="lv")
                nc.scalar.activation(out=lv[:], in_=ve[:], func=AF.Ln, scale=1.0)
                isd = wp.tile([D1, 1], f32, tag="isd")
                nc.scalar.activation(out=isd[:], in_=lv[:], func=AF.Exp, scale=-0.5)
                scale = wp.tile([D1, 1], f32, tag="scale")
                nc.vector.tensor_tensor(out=scale[:], in0=isd[:],
                                        in1=gmt[:, l:l + 1], op=OP.mult)
                mshift = wp.tile([D1, 1], f32, tag="mshift")
                nc.vector.tensor_tensor(out=mshift[:], in0=mu[:], in1=scale[:],
                                        op=OP.mult)
                shift = wp.tile([D1, 1], f32, tag="shift")
                nc.vector.tensor_tensor(out=shift[:], in0=bbt[:, l:l + 1],
                                        in1=mshift[:], op=OP.subtract)
                # h = relu((aggr*scale + h) + shift); the three consumers
                # (f32 residual, f8 AllGather payload, bf16 matmul copy) are
                # produced from asb concurrently on DVE/ACT/Pool
                HB = SHARD_P // 2
                for hh in range(2):
                    hsl = slice(hh * HB, (hh + 1) * HB)
                    nc.vector.scalar_tensor_tensor(
                        out=asb[:, hsl], in0=aggr_sb[:, hsl], scalar=scale[:],
                        in1=hT_own[:, hsl], op0=OP.mult, op1=OP.add)
                if l + 1 < L:
                    h8n = stp.tile([D1, SHARD_P], f8, tag="h8")
                    for hh in range(2):
                        hsl = slice(hh * HB, (hh + 1) * HB)
                        nc.scalar.activation(out=h8n[:, hsl], in_=asb[:, hsl],
                                             func=AF.Relu, bias=shift[:],
                                             scale=1.0)
                        nc.sync.dma_start(out=ag_in[:, hsl], in_=h8n[:, hsl])
                if l + 1 < L:
                    nc.vector.tensor_scalar(out=hT_own[:], in0=asb[:],
                                            scalar1=shift[:], scalar2=0.0,
                                            op0=OP.add, op1=OP.max)
                    nc.gpsimd.tensor_copy(out=hb_own[:], in_=hT_own[:])
                else:
                    # last layer: no AllGather payload to produce on ACT and
                    # no next Qd build needing hb_own; do the relu on ACT in
                    # halves (pipelined behind the stt halves on DVE) so the
                    # pool phase starts sooner
                    for hh in range(2):
                        hsl = slice(hh * HB, (hh + 1) * HB)
                        nc.scalar.activation(out=hT_own[:, hsl],
                                             in_=asb[:, hsl], func=AF.Relu,
                                             bias=shift[:], scale=1.0)

            # ---------- global mean pool ----------
            pool_ps = pp.tile([D1, G], f32, tag="pre")
            for w in range(NWIN):
                tp = ppB.tile([128, D1], f32, tag="bld")
                nc.tensor.transpose(out=tp[:], in_=hT_own[:, w * 128:(w + 1) * 128],
                                    identity=idn[0:D1, 0:D1])
                rows = wp.tile([128, D1], bf16, tag="rows")
                nc.vector.tensor_copy(out=rows[:], in_=tp[:])
                ohg = ohgp.tile([128, G], bf16, tag="ohg")
                nc.vector.tensor_scalar(
                    out=ohg[:], in0=io256[:],
                    scalar1=blc[:, w:w + 1], scalar2=rgp[:, w:w + 1],
                    op0=OP.is_equal, op1=OP.mult)
                nc.tensor.matmul(out=pool_ps[:], lhsT=rows[:], rhs=ohg[:],
                                 start=(w == 0), stop=(w == NWIN - 1))
            poolT = tlp.tile([D1, G], bf16, tag="poolT")
            nc.vector.tensor_copy(out=poolT[:], in_=pool_ps[:])
            nc.sync.dma_start(out=pl_in[:, :], in_=poolT[:])
            nc.gpsimd.collective_compute(
                "AllGather", OP.bypass, replica_groups=rg,
                ins=[pl_in.ap().opt()], outs=[pl_out.ap().opt()])
            pga = tlp.tile([D1, NCORES, G], bf16, tag="pga")
            nc.sync.dma_start(
                out=pga[:],
                in_=pl_out[:, :].rearrange("(c p) g -> p c g", p=D1))
            pg = tlp.tile([D1, G], f32, tag="pg")
            nc.vector.reduce_sum(
                out=pg[:].rearrange("p (g o) -> p g o", o=1),
                in_=pga[:].rearrange("p c g -> p g c"),
                axis=mybir.AxisListType.X)

            # ---------- head ----------
            a = pg
            hw_ = [(l1w[:], l1b[:]), (fw[:, 0:D2], fb[:, 0:1]), (fw[:, D2:2 * D2], fb[:, 1:2])]
            for (wt, bt) in hw_:
                ps = ppB.tile([D2, G], f32, tag="bld")
                nc.tensor.matmul(out=ps[:, 0:G], lhsT=wt, rhs=a[:], start=True, stop=True)
                an = tlp.tile([D2, G], f32, tag="an")
                nc.scalar.activation(out=an[:], in_=ps[:, 0:G], func=AF.Relu,
                                     bias=bt, scale=1.0)
                a = an
            ps = ppB.tile([1, G], f32, tag="bld")
            nc.tensor.matmul(out=ps[:, 0:G], lhsT=l2w[:], rhs=a[:], start=True, stop=True)
            yt = tlp.tile([1, G], f32, tag="yt")
            nc.scalar.activation(out=yt[:], in_=ps[:, 0:G], func=AF.Identity,
                                 bias=l2b[:], scale=1.0)
            nc.sync.dma_start(out=yout[:, :], in_=yt[:])

    nc.compile()
    return nc


def _wrap16(idx):
    """Flat idx list -> [128, n/16] int16: slot i at [i%16, i//16], replicated
    across the 8 Q7 cores."""
    a = idx.reshape(-1, 16).T.astype(np.int16)
    return np.tile(a, (8, 1))


def _preprocess(inputs):
    x = np.asarray(inputs["x"], np.float32)
    ea = np.asarray(inputs["edge_attr"], np.float32)
    ei = np.asarray(inputs["edge_index"]).astype(np.int64)
    batch = np.asarray(inputs["batch"]).astype(np.int64)
    src, dst = ei[0], ei[1]

    cnt = np.bincount(dst, minlength=N).astype(np.float32)
    rc_node = 1.0 / np.maximum(cnt, 1.0)
    gcnt = np.bincount(batch, minlength=G).astype(np.float32)
    rgc = 1.0 / np.maximum(gcnt, 1.0)

    # Degree-balanced node -> (window, slot) assignment: snake-deal nodes in
    # descending-degree order across the 240 global windows, minimizing the
    # max per-window edge count (which sets the uniform chunk pad cw).
    deg_order = np.argsort(-cnt, kind="stable")       # node ids, deg desc
    nwin_g = NCORES * NWIN                            # 240
    perm_loc = np.empty(N, np.int64)                  # node -> global padded id
    for i0 in range(0, N, nwin_g):
        blk = deg_order[i0:i0 + nwin_g]
        j = i0 // nwin_g
        wins = np.arange(len(blk)) if j % 2 == 0 else (len(blk) - 1 - np.arange(len(blk)))
        w_ids = wins
        perm_loc[blk] = (w_ids // NWIN) * SHARD_P + (w_ids % NWIN) * 128 + j
    gperm = perm_loc
    srcg = gperm[src]
    dstg = gperm[dst]
    order = np.argsort(dstg, kind="stable")
    srcg_s, dstg_s, ea_idx = srcg[order], dstg[order], order

    bounds = []
    for c in range(NCORES):
        for w in range(NWIN):
            bounds.append(c * SHARD_P + w * 128)
    bounds.append(NCORES * SHARD_P)
    bpos = np.searchsorted(dstg_s, np.asarray(bounds), side="left")
    percw = {}
    maxcnt = 0
    k = 0
    for c in range(NCORES):
        for w in range(NWIN):
            lo, hi = bpos[k], bpos[k + 1]
            percw[(c, w)] = np.arange(lo, hi)
            maxcnt = max(maxcnt, hi - lo)
            k += 1
    cw = max(1, (maxcnt + 127) // 128)
    etot = NWIN * cw * 128

    # full padded x, rotated per core so block 0 is the own shard
    xfull = np.zeros((NF, NCORES * SHARD_P), np.float32)
    xfull[:, gperm] = x.T
    xfull = xfull.astype(ml_dtypes.bfloat16)

    per_core = []
    for c in range(NCORES):
        qs_idx = np.zeros(etot, np.int64)
        dl = np.full(etot, -1.0, np.float32)
        rc_e = np.ones(etot, np.float32)
        ea_e = np.zeros((etot, EF), np.float32)
        for w in range(NWIN):
            idxs = percw[(c, w)]
            o = w * cw * 128
            k = len(idxs)
            g = srcg_s[idxs]                           # padded global id
            qs_idx[o:o + k] = (g % 128) * NWING + (g // 128)
            loc = dstg_s[idxs] - c * SHARD_P           # 0..3839
            dl[o:o + k] = (loc - w * 128).astype(np.float32)
            rc_e[o:o + k] = rc_node[dst[ea_idx[idxs]]]
            ea_e[o:o + k] = ea[ea_idx[idxs]]
        eaT = np.zeros((64, etot), np.float32)
        eaT[1:EF + 1] = ea_e.T
        eaT[EF + 1] = 1.0
        eaT[EF + 1, dl < 0] = 0.0
        nch = etot // 128
        ohT = np.zeros((128, etot), np.float32)
        vv = dl >= 0
        ohT[dl[vv].astype(np.int64), np.nonzero(vv)[0]] = 1.0
        d = {
            "qs_idxD": _wrap16(qs_idx),
            "ohTD": ohT.astype(ml_dtypes.float8_e4m3),
            "dstloc_p": dl.reshape(nch, 128).T.copy(),
            "rc_p": rc_e.reshape(nch, 128).T.copy(),
            "eaT": eaT.astype(ml_dtypes.float8_e4m3),
        }
        d["xTF"] = xfull
        d["xT"] = xfull[:, c * SHARD_P:(c + 1) * SHARD_P].copy()
        nodes_c = np.nonzero((gperm // SHARD_P) == c)[0]
        locs_c = gperm[nodes_c] - c * SHARD_P
        bl = np.full(SHARD_P, -1.0, np.float32)
        bl[locs_c] = batch[nodes_c].astype(np.float32)
        rg_n = np.zeros(SHARD_P, np.float32)
        rg_n[locs_c] = rgc[batch[nodes_c]]
        d["batchloc"] = bl.reshape(NWIN, 128).T.copy()
        d["rgc_pn"] = rg_n.reshape(NWIN, 128).T.copy()
        per_core.append(d)

    # replicated weights; f-gate halves pre-negated
    wf = np.asarray(inputs["conv_wf"], np.float32)
    wsv = np.asarray(inputs["conv_ws"], np.float32)
    bf = np.asarray(inputs["conv_bf"], np.float32)
    bs = np.asarray(inputs["conv_bs"], np.float32)
    wdst = np.concatenate([-wf[:, 0:D1, :], wsv[:, 0:D1, :]], axis=2)
    wsrc = np.concatenate([-wf[:, D1:2 * D1, :], wsv[:, D1:2 * D1, :]], axis=2)
    wea = np.concatenate([-wf[:, 2 * D1:, :], wsv[:, 2 * D1:, :]], axis=2)
    bias = np.concatenate([-bf, bs], axis=1)[:, None, :]
    wea = np.concatenate([wea, bias], axis=1)
    shared = {
        "lin0w": np.asarray(inputs["lin0_w"], np.float32).astype(ml_dtypes.bfloat16),
        "lin0b": np.asarray(inputs["lin0_b"], np.float32).reshape(D1, 1),
        "wdst": np.transpose(wdst, (1, 0, 2)).reshape(D1, L * 128).astype(ml_dtypes.bfloat16),
        "wsrc": np.transpose(wsrc, (1, 0, 2)).reshape(D1, L * 128).astype(ml_dtypes.bfloat16),
        "wea": np.concatenate([
            np.zeros((1, L * 128), np.float32),
            np.transpose(wea, (1, 0, 2)).reshape(EF + 1, L * 128),
            np.zeros((64 - EF - 2, L * 128), np.float32),
        ], axis=0).astype(ml_dtypes.bfloat16),
        "bng": np.asarray(inputs["bn_gamma"], np.float32).T.copy(),
        "bnb": np.asarray(inputs["bn_beta"], np.float32).T.copy(),
        "lin1w": np.asarray(inputs["lin1_w"], np.float32),
        "lin1b": np.asarray(inputs["lin1_b"], np.float32).reshape(D2, 1),
        "fcw": np.transpose(np.asarray(inputs["fc_w"], np.float32), (1, 0, 2)).reshape(D2, FC * D2),
        "fcb": np.asarray(inputs["fc_b"], np.float32).T.copy(),
        "lin2w": np.asarray(inputs["lin2_w"], np.float32).reshape(D2, 1),
        "lin2b": np.asarray(inputs["lin2_b"], np.float32).reshape(1, 1),
        "iota128": np.broadcast_to(np.arange(128, dtype=np.float32)[None, :],
                                   (128, 128)).astype(ml_dtypes.bfloat16),
        "iota256": np.broadcast_to(np.arange(G, dtype=np.float32)[None, :],
                                   (128, G)).astype(ml_dtypes.bfloat16),
        "ident": np.eye(128, dtype=np.float32),
        "identb": np.eye(128, dtype=np.float32).astype(ml_dtypes.bfloat16),
    }
    in_maps = [dict(shared, **pc) for pc in per_core]
    return in_maps, cw


def kernel(**inputs):
    from concourse.bass_utils import run_bass_kernel_spmd

    in_maps, cw = _preprocess(inputs)
    key = ("nc", cw)
    if key not in _CACHE:
        _CACHE[key] = _build_nc(cw)
    nc = _CACHE[key]
    res = run_bass_kernel_spmd(nc, in_maps, core_ids=list(range(NCORES)))
    return res.results[0]["y"].reshape(G).astype(np.float32)


# revision 61
# speedup vs baseline: 1.0151x; 1.0002x over previous
"""CGCNN message-passing kernel for 8 Trainium2 NeuronCores (Bass/Tile), v9.

Data-parallel by dst shard; gather-based edge pipeline:
- Host: nodes are dealt into 240 global windows (8 cores x 30 windows x 128
  slots) in descending-degree snake order, equalizing per-window edge counts
  so the uniform chunks-per-window pad cw is minimal (16). Edges go to the
  core owning their dst, grouped by dst window, chunk-padded to cw.
- lin0 is computed for ALL nodes redundantly on every core (from a replicated
  full xT) into a DRAM fp8 table, so layer 0 needs no h AllGather; layers
  1..L-1 AllGather h in fp8 (staged at the previous layer's BN boundary so
  the collective launches as early as possible).
- Per layer, per core:
  * Qd table (own shard, SBUF bf16 [128, 30, 128]) = h_own @ Wdst.
  * Full Qs table = h_full @ Wsrc into DRAM [30720, 128] bf16 in
    partition-major row order (node g -> row (g%128)*240 + g//128); PSUM->SBUF
    staging copies round-robin over DVE/ACT to minimize build latency.
  * Per 1024-edge tile: one dma_gather pulls per-edge Qs rows (1024 x 256B
    descriptors; 1024 = SWDGE ring capacity); Qe = ea(fp8) @ Wea by matmul
    (edge attrs streamed fp8, 4 tiles per DMA); the dst contribution expands
    via a host-precomputed one-hot (fp8, SBUF-resident, layer-invariant)
    matmul against the SBUF Qd table. All three accumulate in PSUM per
    128-edge chunk.
  * Nonlinearity: joint exp u = [e^-a | e^b] (f-gate weights pre-negated),
    v = ln(1+u); 1/3 of tiles compute sigmoid(a) = e^-v_f on ACT, 2/3 as
    1/(1+u_f) on DVE (bf16, engine balance). m = 2*sigmoid(a)*softplus(b);
    the factor 2 is absorbed exactly by BatchNorm using 4*EPS.
  * Aggregation one-hots (is_equal(iota, dst) * 1/cnt, bf16 on DVE) are
    pre-built per tile, and the aggregation matmuls are deferred by one tile
    so the in-order PE queue never stalls on the ACT/DVE nonlinearity chain.
  * Segment-mean accumulates per dst window in PSUM (agg PSUM shares banks
    with the build-phase staging, freeing a third pre-PSUM buffer); BatchNorm
    batch stats via a tiny stats AllGather + local sum; the residual
    (scalar_tensor_tensor + relu) is computed in halves, with the fp8
    AllGather payload produced on ACT in parallel with the f32/bf16 h copies
    on DVE/Pool.
- Global mean pool via one-hot matmul, bf16 partials AllGathered and summed
  locally, head MLP computed redundantly on every core.
"""
import numpy as np
import ml_dtypes

N = 30000
E = 480000
NF = 92
EF = 50
D1 = 64
D2 = 64
L = 3
FC = 2
G = 256
EPS = 1e-5
NCORES = 8
SHARD = N // NCORES            # 3750
SHARD_P = 3840                 # padded shard (30 windows of 128)
NWIN = SHARD_P // 128          # 30
NWING = NCORES * NWIN          # 240 global windows
TBL = NCORES * SHARD_P         # 30720 table rows

_CACHE = {}



def _build_nc(cw):
    """Build the SPMD bass module. cw = chunks per dst window (uniform)."""
    import concourse.mybir as mybir
    from concourse import bacc
    from concourse.tile import TileContext

    f32 = mybir.dt.float32
    bf16 = mybir.dt.bfloat16
    f8 = mybir.dt.float8e4
    i16 = mybir.dt.int16
    AF = mybir.ActivationFunctionType
    OP = mybir.AluOpType

    nchunk = NWIN * cw                 # chunks per layer per core
    etot = nchunk * 128                # padded edges per core
    ntile = (nchunk + 7) // 8          # 8-chunk (1024-edge) PSUM tiles

    import concourse.hw_specs as _hw
    import concourse.bacc as _bacc_mod
    _real_tables = _hw.get_activation_tables("gen3")
    _combined = "natural_log_exp_and_others"
    if _combined in _real_tables:
        _patched = {
            k: (v if k == _combined else (v - {AF.Exp, AF.Ln}))
            for k, v in _real_tables.items()
        }
        _bacc_mod.get_activation_tables = lambda arch: _patched

    nc = bacc.Bacc(None, target_bir_lowering=False)

    # ---- inputs (per core) ----
    xTF = nc.dram_tensor("xTF", [NF, TBL], bf16, kind="ExternalInput")
    xT = nc.dram_tensor("xT", [NF, SHARD_P], bf16, kind="ExternalInput")
    eaT = nc.dram_tensor("eaT", [64, etot], f8, kind="ExternalInput")
    qs_idxD = nc.dram_tensor("qs_idxD", [128, etot // 16], i16, kind="ExternalInput")
    ohTD = nc.dram_tensor("ohTD", [128, etot], f8, kind="ExternalInput")
    dstloc_p = nc.dram_tensor("dstloc_p", [128, nchunk], f32, kind="ExternalInput")
    rc_p = nc.dram_tensor("rc_p", [128, nchunk], f32, kind="ExternalInput")
    batchloc = nc.dram_tensor("batchloc", [128, NWIN], f32, kind="ExternalInput")
    rgc_pn = nc.dram_tensor("rgc_pn", [128, NWIN], f32, kind="ExternalInput")
    # weights (replicated; f-gate halves pre-negated)
    lin0w = nc.dram_tensor("lin0w", [NF, D1], bf16, kind="ExternalInput")
    lin0b = nc.dram_tensor("lin0b", [D1, 1], f32, kind="ExternalInput")
    wdst = nc.dram_tensor("wdst", [D1, L * 128], bf16, kind="ExternalInput")
    wsrc = nc.dram_tensor("wsrc", [D1, L * 128], bf16, kind="ExternalInput")
    wea = nc.dram_tensor("wea", [64, L * 128], bf16, kind="ExternalInput")
    bng = nc.dram_tensor("bng", [D1, L + 1], f32, kind="ExternalInput")
    bnb = nc.dram_tensor("bnb", [D1, L], f32, kind="ExternalInput")
    lin1w = nc.dram_tensor("lin1w", [D1, D2], f32, kind="ExternalInput")
    lin1b = nc.dram_tensor("lin1b", [D2, 1], f32, kind="ExternalInput")
    fcw = nc.dram_tensor("fcw", [D2, FC * D2], f32, kind="ExternalInput")
    fcb = nc.dram_tensor("fcb", [D2, FC], f32, kind="ExternalInput")
    lin2w = nc.dram_tensor("lin2w", [D2, 1], f32, kind="ExternalInput")
    lin2b = nc.dram_tensor("lin2b", [1, 1], f32, kind="ExternalInput")
    iota128 = nc.dram_tensor("iota128", [128, 128], bf16, kind="ExternalInput")
    iota256 = nc.dram_tensor("iota256", [128, G], bf16, kind="ExternalInput")
    ident = nc.dram_tensor("ident", [128, 128], f32, kind="ExternalInput")
    identb = nc.dram_tensor("identb", [128, 128], bf16, kind="ExternalInput")

    yout = nc.dram_tensor("y", [1, G], f32, kind="ExternalOutput")

    # ---- DRAM scratch ----
    QsD = nc.dram_tensor("QsD", [TBL, 128], bf16)          # row p*NWING+W
    h1f8 = nc.dram_tensor("h1f8", [NCORES * D1, SHARD_P], f8)
    ag_in = nc.dram_tensor("ag_in", [D1, SHARD_P], f8)
    ag_out = nc.dram_tensor("ag_out", [NCORES * D1, SHARD_P], f8,
                            addr_space="Shared")
    ar_in = nc.dram_tensor("ar_in", [D1, 2], f32)
    ar_out = nc.dram_tensor("ar_out", [NCORES * D1, 2], f32, addr_space="Shared")
    pl_in = nc.dram_tensor("pl_in", [D1, G], bf16)
    pl_out = nc.dram_tensor("pl_out", [NCORES * D1, G], bf16, addr_space="Shared")

    rg = [list(range(NCORES))]
    QsD3 = QsD[:, :].rearrange("(p w) f -> p w f", p=128)   # [128, NWING, 128]

    from contextlib import ExitStack
    with TileContext(nc) as tc:
        with ExitStack() as _es:
            cp = _es.enter_context(tc.tile_pool(name="const", bufs=1))
            bigp = _es.enter_context(tc.tile_pool(name="big", bufs=1))
            wp = _es.enter_context(tc.tile_pool(name="work", bufs=3))
            tlp = _es.enter_context(tc.tile_pool(name="tail", bufs=1))
            gp = _es.enter_context(tc.tile_pool(name="gat", bufs=4))
            ep = _es.enter_context(tc.tile_pool(name="ea", bufs=3))
            nlp = _es.enter_context(tc.tile_pool(name="nl", bufs=3))
            ohp = _es.enter_context(tc.tile_pool(name="oh", bufs=13))
            ohgp = _es.enter_context(tc.tile_pool(name="ohg", bufs=5))
            stp = _es.enter_context(tc.tile_pool(name="st", bufs=2))
            sgp = _es.enter_context(tc.tile_pool(name="sgp", bufs=3))
            scp = _es.enter_context(tc.tile_pool(name="scr", bufs=1))
            pp = _es.enter_context(tc.tile_pool(name="pre", bufs=3, space="PSUM"))
            ppB = _es.enter_context(tc.tile_pool(name="psB", bufs=2, space="PSUM"))
            # ---------- constants ----------
            def load_const(t, dram, shape, dtype=f32):
                tt = cp.tile(shape, dtype, tag=t)
                nc.sync.dma_start(out=tt[:], in_=dram)
                return tt

            l0w = load_const("l0w", lin0w[:, :], [NF, D1], bf16)
            l0b = load_const("l0b", lin0b[:, :], [D1, 1])

            # ---------- resident state ----------
            hT_own = bigp.tile([D1, SHARD_P], f32, tag="hown")
            hb_own = bigp.tile([D1, SHARD_P], bf16, tag="hbown")
            aggr_sb = bigp.tile([D1, SHARD_P], bf16, tag="aggr")
            qd_sb = bigp.tile([128, NWIN, 128], bf16, tag="qdsb")
            asb = scp.tile([D1, SHARD_P], f32, tag="asb")

            # ---------- lin0 for ALL nodes (no AllGather for layer 0) ----
            # h1f8 holds relu(x @ lin0_w + b) for all 8 shards (global order),
            # computed redundantly on every core from the replicated xTF.
            HL = SHARD_P // 2
            for s_ in range(NCORES):
                h8s = stp.tile([D1, SHARD_P], f8, tag="h8")
                for hh in range(2):
                    xt = sgp.tile([NF, HL], bf16, tag="qsst")
                    o = s_ * SHARD_P + hh * HL
                    nc.sync.dma_start(out=xt[:], in_=xTF[:, o:o + HL])
                    for j in range(4):
                        sl = slice(j * 480, (j + 1) * 480)
                        ph = ppB.tile([D1, 512], f32, tag="bld")
                        nc.tensor.matmul(out=ph[:, :480], lhsT=l0w[:],
                                         rhs=xt[:, sl], start=True, stop=True)
                        osl = slice(hh * HL + j * 480, hh * HL + (j + 1) * 480)
                        if j % 2 == 0:
                            nc.scalar.activation(
                                out=h8s[:, osl],
                                in_=ph[:, :480], func=AF.Relu, bias=l0b[:],
                                scale=1.0)
                        else:
                            nc.vector.tensor_scalar(
                                out=h8s[:, osl], in0=ph[:, :480],
                                scalar1=l0b[:], scalar2=0.0,
                                op0=OP.add, op1=OP.max)
                nc.sync.dma_start(out=h1f8[s_ * D1:(s_ + 1) * D1, :], in_=h8s[:])

            # own-shard h in f32 from the per-core xT input
            for hh in range(2):
                xt0 = sgp.tile([NF, HL], bf16, tag="qsst")
                nc.sync.dma_start(out=xt0[:], in_=xT[:, hh * HL:(hh + 1) * HL])
                for j in range(4):
                    sl = slice(hh * HL + j * 480, hh * HL + (j + 1) * 480)
                    ph = ppB.tile([D1, 512], f32, tag="bld")
                    nc.tensor.matmul(out=ph[:, :480], lhsT=l0w[:],
                                     rhs=xt0[:, j * 480:(j + 1) * 480],
                                     start=True, stop=True)
                    nc.scalar.activation(out=hT_own[:, sl], in_=ph[:, :480],
                                         func=AF.Relu, bias=l0b[:], scale=1.0)
                    nc.vector.tensor_scalar(
                        out=hb_own[:, sl], in0=ph[:, :480],
                        scalar1=l0b[:], scalar2=0.0, op0=OP.add, op1=OP.max)

            # remaining constants: emitted after lin0 so their DMA (notably
            # the 60KB/partition one-hot + gather indices) doesn't serialize
            # ahead of the xTF streams in the SP/DMA queues
            ws = load_const("ws", wsrc[:, :], [D1, L * 128], bf16)
            wd = load_const("wd", wdst[:, :], [D1, L * 128], bf16)
            we = load_const("we", wea[:, :], [64, L * 128], bf16)
            io128 = load_const("io128", iota128[:, :], [128, 128], bf16)
            idnb = load_const("idnb", identb[:, :], [128, 128], bf16)
            dlp = load_const("dlp", dstloc_p[:, :], [128, nchunk])
            rcp = load_const("rcp", rc_p[:, :], [128, nchunk])
            gmt = load_const("gmt", bng[:, :], [D1, L + 1])
            bbt = load_const("bbt", bnb[:, :], [D1, L])
            io256 = load_const("io256", iota256[:, :], [128, G], bf16)
            idn = load_const("idn", ident[:, :], [128, 128])
            blc = load_const("blc", batchloc[:, :], [128, NWIN])
            rgp = load_const("rgp", rgc_pn[:, :], [128, NWIN])
            l1w = load_const("l1w", lin1w[:, :], [D1, D2])
            l1b = load_const("l1b", lin1b[:, :], [D2, 1])
            fw = load_const("fw", fcw[:, :], [D2, FC * D2])
            fb = load_const("fb", fcb[:, :], [D2, FC])
            l2w = load_const("l2w", lin2w[:, :], [D2, 1])
            l2b = load_const("l2b", lin2b[:, :], [1, 1])
            qsix = load_const("qsix", qs_idxD[:, :], [128, etot // 16], i16)
            ohT_res = cp.tile([128, nchunk, 128], f8, tag="ohres")
            nc.sync.dma_start(
                out=ohT_res[:].rearrange("p a b -> p (a b)"), in_=ohTD[:, :])

            # ---------- layers ----------
            for l in range(L):
                wd_l = wd[:, l * 128:(l + 1) * 128]
                ws_l = ws[:, l * 128:(l + 1) * 128]
                we_l = we[:, l * 128:(l + 1) * 128]

                if l == 0:
                    src_dram = h1f8
                else:
                    # --- AllGather h (fp8, staged into ag_in at layer end) ---
                    nc.gpsimd.collective_compute(
                        "AllGather", OP.bypass, replica_groups=rg,
                        ins=[ag_in.ap().opt()], outs=[ag_out.ap().opt()])
                    src_dram = ag_out

                # --- Qd table build (own shard) ---
                for w0 in range(0, NWIN, 4):
                    kk = min(4, NWIN - w0)
                    qp = ppB.tile([128, 512], f32, tag="bld")
                    for k in range(kk):
                        w = w0 + k
                        nc.tensor.matmul(
                            out=qp[:, k * 128:(k + 1) * 128],
                            lhsT=hb_own[:, w * 128:(w + 1) * 128],
                            rhs=wd_l, start=True, stop=True)
                    nc.vector.tensor_copy(
                        out=qd_sb[:, w0:w0 + kk, :].rearrange("p a b -> p (a b)"),
                        in_=qp[:, :kk * 128])

                # --- Qs table build (all nodes, per gathered shard) -> QsD ---
                ws8 = stp.tile([D1, 128], f8, tag="ws8")
                nc.scalar.activation(out=ws8[:], in_=ws_l,
                                     func=AF.Identity, scale=1.0)
                ncopy = 0
                for s_ in range(NCORES):
                    hb_sh = stp.tile([D1, SHARD_P], f8, tag="h8")
                    nc.sync.dma_start(out=hb_sh[:],
                                      in_=src_dram[s_ * D1:(s_ + 1) * D1, :])
                    for wB in range(0, NWIN, 16):
                        kB = min(16, NWIN - wB)
                        sg_t = sgp.tile([128, 16, 128], bf16, tag="qsst")
                        for w0 in range(wB, wB + kB, 4):
                            kk = min(4, wB + kB - w0)
                            qp = ppB.tile([128, 512], f32, tag="bld")
                            for k in range(kk):
                                w = w0 + k
                                nc.tensor.matmul(
                                    out=qp[:, k * 128:(k + 1) * 128],
                                    lhsT=hb_sh[:, w * 128:(w + 1) * 128],
                                    rhs=ws8[:], start=True, stop=True)
                            dst_ap = sg_t[:, w0 - wB:w0 - wB + kk, :] \
                                .rearrange("p a b -> p (a b)")
                            eng = ncopy % 5
                            ncopy += 1
                            if eng in (0, 2, 4):
                                nc.vector.tensor_copy(
                                    out=dst_ap, in_=qp[:, :kk * 128])
                            else:
                                nc.scalar.activation(
                                    out=dst_ap, in_=qp[:, :kk * 128],
                                    func=AF.Identity, scale=1.0)
                        W0 = s_ * NWIN + wB
                        nc.sync.dma_start(out=QsD3[:, W0:W0 + kB, :],
                                          in_=sg_t[:, :kB, :])

                # --- edge pipeline ---
                st1g = wp.tile([D1, 8], f32, tag="st1g")
                st2g = wp.tile([D1, 8], f32, tag="st2g")
                agg = None
                qs_g = None
                aggst = {"agg": None}

                def emit_agg(m, ohs_t, t, te):
                    # aggregation for tile t, deferred one tile so the PE
                    # queue never stalls waiting for m
                    for c in range(te):
                        gc = t * 8 + c
                        w = gc // cw
                        if gc % (4 * cw) == 0:
                            agg_new = ppB.tile([D1, 512], f32, tag="bld")
                            aggst["agg"] = agg_new
                        agg = aggst["agg"]
                        nc.tensor.matmul(
                            out=agg[:, (w % 4) * 128:(w % 4 + 1) * 128],
                            lhsT=m[:, c, :], rhs=ohs_t[c][:],
                            start=(gc % cw == 0), stop=(gc % cw == cw - 1))
                        if gc % (4 * cw) == 4 * cw - 1 or gc == nchunk - 1:
                            grp = w // 4
                            lo = grp * 512
                            hi = min(lo + 512, SHARD_P)
                            nc.scalar.activation(
                                out=aggr_sb[:, lo:hi], in_=agg[:, :hi - lo],
                                func=AF.Identity, scale=1.0)
                            nc.vector.reduce_sum(
                                out=st1g[:, grp:grp + 1],
                                in_=aggr_sb[:, lo:hi],
                                axis=mybir.AxisListType.X)
                            sqg = nlp.tile([D1, 512], bf16, tag="sqg")
                            nc.vector.tensor_tensor(
                                out=sqg[:, :hi - lo], in0=aggr_sb[:, lo:hi],
                                in1=aggr_sb[:, lo:hi], op=OP.mult)
                            nc.vector.reduce_sum(
                                out=st2g[:, grp:grp + 1],
                                in_=sqg[:, :hi - lo],
                                axis=mybir.AxisListType.X)

                pend = []
                for t in range(ntile):
                    te = min(8, nchunk - t * 8)          # chunks this tile
                    ne = te * 128                        # edges this tile
                    if t % 4 == 0:
                        tc32 = min(32, nchunk - t * 8)
                        et = ep.tile([64, 4096], f8, tag="et")
                        nc.sync.dma_start(
                            out=et[:, :tc32 * 128],
                            in_=eaT[:, t * 1024: t * 1024 + tc32 * 128])
                    qs_g = gp.tile([128, 8, 128], bf16, tag="qsg")
                    nc.gpsimd.dma_gather(
                        qs_g[:, :te, :], QsD[:, :],
                        qsix[:, t * 64: t * 64 + te * 8],
                        te * 128, te * 128, 128)
                    half = 0
                    qhalf = (t % 4) * 8

                    # one-hot aggregation matrices: const-only deps, built
                    # ahead so the agg matmuls never wait on DVE
                    ohs_t = []
                    for c in range(te):
                        gc = t * 8 + c
                        oh_ = ohp.tile([128, 128], bf16, tag="ohS")
                        nc.vector.tensor_scalar(
                            out=oh_[:], in0=io128[:],
                            scalar1=dlp[:, gc:gc + 1], scalar2=rcp[:, gc:gc + 1],
                            op0=OP.is_equal, op1=OP.mult)
                        ohs_t.append(oh_)

                    pre = pp.tile([128, 1024], f32, tag="pre")
                    qs_f = qs_g[:].rearrange("p a b -> p (a b)")
                    for c in range(te):
                        gc = t * 8 + c
                        w = gc // cw
                        csl = slice(c * 128, (c + 1) * 128)
                        csl2 = slice((half + c) * 128, (half + c + 1) * 128)
                        csl4 = slice((qhalf + c) * 128, (qhalf + c + 1) * 128)
                        nc.tensor.matmul(out=pre[:, csl], lhsT=et[:, csl4],
                                         rhs=we_l, start=True, stop=False)
                        nc.tensor.matmul(out=pre[:, csl], lhsT=idnb[:],
                                         rhs=qs_f[:, csl2], start=False, stop=False)
                        nc.tensor.matmul(out=pre[:, csl], lhsT=ohT_res[:, gc, :],
                                         rhs=qd_sb[:, w, :], start=False, stop=True)

                    # nonlinearity: m = (1+tanh(a/2)) * softplus(b)
                    #             = 2*sigmoid(a)*softplus(b)  (2 absorbed by BN)
                    # nonlinearity: u = [e^-a | e^b], v = ln(1+u) = [sp(-a)|sp(b)]
                    # even tiles (ACT): sigma = e^-sp(-a); odd tiles (DVE):
                    # sigma = 1/(1+e^-a). m = 2*sigma*sp(b) (2 absorbed by BN
                    # via 4*EPS).
                    uf = nlp.tile([128, 8, 128], bf16, tag="uf")
                    nc.scalar.activation(
                        out=uf[:, :te, :].rearrange("p a b -> p (a b)"),
                        in_=pre[:, :ne], func=AF.Exp, scale=1.0)
                    m = nlp.tile([128, 8, 64], bf16, tag="m")
                    if t % 3 == 0:
                        vf = nlp.tile([128, 8, 128], bf16, tag="vf")
                        nc.scalar.activation(
                            out=vf[:, :te, :].rearrange("p a b -> p (a b)"),
                            in_=uf[:, :te, :].rearrange("p a b -> p (a b)"),
                            func=AF.Ln, bias=1.0, scale=1.0)
                        sg = nlp.tile([128, 8, 64], bf16, tag="sg")
                        nc.scalar.activation(out=sg[:, :te, :],
                                             in_=vf[:, :te, 0:64],
                                             func=AF.Exp, scale=-1.0)
                        nc.vector.scalar_tensor_tensor(
                            out=m[:, :te, :], in0=sg[:, :te, :], scalar=2.0,
                            in1=vf[:, :te, 64:128], op0=OP.mult, op1=OP.mult)
                    else:
                        vs = nlp.tile([128, 8, 64], bf16, tag="vs")
                        nc.scalar.activation(out=vs[:, :te, :],
                                             in_=uf[:, :te, 64:128],
                                             func=AF.Ln, bias=1.0, scale=1.0)
                        w1 = nlp.tile([128, 8, 64], bf16, tag="sg")
                        with nc.allow_low_precision(reason="sigmoid in bf16"):
                            nc.vector.tensor_scalar(out=w1[:, :te, :],
                                                    in0=uf[:, :te, 0:64],
                                                    scalar1=1.0, scalar2=None,
                                                    op0=OP.add)
                            nc.vector.reciprocal(out=w1[:, :te, :],
                                                 in_=w1[:, :te, :])
                        nc.vector.scalar_tensor_tensor(
                            out=m[:, :te, :], in0=w1[:, :te, :], scalar=2.0,
                            in1=vs[:, :te, :], op0=OP.mult, op1=OP.mult)

                    pend.append((m, ohs_t, t, te))
                    if len(pend) > 1:
                        emit_agg(*pend.pop(0))
                while pend:
                    emit_agg(*pend.pop(0))

                # --- BN stats + AllReduce ---
                st = wp.tile([D1, 2], f32, tag="st")
                nc.vector.reduce_sum(out=st[:, 0:1], in_=st1g[:],
                                     axis=mybir.AxisListType.X)
                nc.vector.reduce_sum(out=st[:, 1:2], in_=st2g[:],
                                     axis=mybir.AxisListType.X)
                nc.sync.dma_start(out=ar_in[:, :], in_=st[:])
                nc.gpsimd.collective_compute(
                    "AllGather", OP.bypass, replica_groups=rg,
                    ins=[ar_in.ap().opt()], outs=[ar_out.ap().opt()])
                stga = wp.tile([D1, 2, NCORES], f32, tag="stga")
                nc.sync.dma_start(
                    out=stga[:],
                    in_=ar_out[:, :].rearrange("(c p) s -> p s c", p=D1))
                stg = wp.tile([D1, 2], f32, tag="stg")
                nc.vector.reduce_sum(
                    out=stg[:].rearrange("p (s o) -> p s o", o=1),
                    in_=stga[:], axis=mybir.AxisListType.X)
                mu = wp.tile([D1, 1], f32, tag="mu")
                nc.vector.tensor_scalar(out=mu[:], in0=stg[:, 0:1],
                                        scalar1=1.0 / N, scalar2=None, op0=OP.mult)
                ex2 = wp.tile([D1, 1], f32, tag="ex2")
                nc.vector.tensor_scalar(out=ex2[:], in0=stg[:, 1:2],
                                        scalar1=1.0 / N, scalar2=None, op0=OP.mult)
                mu2 = wp.tile([D1, 1], f32, tag="mu2")
                nc.vector.tensor_tensor(out=mu2[:], in0=mu[:], in1=mu[:], op=OP.mult)
                var = wp.tile([D1, 1], f32, tag="var")
                nc.vector.tensor_tensor(out=var[:], in0=ex2[:], in1=mu2[:],
                                        op=OP.subtract)
                lv = wp.tile([D1, 1], f32, tag="lv")
                # m carries a factor 2 -> aggr/mu scale by 2, var by 4; using
                # 4*EPS (folded into the Ln bias) makes BN output exactly
                # match the reference.
                nc.scalar.activation(out=lv[:], in_=var[:], func=AF.Ln,
                                     bias=gmt[:, L:L + 1], scale=1.0)
                isd = wp.tile([D1, 1], f32, tag="isd")
                nc.scalar.activation(out=isd[:], in_=lv[:], func=AF.Exp, scale=-0.5)
                scale = wp.tile([D1, 1], f32, tag="scale")
                nc.vector.tensor_tensor(out=scale[:], in0=isd[:],
                                        in1=gmt[:, l:l + 1], op=OP.mult)
                mshift = wp.tile([D1, 1], f32, tag="mshift")
                nc.vector.tensor_tensor(out=mshift[:], in0=mu[:], in1=scale[:],
                                        op=OP.mult)
                shift = wp.tile([D1, 1], f32, tag="shift")
                nc.vector.tensor_tensor(out=shift[:], in0=bbt[:, l:l + 1],
                                        in1=mshift[:], op=OP.subtract)
                # h = relu((aggr*scale + h) + shift); the three consumers
                # (f32 residual, f8 AllGather payload, bf16 matmul copy) are
                # produced from asb concurrently on DVE/ACT/Pool
                HB = SHARD_P // 2
                for hh in range(2):
                    hsl = slice(hh * HB, (hh + 1) * HB)
                    nc.vector.scalar_tensor_tensor(
                        out=asb[:, hsl], in0=aggr_sb[:, hsl], scalar=scale[:],
                        in1=hT_own[:, hsl], op0=OP.mult, op1=OP.add)
                if l + 1 < L:
                    h8n = stp.tile([D1, SHARD_P], f8, tag="h8")
                    for hh in range(2):
                        hsl = slice(hh * HB, (hh + 1) * HB)
                        nc.scalar.activation(out=h8n[:, hsl], in_=asb[:, hsl],
                                             func=AF.Relu, bias=shift[:],
                                             scale=1.0)
                        nc.sync.dma_start(out=ag_in[:, hsl], in_=h8n[:, hsl])
                if l + 1 < L:
                    nc.vector.tensor_scalar(out=hT_own[:], in0=asb[:],
                                            scalar1=shift[:], scalar2=0.0,
                                            op0=OP.add, op1=OP.max)
                    nc.gpsimd.tensor_copy(out=hb_own[:], in_=hT_own[:])
                else:
                    # last layer: no AllGather payload to produce on ACT and
                    # no next Qd build needing hb_own; do the relu on ACT in
                    # halves (pipelined behind the stt halves on DVE) so the
                    # pool phase starts sooner
                    for hh in range(2):
                        hsl = slice(hh * HB, (hh + 1) * HB)
                        nc.scalar.activation(out=hT_own[:, hsl],
                                             in_=asb[:, hsl], func=AF.Relu,
                                             bias=shift[:], scale=1.0)

            # ---------- global mean pool ----------
            pool_ps = pp.tile([D1, G], f32, tag="pre")
            for w in range(NWIN):
                tp = ppB.tile([128, D1], f32, tag="bld")
                nc.tensor.transpose(out=tp[:], in_=hT_own[:, w * 128:(w + 1) * 128],
                                    identity=idn[0:D1, 0:D1])
                rows = wp.tile([128, D1], bf16, tag="rows")
                nc.vector.tensor_copy(out=rows[:], in_=tp[:])
                ohg = ohgp.tile([128, G], bf16, tag="ohg")
                nc.vector.tensor_scalar(
                    out=ohg[:], in0=io256[:],
                    scalar1=blc[:, w:w + 1], scalar2=rgp[:, w:w + 1],
                    op0=OP.is_equal, op1=OP.mult)
                nc.tensor.matmul(out=pool_ps[:], lhsT=rows[:], rhs=ohg[:],
                                 start=(w == 0), stop=(w == NWIN - 1))
            poolT = tlp.tile([D1, G], bf16, tag="poolT")
            nc.vector.tensor_copy(out=poolT[:], in_=pool_ps[:])
            nc.sync.dma_start(out=pl_in[:, :], in_=poolT[:])
            nc.gpsimd.collective_compute(
                "AllGather", OP.bypass, replica_groups=rg,
                ins=[pl_in.ap().opt()], outs=[pl_out.ap().opt()])
            pga = tlp.tile([D1, NCORES, G], bf16, tag="pga")
            nc.sync.dma_start(
                out=pga[:],
                in_=pl_out[:, :].rearrange("(c p) g -> p c g", p=D1))
            pg = tlp.tile([D1, G], f32, tag="pg")
            nc.vector.reduce_sum(
                out=pg[:].rearrange("p (g o) -> p g o", o=1),
                in_=pga[:].rearrange("p c g -> p g c"),
                axis=mybir.AxisListType.X)

            # ---------- head ----------
            a = pg
            hw_ = [(l1w[:], l1b[:]), (fw[:, 0:D2], fb[:, 0:1]), (fw[:, D2:2 * D2], fb[:, 1:2])]
            for (wt, bt) in hw_:
                ps = ppB.tile([D2, G], f32, tag="bld")
                nc.tensor.matmul(out=ps[:, 0:G], lhsT=wt, rhs=a[:], start=True, stop=True)
                an = tlp.tile([D2, G], f32, tag="an")
                nc.scalar.activation(out=an[:], in_=ps[:, 0:G], func=AF.Relu,
                                     bias=bt, scale=1.0)
                a = an
            ps = ppB.tile([1, G], f32, tag="bld")
            nc.tensor.matmul(out=ps[:, 0:G], lhsT=l2w[:], rhs=a[:], start=True, stop=True)
            yt = tlp.tile([1, G], f32, tag="yt")
            nc.scalar.activation(out=yt[:], in_=ps[:, 0:G], func=AF.Identity,
                                 bias=l2b[:], scale=1.0)
            nc.sync.dma_start(out=yout[:, :], in_=yt[:])

    nc.compile()
    return nc


def _wrap16(idx):
    """Flat idx list -> [128, n/16] int16: slot i at [i%16, i//16], replicated
    across the 8 Q7 cores."""
    a = idx.reshape(-1, 16).T.astype(np.int16)
    return np.tile(a, (8, 1))


def _preprocess(inputs):
    x = np.asarray(inputs["x"], np.float32)
    ea = np.asarray(inputs["edge_attr"], np.float32)
    ei = np.asarray(inputs["edge_index"]).astype(np.int64)
    batch = np.asarray(inputs["batch"]).astype(np.int64)
    src, dst = ei[0], ei[1]

    cnt = np.bincount(dst, minlength=N).astype(np.float32)
    rc_node = 1.0 / np.maximum(cnt, 1.0)
    gcnt = np.bincount(batch, minlength=G).astype(np.float32)
    rgc = 1.0 / np.maximum(gcnt, 1.0)

    # Degree-balanced node -> (window, slot) assignment: snake-deal nodes in
    # descending-degree order across the 240 global windows, minimizing the
    # max per-window edge count (which sets the uniform chunk pad cw).
    deg_order = np.argsort(-cnt, kind="stable")       # node ids, deg desc
    nwin_g = NCORES * NWIN                            # 240
    perm_loc = np.empty(N, np.int64)                  # node -> global padded id
    for i0 in range(0, N, nwin_g):
        blk = deg_order[i0:i0 + nwin_g]
        j = i0 // nwin_g
        wins = np.arange(len(blk)) if j % 2 == 0 else (len(blk) - 1 - np.arange(len(blk)))
        w_ids = wins
        perm_loc[blk] = (w_ids // NWIN) * SHARD_P + (w_ids % NWIN) * 128 + j
    gperm = perm_loc
    srcg = gperm[src]
    dstg = gperm[dst]
    order = np.argsort(dstg, kind="stable")
    srcg_s, dstg_s, ea_idx = srcg[order], dstg[order], order

    bounds = []
    for c in range(NCORES):
        for w in range(NWIN):
            bounds.append(c * SHARD_P + w * 128)
    bounds.append(NCORES * SHARD_P)
    bpos = np.searchsorted(dstg_s, np.asarray(bounds), side="left")
    percw = {}
    maxcnt = 0
    k = 0
    for c in range(NCORES):
        for w in range(NWIN):
            lo, hi = bpos[k], bpos[k + 1]
            percw[(c, w)] = np.arange(lo, hi)
            maxcnt = max(maxcnt, hi - lo)
            k += 1
    cw = max(1, (maxcnt + 127) // 128)
    etot = NWIN * cw * 128

    # full padded x, rotated per core so block 0 is the own shard
    xfull = np.zeros((NF, NCORES * SHARD_P), np.float32)
    xfull[:, gperm] = x.T
    xfull = xfull.astype(ml_dtypes.bfloat16)

    per_core = []
    for c in range(NCORES):
        qs_idx = np.zeros(etot, np.int64)
        dl = np.full(etot, -1.0, np.float32)
        rc_e = np.ones(etot, np.float32)
        ea_e = np.zeros((etot, EF), np.float32)
        for w in range(NWIN):
            idxs = percw[(c, w)]
            o = w * cw * 128
            k = len(idxs)
            g = srcg_s[idxs]                           # padded global id
            qs_idx[o:o + k] = (g % 128) * NWING + (g // 128)
            loc = dstg_s[idxs] - c * SHARD_P           # 0..3839
            dl[o:o + k] = (loc - w * 128).astype(np.float32)
            rc_e[o:o + k] = rc_node[dst[ea_idx[idxs]]]
            ea_e[o:o + k] = ea[ea_idx[idxs]]
        eaT = np.zeros((64, etot), np.float32)
        eaT[1:EF + 1] = ea_e.T
        eaT[EF + 1] = 1.0
        eaT[EF + 1, dl < 0] = 0.0
        nch = etot // 128
        ohT = np.zeros((128, etot), np.float32)
        vv = dl >= 0
        ohT[dl[vv].astype(np.int64), np.nonzero(vv)[0]] = 1.0
        d = {
            "qs_idxD": _wrap16(qs_idx),
            "ohTD": ohT.astype(ml_dtypes.float8_e4m3),
            "dstloc_p": dl.reshape(nch, 128).T.copy(),
            "rc_p": rc_e.reshape(nch, 128).T.copy(),
            "eaT": eaT.astype(ml_dtypes.float8_e4m3),
        }
        d["xTF"] = xfull
        d["xT"] = xfull[:, c * SHARD_P:(c + 1) * SHARD_P].copy()
        nodes_c = np.nonzero((gperm // SHARD_P) == c)[0]
        locs_c = gperm[nodes_c] - c * SHARD_P
        bl = np.full(SHARD_P, -1.0, np.float32)
        bl[locs_c] = batch[nodes_c].astype(np.float32)
        rg_n = np.zeros(SHARD_P, np.float32)
        rg_n[locs_c] = rgc[batch[nodes_c]]
        d["batchloc"] = bl.reshape(NWIN, 128).T.copy()
        d["rgc_pn"] = rg_n.reshape(NWIN, 128).T.copy()
        per_core.append(d)

    # replicated weights; f-gate halves pre-negated
    wf = np.asarray(inputs["conv_wf"], np.float32)
    wsv = np.asarray(inputs["conv_ws"], np.float32)
    bf = np.asarray(inputs["conv_bf"], np.float32)
    bs = np.asarray(inputs["conv_bs"], np.float32)
    wdst = np.concatenate([-wf[:, 0:D1, :], wsv[:, 0:D1, :]], axis=2)
    wsrc = np.concatenate([-wf[:, D1:2 * D1, :], wsv[:, D1:2 * D1, :]], axis=2)
    wea = np.concatenate([-wf[:, 2 * D1:, :], wsv[:, 2 * D1:, :]], axis=2)
    bias = np.concatenate([-bf, bs], axis=1)[:, None, :]
    wea = np.concatenate([wea, bias], axis=1)
    shared = {
        "lin0w": np.asarray(inputs["lin0_w"], np.float32).astype(ml_dtypes.bfloat16),
        "lin0b": np.asarray(inputs["lin0_b"], np.float32).reshape(D1, 1),
        "wdst": np.transpose(wdst, (1, 0, 2)).reshape(D1, L * 128).astype(ml_dtypes.bfloat16),
        "wsrc": np.transpose(wsrc, (1, 0, 2)).reshape(D1, L * 128).astype(ml_dtypes.bfloat16),
        "wea": np.concatenate([
            np.zeros((1, L * 128), np.float32),
            np.transpose(wea, (1, 0, 2)).reshape(EF + 1, L * 128),
            np.zeros((64 - EF - 2, L * 128), np.float32),
        ], axis=0).astype(ml_dtypes.bfloat16),
        "bng": np.concatenate(
            [np.asarray(inputs["bn_gamma"], np.float32).T,
             np.full((D1, 1), 4.0 * EPS, np.float32)], axis=1),
        "bnb": np.asarray(inputs["bn_beta"], np.float32).T.copy(),
        "lin1w": np.asarray(inputs["lin1_w"], np.float32),
        "lin1b": np.asarray(inputs["lin1_b"], np.float32).reshape(D2, 1),
        "fcw": np.transpose(np.asarray(inputs["fc_w"], np.float32), (1, 0, 2)).reshape(D2, FC * D2),
        "fcb": np.asarray(inputs["fc_b"], np.float32).T.copy(),
        "lin2w": np.asarray(inputs["lin2_w"], np.float32).reshape(D2, 1),
        "lin2b": np.asarray(inputs["lin2_b"], np.float32).reshape(1, 1),
        "iota128": np.broadcast_to(np.arange(128, dtype=np.float32)[None, :],
                                   (128, 128)).astype(ml_dtypes.bfloat16),
        "iota256": np.broadcast_to(np.arange(G, dtype=np.float32)[None, :],
                                   (128, G)).astype(ml_dtypes.bfloat16),
        "ident": np.eye(128, dtype=np.float32),
        "identb": np.eye(128, dtype=np.float32).astype(ml_dtypes.bfloat16),
    }
    in_maps = [dict(shared, **pc) for pc in per_core]
    return in_maps, cw


def kernel(**inputs):
    from concourse.bass_utils import run_bass_kernel_spmd

    in_maps, cw = _preprocess(inputs)
    key = ("nc", cw)
    if key not in _CACHE:
        _CACHE[key] = _build_nc(cw)
    nc = _CACHE[key]
    res = run_bass_kernel_spmd(nc, in_maps, core_ids=list(range(NCORES)))
    return res.results[0]["y"].reshape(G).astype(np.float32)


# revision 63
# speedup vs baseline: 1.0152x; 1.0001x over previous
"""CGCNN message-passing kernel for 8 Trainium2 NeuronCores (Bass/Tile), v9.

Data-parallel by dst shard; gather-based edge pipeline:
- Host: nodes are dealt into 240 global windows (8 cores x 30 windows x 128
  slots) in descending-degree snake order, equalizing per-window edge counts
  so the uniform chunks-per-window pad cw is minimal (16). Edges go to the
  core owning their dst, grouped by dst window, chunk-padded to cw.
- lin0 is computed for ALL nodes redundantly on every core (from a replicated
  full xT) into a DRAM fp8 table, so layer 0 needs no h AllGather; layers
  1..L-1 AllGather h in fp8 (staged at the previous layer's BN boundary so
  the collective launches as early as possible).
- Per layer, per core:
  * Qd table (own shard, SBUF bf16 [128, 30, 128]) = h_own @ Wdst.
  * Full Qs table = h_full @ Wsrc into DRAM [30720, 128] bf16 in
    partition-major row order (node g -> row (g%128)*240 + g//128); PSUM->SBUF
    staging copies round-robin over DVE/ACT to minimize build latency.
  * Per 1024-edge tile: one dma_gather pulls per-edge Qs rows (1024 x 256B
    descriptors; 1024 = SWDGE ring capacity); Qe = ea(fp8) @ Wea by matmul
    (edge attrs streamed fp8, 4 tiles per DMA); the dst contribution expands
    via a host-precomputed one-hot (fp8, SBUF-resident, layer-invariant)
    matmul against the SBUF Qd table. All three accumulate in PSUM per
    128-edge chunk.
  * Nonlinearity: joint exp u = [e^-a | e^b] (f-gate weights pre-negated),
    v = ln(1+u); 1/3 of tiles compute sigmoid(a) = e^-v_f on ACT, 2/3 as
    1/(1+u_f) on DVE (bf16, engine balance). m = 2*sigmoid(a)*softplus(b);
    the factor 2 is absorbed exactly by BatchNorm using 4*EPS.
  * Aggregation one-hots (is_equal(iota, dst) * 1/cnt, bf16 on DVE) are
    pre-built per tile, and the aggregation matmuls are deferred by one tile
    so the in-order PE queue never stalls on the ACT/DVE nonlinearity chain.
  * Segment-mean accumulates per dst window in PSUM (agg PSUM shares banks
    with the build-phase staging, freeing a third pre-PSUM buffer); BatchNorm
    batch stats via a tiny stats AllGather + local sum; the residual
    (scalar_tensor_tensor + relu) is computed in halves, with the fp8
    AllGather payload produced on ACT in parallel with the f32/bf16 h copies
    on DVE/Pool.
- Global mean pool via one-hot matmul, bf16 partials AllGathered and summed
  locally, head MLP computed redundantly on every core.
"""
import numpy as np
import ml_dtypes

N = 30000
E = 480000
NF = 92
EF = 50
D1 = 64
D2 = 64
L = 3
FC = 2
G = 256
EPS = 1e-5
NCORES = 8
SHARD = N // NCORES            # 3750
SHARD_P = 3840                 # padded shard (30 windows of 128)
NWIN = SHARD_P // 128          # 30
NWING = NCORES * NWIN          # 240 global windows
TBL = NCORES * SHARD_P         # 30720 table rows

_CACHE = {}



def _build_nc(cw):
    """Build the SPMD bass module. cw = chunks per dst window (uniform)."""
    import concourse.mybir as mybir
    from concourse import bacc
    from concourse.tile import TileContext

    f32 = mybir.dt.float32
    bf16 = mybir.dt.bfloat16
    f8 = mybir.dt.float8e4
    i16 = mybir.dt.int16
    AF = mybir.ActivationFunctionType
    OP = mybir.AluOpType

    nchunk = NWIN * cw                 # chunks per layer per core
    etot = nchunk * 128                # padded edges per core
    ntile = (nchunk + 7) // 8          # 8-chunk (1024-edge) PSUM tiles

    import concourse.hw_specs as _hw
    import concourse.bacc as _bacc_mod
    _real_tables = _hw.get_activation_tables("gen3")
    _combined = "natural_log_exp_and_others"
    if _combined in _real_tables:
        _patched = {
            k: (v if k == _combined else (v - {AF.Exp, AF.Ln}))
            for k, v in _real_tables.items()
        }
        _bacc_mod.get_activation_tables = lambda arch: _patched

    nc = bacc.Bacc(None, target_bir_lowering=False)

    # ---- inputs (per core) ----
    xTF = nc.dram_tensor("xTF", [NF, TBL], bf16, kind="ExternalInput")
    xT = nc.dram_tensor("xT", [NF, SHARD_P], bf16, kind="ExternalInput")
    eaT = nc.dram_tensor("eaT", [64, etot], f8, kind="ExternalInput")
    qs_idxD = nc.dram_tensor("qs_idxD", [128, etot // 16], i16, kind="ExternalInput")
    ohTD = nc.dram_tensor("ohTD", [128, etot], f8, kind="ExternalInput")
    dstloc_p = nc.dram_tensor("dstloc_p", [128, nchunk], f32, kind="ExternalInput")
    rc_p = nc.dram_tensor("rc_p", [128, nchunk], f32, kind="ExternalInput")
    batchloc = nc.dram_tensor("batchloc", [128, NWIN], f32, kind="ExternalInput")
    rgc_pn = nc.dram_tensor("rgc_pn", [128, NWIN], f32, kind="ExternalInput")
    # weights (replicated; f-gate halves pre-negated)
    lin0w = nc.dram_tensor("lin0w", [NF, D1], bf16, kind="ExternalInput")
    lin0b = nc.dram_tensor("lin0b", [D1, 1], f32, kind="ExternalInput")
    wdst = nc.dram_tensor("wdst", [D1, L * 128], bf16, kind="ExternalInput")
    wsrc = nc.dram_tensor("wsrc", [D1, L * 128], bf16, kind="ExternalInput")
    wea = nc.dram_tensor("wea", [64, L * 128], bf16, kind="ExternalInput")
    bng = nc.dram_tensor("bng", [D1, L + 1], f32, kind="ExternalInput")
    bnb = nc.dram_tensor("bnb", [D1, L], f32, kind="ExternalInput")
    lin1w = nc.dram_tensor("lin1w", [D1, D2], f32, kind="ExternalInput")
    lin1b = nc.dram_tensor("lin1b", [D2, 1], f32, kind="ExternalInput")
    fcw = nc.dram_tensor("fcw", [D2, FC * D2], f32, kind="ExternalInput")
    fcb = nc.dram_tensor("fcb", [D2, FC], f32, kind="ExternalInput")
    lin2w = nc.dram_tensor("lin2w", [D2, 1], f32, kind="ExternalInput")
    lin2b = nc.dram_tensor("lin2b", [1, 1], f32, kind="ExternalInput")
    iota128 = nc.dram_tensor("iota128", [128, 128], bf16, kind="ExternalInput")
    iota256 = nc.dram_tensor("iota256", [128, G], bf16, kind="ExternalInput")
    ident = nc.dram_tensor("ident", [128, 128], f32, kind="ExternalInput")
    identb = nc.dram_tensor("identb", [128, 128], bf16, kind="ExternalInput")

    yout = nc.dram_tensor("y", [1, G], f32, kind="ExternalOutput")

    # ---- DRAM scratch ----
    QsD = nc.dram_tensor("QsD", [TBL, 128], bf16)          # row p*NWING+W
    h1f8 = nc.dram_tensor("h1f8", [NCORES * D1, SHARD_P], f8)
    ag_in = nc.dram_tensor("ag_in", [D1, SHARD_P], f8)
    ag_out = nc.dram_tensor("ag_out", [NCORES * D1, SHARD_P], f8,
                            addr_space="Shared")
    ar_in = nc.dram_tensor("ar_in", [D1, 2], f32)
    ar_out = nc.dram_tensor("ar_out", [NCORES * D1, 2], f32, addr_space="Shared")
    pl_in = nc.dram_tensor("pl_in", [D1, G], bf16)
    pl_out = nc.dram_tensor("pl_out", [NCORES * D1, G], bf16, addr_space="Shared")

    rg = [list(range(NCORES))]
    QsD3 = QsD[:, :].rearrange("(p w) f -> p w f", p=128)   # [128, NWING, 128]

    from contextlib import ExitStack
    with TileContext(nc) as tc:
        with ExitStack() as _es:
            cp = _es.enter_context(tc.tile_pool(name="const", bufs=1))
            bigp = _es.enter_context(tc.tile_pool(name="big", bufs=1))
            wp = _es.enter_context(tc.tile_pool(name="work", bufs=3))
            tlp = _es.enter_context(tc.tile_pool(name="tail", bufs=1))
            gp = _es.enter_context(tc.tile_pool(name="gat", bufs=4))
            ep = _es.enter_context(tc.tile_pool(name="ea", bufs=3))
            nlp = _es.enter_context(tc.tile_pool(name="nl", bufs=3))
            ohp = _es.enter_context(tc.tile_pool(name="oh", bufs=13))
            ohgp = _es.enter_context(tc.tile_pool(name="ohg", bufs=5))
            stp = _es.enter_context(tc.tile_pool(name="st", bufs=2))
            sgp = _es.enter_context(tc.tile_pool(name="sgp", bufs=3))
            scp = _es.enter_context(tc.tile_pool(name="scr", bufs=1))
            pp = _es.enter_context(tc.tile_pool(name="pre", bufs=3, space="PSUM"))
            ppB = _es.enter_context(tc.tile_pool(name="psB", bufs=2, space="PSUM"))
            # ---------- constants ----------
            def load_const(t, dram, shape, dtype=f32):
                tt = cp.tile(shape, dtype, tag=t)
                nc.sync.dma_start(out=tt[:], in_=dram)
                return tt

            l0w = load_const("l0w", lin0w[:, :], [NF, D1], bf16)
            l0b = load_const("l0b", lin0b[:, :], [D1, 1])

            # ---------- resident state ----------
            hT_own = bigp.tile([D1, SHARD_P], f32, tag="hown")
            hb_own = bigp.tile([D1, SHARD_P], bf16, tag="hbown")
            aggr_sb = bigp.tile([D1, SHARD_P], bf16, tag="aggr")
            qd_sb = bigp.tile([128, NWIN, 128], bf16, tag="qdsb")
            asb = scp.tile([D1, SHARD_P], f32, tag="asb")

            # ---------- lin0 for ALL nodes (no AllGather for layer 0) ----
            # h1f8 holds relu(x @ lin0_w + b) for all 8 shards (global order),
            # computed redundantly on every core from the replicated xTF.
            HL = SHARD_P // 2
            for s_ in range(NCORES):
                h8s = stp.tile([D1, SHARD_P], f8, tag="h8")
                for hh in range(2):
                    xt = sgp.tile([NF, HL], bf16, tag="qsst")
                    o = s_ * SHARD_P + hh * HL
                    nc.sync.dma_start(out=xt[:], in_=xTF[:, o:o + HL])
                    for j in range(4):
                        sl = slice(j * 480, (j + 1) * 480)
                        ph = ppB.tile([D1, 512], f32, tag="bld")
                        nc.tensor.matmul(out=ph[:, :480], lhsT=l0w[:],
                                         rhs=xt[:, sl], start=True, stop=True)
                        osl = slice(hh * HL + j * 480, hh * HL + (j + 1) * 480)
                        if j % 2 == 0:
                            nc.scalar.activation(
                                out=h8s[:, osl],
                                in_=ph[:, :480], func=AF.Relu, bias=l0b[:],
                                scale=1.0)
                        else:
                            nc.vector.tensor_scalar(
                                out=h8s[:, osl], in0=ph[:, :480],
                                scalar1=l0b[:], scalar2=0.0,
                                op0=OP.add, op1=OP.max)
                nc.sync.dma_start(out=h1f8[s_ * D1:(s_ + 1) * D1, :], in_=h8s[:])

            # own-shard h in f32 from the per-core xT input
            for hh in range(2):
                xt0 = sgp.tile([NF, HL], bf16, tag="qsst")
                nc.sync.dma_start(out=xt0[:], in_=xT[:, hh * HL:(hh + 1) * HL])
                for j in range(4):
                    sl = slice(hh * HL + j * 480, hh * HL + (j + 1) * 480)
                    ph = ppB.tile([D1, 512], f32, tag="bld")
                    nc.tensor.matmul(out=ph[:, :480], lhsT=l0w[:],
                                     rhs=xt0[:, j * 480:(j + 1) * 480],
                                     start=True, stop=True)
                    nc.scalar.activation(out=hT_own[:, sl], in_=ph[:, :480],
                                         func=AF.Relu, bias=l0b[:], scale=1.0)
                    nc.vector.tensor_scalar(
                        out=hb_own[:, sl], in0=ph[:, :480],
                        scalar1=l0b[:], scalar2=0.0, op0=OP.add, op1=OP.max)

            # remaining constants: emitted after lin0 so their DMA (notably
            # the 60KB/partition one-hot + gather indices) doesn't serialize
            # ahead of the xTF streams in the SP/DMA queues
            ws = load_const("ws", wsrc[:, :], [D1, L * 128], bf16)
            wd = load_const("wd", wdst[:, :], [D1, L * 128], bf16)
            we = load_const("we", wea[:, :], [64, L * 128], bf16)
            io128 = load_const("io128", iota128[:, :], [128, 128], bf16)
            idnb = load_const("idnb", identb[:, :], [128, 128], bf16)
            dlp = load_const("dlp", dstloc_p[:, :], [128, nchunk])
            rcp = load_const("rcp", rc_p[:, :], [128, nchunk])
            gmt = load_const("gmt", bng[:, :], [D1, L + 1])
            bbt = load_const("bbt", bnb[:, :], [D1, L])
            io256 = load_const("io256", iota256[:, :], [128, G], bf16)
            idn = load_const("idn", ident[:, :], [128, 128])
            blc = load_const("blc", batchloc[:, :], [128, NWIN])
            rgp = load_const("rgp", rgc_pn[:, :], [128, NWIN])
            l1w = load_const("l1w", lin1w[:, :], [D1, D2])
            l1b = load_const("l1b", lin1b[:, :], [D2, 1])
            fw = load_const("fw", fcw[:, :], [D2, FC * D2])
            fb = load_const("fb", fcb[:, :], [D2, FC])
            l2w = load_const("l2w", lin2w[:, :], [D2, 1])
            l2b = load_const("l2b", lin2b[:, :], [1, 1])
            qsix = load_const("qsix", qs_idxD[:, :], [128, etot // 16], i16)
            ohT_res = cp.tile([128, nchunk, 128], f8, tag="ohres")
            nc.sync.dma_start(
                out=ohT_res[:].rearrange("p a b -> p (a b)"), in_=ohTD[:, :])

            # ---------- layers ----------
            for l in range(L):
                wd_l = wd[:, l * 128:(l + 1) * 128]
                ws_l = ws[:, l * 128:(l + 1) * 128]
                we_l = we[:, l * 128:(l + 1) * 128]

                if l == 0:
                    src_dram = h1f8
                else:
                    # --- AllGather h (fp8, staged into ag_in at layer end) ---
                    nc.gpsimd.collective_compute(
                        "AllGather", OP.bypass, replica_groups=rg,
                        ins=[ag_in.ap().opt()], outs=[ag_out.ap().opt()])
                    src_dram = ag_out

                # --- Qd table build (own shard) ---
                for w0 in range(0, NWIN, 4):
                    kk = min(4, NWIN - w0)
                    qp = ppB.tile([128, 512], f32, tag="bld")
                    for k in range(kk):
                        w = w0 + k
                        nc.tensor.matmul(
                            out=qp[:, k * 128:(k + 1) * 128],
                            lhsT=hb_own[:, w * 128:(w + 1) * 128],
                            rhs=wd_l, start=True, stop=True)
                    nc.vector.tensor_copy(
                        out=qd_sb[:, w0:w0 + kk, :].rearrange("p a b -> p (a b)"),
                        in_=qp[:, :kk * 128])

                # --- Qs table build (all nodes, per gathered shard) -> QsD ---
                ws8 = stp.tile([D1, 128], f8, tag="ws8")
                nc.scalar.activation(out=ws8[:], in_=ws_l,
                                     func=AF.Identity, scale=1.0)
                ncopy = 0
                for s_ in range(NCORES):
                    hb_sh = stp.tile([D1, SHARD_P], f8, tag="h8")
                    nc.sync.dma_start(out=hb_sh[:],
                                      in_=src_dram[s_ * D1:(s_ + 1) * D1, :])
                    for wB in range(0, NWIN, 16):
                        kB = min(16, NWIN - wB)
                        sg_t = sgp.tile([128, 16, 128], bf16, tag="qsst")
                        for w0 in range(wB, wB + kB, 4):
                            kk = min(4, wB + kB - w0)
                            qp = ppB.tile([128, 512], f32, tag="bld")
                            for k in range(kk):
                                w = w0 + k
                                nc.tensor.matmul(
                                    out=qp[:, k * 128:(k + 1) * 128],
                                    lhsT=hb_sh[:, w * 128:(w + 1) * 128],
                                    rhs=ws8[:], start=True, stop=True)
                            dst_ap = sg_t[:, w0 - wB:w0 - wB + kk, :] \
                                .rearrange("p a b -> p (a b)")
                            eng = ncopy % 5
                            ncopy += 1
                            if eng in (0, 2, 4):
                                nc.vector.tensor_copy(
                                    out=dst_ap, in_=qp[:, :kk * 128])
                            else:
                                nc.scalar.activation(
                                    out=dst_ap, in_=qp[:, :kk * 128],
                                    func=AF.Identity, scale=1.0)
                        W0 = s_ * NWIN + wB
                        nc.sync.dma_start(out=QsD3[:, W0:W0 + kB, :],
                                          in_=sg_t[:, :kB, :])

                # --- edge pipeline ---
                st1g = wp.tile([D1, 8], f32, tag="st1g")
                st2g = wp.tile([D1, 8], f32, tag="st2g")
                agg = None
                qs_g = None
                aggst = {"agg": None}

                def emit_agg(m, ohs_t, t, te):
                    # aggregation for tile t, deferred one tile so the PE
                    # queue never stalls waiting for m
                    for c in range(te):
                        gc = t * 8 + c
                        w = gc // cw
                        if gc % (4 * cw) == 0:
                            agg_new = ppB.tile([D1, 512], f32, tag="bld")
                            aggst["agg"] = agg_new
                        agg = aggst["agg"]
                        nc.tensor.matmul(
                            out=agg[:, (w % 4) * 128:(w % 4 + 1) * 128],
                            lhsT=m[:, c, :], rhs=ohs_t[c][:],
                            start=(gc % cw == 0), stop=(gc % cw == cw - 1))
                        if gc % (4 * cw) == 4 * cw - 1 or gc == nchunk - 1:
                            grp = w // 4
                            lo = grp * 512
                            hi = min(lo + 512, SHARD_P)
                            nc.scalar.activation(
                                out=aggr_sb[:, lo:hi], in_=agg[:, :hi - lo],
                                func=AF.Identity, scale=1.0)
                            nc.vector.reduce_sum(
                                out=st1g[:, grp:grp + 1],
                                in_=aggr_sb[:, lo:hi],
                                axis=mybir.AxisListType.X)
                            sqg = nlp.tile([D1, 512], bf16, tag="sqg")
                            nc.vector.tensor_tensor(
                                out=sqg[:, :hi - lo], in0=aggr_sb[:, lo:hi],
                                in1=aggr_sb[:, lo:hi], op=OP.mult)
                            nc.vector.reduce_sum(
                                out=st2g[:, grp:grp + 1],
                                in_=sqg[:, :hi - lo],
                                axis=mybir.AxisListType.X)

                pend = []
                for t in range(ntile):
                    te = min(8, nchunk - t * 8)          # chunks this tile
                    ne = te * 128                        # edges this tile
                    if t % 4 == 0:
                        tc32 = min(32, nchunk - t * 8)
                        et = ep.tile([64, 4096], f8, tag="et")
                        nc.sync.dma_start(
                            out=et[:, :tc32 * 128],
                            in_=eaT[:, t * 1024: t * 1024 + tc32 * 128])
                    qs_g = gp.tile([128, 8, 128], bf16, tag="qsg")
                    nc.gpsimd.dma_gather(
                        qs_g[:, :te, :], QsD[:, :],
                        qsix[:, t * 64: t * 64 + te * 8],
                        te * 128, te * 128, 128)
                    half = 0
                    qhalf = (t % 4) * 8

                    # one-hot aggregation matrices: const-only deps, built
                    # ahead so the agg matmuls never wait on DVE
                    ohs_t = []
                    for c in range(te):
                        gc = t * 8 + c
                        oh_ = ohp.tile([128, 128], bf16, tag="ohS")
                        nc.vector.tensor_scalar(
                            out=oh_[:], in0=io128[:],
                            scalar1=dlp[:, gc:gc + 1], scalar2=rcp[:, gc:gc + 1],
                            op0=OP.is_equal, op1=OP.mult)
                        ohs_t.append(oh_)

                    pre = pp.tile([128, 1024], f32, tag="pre")
                    qs_f = qs_g[:].rearrange("p a b -> p (a b)")
                    for c in range(te):
                        gc = t * 8 + c
                        w = gc // cw
                        csl = slice(c * 128, (c + 1) * 128)
                        csl2 = slice((half + c) * 128, (half + c + 1) * 128)
                        csl4 = slice((qhalf + c) * 128, (qhalf + c + 1) * 128)
                        nc.tensor.matmul(out=pre[:, csl], lhsT=et[:, csl4],
                                         rhs=we_l, start=True, stop=False)
                        nc.tensor.matmul(out=pre[:, csl], lhsT=idnb[:],
                                         rhs=qs_f[:, csl2], start=False, stop=False)
                        nc.tensor.matmul(out=pre[:, csl], lhsT=ohT_res[:, gc, :],
                                         rhs=qd_sb[:, w, :], start=False, stop=True)

                    # nonlinearity: m = (1+tanh(a/2)) * softplus(b)
                    #             = 2*sigmoid(a)*softplus(b)  (2 absorbed by BN)
                    # nonlinearity: u = [e^-a | e^b], v = ln(1+u) = [sp(-a)|sp(b)]
                    # even tiles (ACT): sigma = e^-sp(-a); odd tiles (DVE):
                    # sigma = 1/(1+e^-a). m = 2*sigma*sp(b) (2 absorbed by BN
                    # via 4*EPS).
                    uf = nlp.tile([128, 8, 128], bf16, tag="uf")
                    nc.scalar.activation(
                        out=uf[:, :te, :].rearrange("p a b -> p (a b)"),
                        in_=pre[:, :ne], func=AF.Exp, scale=1.0)
                    m = nlp.tile([128, 8, 64], bf16, tag="m")
                    if t % 3 == 0:
                        vf = nlp.tile([128, 8, 128], bf16, tag="vf")
                        nc.scalar.activation(
                            out=vf[:, :te, :].rearrange("p a b -> p (a b)"),
                            in_=uf[:, :te, :].rearrange("p a b -> p (a b)"),
                            func=AF.Ln, bias=1.0, scale=1.0)
                        sg = nlp.tile([128, 8, 64], bf16, tag="sg")
                        nc.scalar.activation(out=sg[:, :te, :],
                                             in_=vf[:, :te, 0:64],
                                             func=AF.Exp, scale=-1.0)
                        nc.vector.scalar_tensor_tensor(
                            out=m[:, :te, :], in0=sg[:, :te, :], scalar=2.0,
                            in1=vf[:, :te, 64:128], op0=OP.mult, op1=OP.mult)
                    else:
                        vs = nlp.tile([128, 8, 64], bf16, tag="vs")
                        nc.scalar.activation(out=vs[:, :te, :],
                                             in_=uf[:, :te, 64:128],
                                             func=AF.Ln, bias=1.0, scale=1.0)
                        w1 = nlp.tile([128, 8, 64], bf16, tag="sg")
                        with nc.allow_low_precision(reason="sigmoid in bf16"):
                            nc.vector.tensor_scalar(out=w1[:, :te, :],
                                                    in0=uf[:, :te, 0:64],
                                                    scalar1=1.0, scalar2=None,
                                                    op0=OP.add)
                            nc.vector.reciprocal(out=w1[:, :te, :],
                                                 in_=w1[:, :te, :])
                        nc.vector.scalar_tensor_tensor(
                            out=m[:, :te, :], in0=w1[:, :te, :], scalar=2.0,
                            in1=vs[:, :te, :], op0=OP.mult, op1=OP.mult)

                    pend.append((m, ohs_t, t, te))
                    if len(pend) > 1:
                        emit_agg(*pend.pop(0))
                while pend:
                    emit_agg(*pend.pop(0))

                # --- BN stats + AllReduce ---
                st = wp.tile([D1, 2], f32, tag="st")
                nc.vector.reduce_sum(out=st[:, 0:1], in_=st1g[:],
                                     axis=mybir.AxisListType.X)
                nc.vector.reduce_sum(out=st[:, 1:2], in_=st2g[:],
                                     axis=mybir.AxisListType.X)
                nc.sync.dma_start(out=ar_in[:, :], in_=st[:])
                nc.gpsimd.collective_compute(
                    "AllGather", OP.bypass, replica_groups=rg,
                    ins=[ar_in.ap().opt()], outs=[ar_out.ap().opt()])
                stga = wp.tile([D1, 2, NCORES], f32, tag="stga")
                nc.sync.dma_start(
                    out=stga[:],
                    in_=ar_out[:, :].rearrange("(c p) s -> p s c", p=D1))
                stg = wp.tile([D1, 2], f32, tag="stg")
                nc.vector.reduce_sum(
                    out=stg[:].rearrange("p (s o) -> p s o", o=1),
                    in_=stga[:], axis=mybir.AxisListType.X)
                mu = wp.tile([D1, 1], f32, tag="mu")
                nc.vector.tensor_scalar(out=mu[:], in0=stg[:, 0:1],
                                        scalar1=1.0 / N, scalar2=None, op0=OP.mult)
                ex2 = wp.tile([D1, 1], f32, tag="ex2")
                nc.vector.tensor_scalar(out=ex2[:], in0=stg[:, 1:2],
                                        scalar1=1.0 / N, scalar2=None, op0=OP.mult)
                nvar = wp.tile([D1, 1], f32, tag="var")
                nc.vector.scalar_tensor_tensor(
                    out=nvar[:], in0=mu[:], scalar=mu[:], in1=ex2[:],
                    op0=OP.mult, op1=OP.subtract)
                lv = wp.tile([D1, 1], f32, tag="lv")
                # m carries a factor 2 -> aggr/mu scale by 2, var by 4; 4*EPS
                # rides as an extra bng column; nvar = mu^2-ex2 = -var, so the
                # Ln input is nvar*(-1) + 4*EPS.
                nc.scalar.activation(out=lv[:], in_=nvar[:], func=AF.Ln,
                                     bias=gmt[:, L:L + 1], scale=-1.0)
                isd = wp.tile([D1, 1], f32, tag="isd")
                nc.scalar.activation(out=isd[:], in_=lv[:], func=AF.Exp, scale=-0.5)
                scale = wp.tile([D1, 1], f32, tag="scale")
                nc.vector.tensor_tensor(out=scale[:], in0=isd[:],
                                        in1=gmt[:, l:l + 1], op=OP.mult)
                mshift = wp.tile([D1, 1], f32, tag="mshift")
                nc.vector.tensor_tensor(out=mshift[:], in0=mu[:], in1=scale[:],
                                        op=OP.mult)
                shift = wp.tile([D1, 1], f32, tag="shift")
                nc.vector.tensor_tensor(out=shift[:], in0=bbt[:, l:l + 1],
                                        in1=mshift[:], op=OP.subtract)
                # h = relu((aggr*scale + h) + shift); the three consumers
                # (f32 residual, f8 AllGather payload, bf16 matmul copy) are
                # produced from asb concurrently on DVE/ACT/Pool
                HB = SHARD_P // 2
                for hh in range(2):
                    hsl = slice(hh * HB, (hh + 1) * HB)
                    nc.vector.scalar_tensor_tensor(
                        out=asb[:, hsl], in0=aggr_sb[:, hsl], scalar=scale[:],
                        in1=hT_own[:, hsl], op0=OP.mult, op1=OP.add)
                if l + 1 < L:
                    h8n = stp.tile([D1, SHARD_P], f8, tag="h8")
                    for hh in range(2):
                        hsl = slice(hh * HB, (hh + 1) * HB)
                        nc.scalar.activation(out=h8n[:, hsl], in_=asb[:, hsl],
                                             func=AF.Relu, bias=shift[:],
                                             scale=1.0)
                        nc.sync.dma_start(out=ag_in[:, hsl], in_=h8n[:, hsl])
                if l + 1 < L:
                    nc.vector.tensor_scalar(out=hT_own[:], in0=asb[:],
                                            scalar1=shift[:], scalar2=0.0,
                                            op0=OP.add, op1=OP.max)
                    nc.gpsimd.tensor_copy(out=hb_own[:], in_=hT_own[:])
                else:
                    # last layer: no AllGather payload to produce on ACT and
                    # no next Qd build needing hb_own; do the relu on ACT in
                    # halves (pipelined behind the stt halves on DVE) so the
                    # pool phase starts sooner
                    for hh in range(2):
                        hsl = slice(hh * HB, (hh + 1) * HB)
                        nc.scalar.activation(out=hT_own[:, hsl],
                                             in_=asb[:, hsl], func=AF.Relu,
                                             bias=shift[:], scale=1.0)

            # ---------- global mean pool ----------
            pool_ps = pp.tile([D1, G], f32, tag="pre")
            for w in range(NWIN):
                tp = ppB.tile([128, D1], f32, tag="bld")
                nc.tensor.transpose(out=tp[:], in_=hT_own[:, w * 128:(w + 1) * 128],
                                    identity=idn[0:D1, 0:D1])
                rows = wp.tile([128, D1], bf16, tag="rows")
                nc.vector.tensor_copy(out=rows[:], in_=tp[:])
                ohg = ohgp.tile([128, G], bf16, tag="ohg")
                nc.vector.tensor_scalar(
                    out=ohg[:], in0=io256[:],
                    scalar1=blc[:, w:w + 1], scalar2=rgp[:, w:w + 1],
                    op0=OP.is_equal, op1=OP.mult)
                nc.tensor.matmul(out=pool_ps[:], lhsT=rows[:], rhs=ohg[:],
                                 start=(w == 0), stop=(w == NWIN - 1))
            poolT = tlp.tile([D1, G], bf16, tag="poolT")
            nc.vector.tensor_copy(out=poolT[:], in_=pool_ps[:])
            nc.sync.dma_start(out=pl_in[:, :], in_=poolT[:])
            nc.gpsimd.collective_compute(
                "AllGather", OP.bypass, replica_groups=rg,
                ins=[pl_in.ap().opt()], outs=[pl_out.ap().opt()])
            pga = tlp.tile([D1, NCORES, G], bf16, tag="pga")
            nc.sync.dma_start(
                out=pga[:],
                in_=pl_out[:, :].rearrange("(c p) g -> p c g", p=D1))
            pg = tlp.tile([D1, G], f32, tag="pg")
            nc.vector.reduce_sum(
                out=pg[:].rearrange("p (g o) -> p g o", o=1),
                in_=pga[:].rearrange("p c g -> p g c"),
                axis=mybir.AxisListType.X)

            # ---------- head ----------
            a = pg
            hw_ = [(l1w[:], l1b[:]), (fw[:, 0:D2], fb[:, 0:1]), (fw[:, D2:2 * D2], fb[:, 1:2])]
            for (wt, bt) in hw_:
                ps = ppB.tile([D2, G], f32, tag="bld")
                nc.tensor.matmul(out=ps[:, 0:G], lhsT=wt, rhs=a[:], start=True, stop=True)
                an = tlp.tile([D2, G], f32, tag="an")
                nc.scalar.activation(out=an[:], in_=ps[:, 0:G], func=AF.Relu,
                                     bias=bt, scale=1.0)
                a = an
            ps = ppB.tile([1, G], f32, tag="bld")
            nc.tensor.matmul(out=ps[:, 0:G], lhsT=l2w[:], rhs=a[:], start=True, stop=True)
            yt = tlp.tile([1, G], f32, tag="yt")
            nc.scalar.activation(out=yt[:], in_=ps[:, 0:G], func=AF.Identity,
                                 bias=l2b[:], scale=1.0)
            nc.sync.dma_start(out=yout[:, :], in_=yt[:])

    nc.compile()
    return nc


def _wrap16(idx):
    """Flat idx list -> [128, n/16] int16: slot i at [i%16, i//16], replicated
    across the 8 Q7 cores."""
    a = idx.reshape(-1, 16).T.astype(np.int16)
    return np.tile(a, (8, 1))


def _preprocess(inputs):
    x = np.asarray(inputs["x"], np.float32)
    ea = np.asarray(inputs["edge_attr"], np.float32)
    ei = np.asarray(inputs["edge_index"]).astype(np.int64)
    batch = np.asarray(inputs["batch"]).astype(np.int64)
    src, dst = ei[0], ei[1]

    cnt = np.bincount(dst, minlength=N).astype(np.float32)
    rc_node = 1.0 / np.maximum(cnt, 1.0)
    gcnt = np.bincount(batch, minlength=G).astype(np.float32)
    rgc = 1.0 / np.maximum(gcnt, 1.0)

    # Degree-balanced node -> (window, slot) assignment: snake-deal nodes in
    # descending-degree order across the 240 global windows, minimizing the
    # max per-window edge count (which sets the uniform chunk pad cw).
    deg_order = np.argsort(-cnt, kind="stable")       # node ids, deg desc
    nwin_g = NCORES * NWIN                            # 240
    perm_loc = np.empty(N, np.int64)                  # node -> global padded id
    for i0 in range(0, N, nwin_g):
        blk = deg_order[i0:i0 + nwin_g]
        j = i0 // nwin_g
        wins = np.arange(len(blk)) if j % 2 == 0 else (len(blk) - 1 - np.arange(len(blk)))
        w_ids = wins
        perm_loc[blk] = (w_ids // NWIN) * SHARD_P + (w_ids % NWIN) * 128 + j
    gperm = perm_loc
    srcg = gperm[src]
    dstg = gperm[dst]
    order = np.argsort(dstg, kind="stable")
    srcg_s, dstg_s, ea_idx = srcg[order], dstg[order], order

    bounds = []
    for c in range(NCORES):
        for w in range(NWIN):
            bounds.append(c * SHARD_P + w * 128)
    bounds.append(NCORES * SHARD_P)
    bpos = np.searchsorted(dstg_s, np.asarray(bounds), side="left")
    percw = {}
    maxcnt = 0
    k = 0
    for c in range(NCORES):
        for w in range(NWIN):
            lo, hi = bpos[k], bpos[k + 1]
            percw[(c, w)] = np.arange(lo, hi)
            maxcnt = max(maxcnt, hi - lo)
            k += 1
    cw = max(1, (maxcnt + 127) // 128)
    etot = NWIN * cw * 128

    # full padded x, rotated per core so block 0 is the own shard
    xfull = np.zeros((NF, NCORES * SHARD_P), np.float32)
    xfull[:, gperm] = x.T
    xfull = xfull.astype(ml_dtypes.bfloat16)

    per_core = []
    for c in range(NCORES):
        qs_idx = np.zeros(etot, np.int64)
        dl = np.full(etot, -1.0, np.float32)
        rc_e = np.ones(etot, np.float32)
        ea_e = np.zeros((etot, EF), np.float32)
        for w in range(NWIN):
            idxs = percw[(c, w)]
            o = w * cw * 128
            k = len(idxs)
            g = srcg_s[idxs]                           # padded global id
            qs_idx[o:o + k] = (g % 128) * NWING + (g // 128)
            loc = dstg_s[idxs] - c * SHARD_P           # 0..3839
            dl[o:o + k] = (loc - w * 128).astype(np.float32)
            rc_e[o:o + k] = rc_node[dst[ea_idx[idxs]]]
            ea_e[o:o + k] = ea[ea_idx[idxs]]
        eaT = np.zeros((64, etot), np.float32)
        eaT[1:EF + 1] = ea_e.T
        eaT[EF + 1] = 1.0
        eaT[EF + 1, dl < 0] = 0.0
        nch = etot // 128
        ohT = np.zeros((128, etot), np.float32)
        vv = dl >= 0
        ohT[dl[vv].astype(np.int64), np.nonzero(vv)[0]] = 1.0
        d = {
            "qs_idxD": _wrap16(qs_idx),
            "ohTD": ohT.astype(ml_dtypes.float8_e4m3),
            "dstloc_p": dl.reshape(nch, 128).T.copy(),
            "rc_p": rc_e.reshape(nch, 128).T.copy(),
            "eaT": eaT.astype(ml_dtypes.float8_e4m3),
        }
        d["xTF"] = xfull
        d["xT"] = xfull[:, c * SHARD_P:(c + 1) * SHARD_P].copy()
        nodes_c = np.nonzero((gperm // SHARD_P) == c)[0]
        locs_c = gperm[nodes_c] - c * SHARD_P
        bl = np.full(SHARD_P, -1.0, np.float32)
        bl[locs_c] = batch[nodes_c].astype(np.float32)
        rg_n = np.zeros(SHARD_P, np.float32)
        rg_n[locs_c] = rgc[batch[nodes_c]]
        d["batchloc"] = bl.reshape(NWIN, 128).T.copy()
        d["rgc_pn"] = rg_n.reshape(NWIN, 128).T.copy()
        per_core.append(d)

    # replicated weights; f-gate halves pre-negated
    wf = np.asarray(inputs["conv_wf"], np.float32)
    wsv = np.asarray(inputs["conv_ws"], np.float32)
    bf = np.asarray(inputs["conv_bf"], np.float32)
    bs = np.asarray(inputs["conv_bs"], np.float32)
    wdst = np.concatenate([-wf[:, 0:D1, :], wsv[:, 0:D1, :]], axis=2)
    wsrc = np.concatenate([-wf[:, D1:2 * D1, :], wsv[:, D1:2 * D1, :]], axis=2)
    wea = np.concatenate([-wf[:, 2 * D1:, :], wsv[:, 2 * D1:, :]], axis=2)
    bias = np.concatenate([-bf, bs], axis=1)[:, None, :]
    wea = np.concatenate([wea, bias], axis=1)
    shared = {
        "lin0w": np.asarray(inputs["lin0_w"], np.float32).astype(ml_dtypes.bfloat16),
        "lin0b": np.asarray(inputs["lin0_b"], np.float32).reshape(D1, 1),
        "wdst": np.transpose(wdst, (1, 0, 2)).reshape(D1, L * 128).astype(ml_dtypes.bfloat16),
        "wsrc": np.transpose(wsrc, (1, 0, 2)).reshape(D1, L * 128).astype(ml_dtypes.bfloat16),
        "wea": np.concatenate([
            np.zeros((1, L * 128), np.float32),
            np.transpose(wea, (1, 0, 2)).reshape(EF + 1, L * 128),
            np.zeros((64 - EF - 2, L * 128), np.float32),
        ], axis=0).astype(ml_dtypes.bfloat16),
        "bng": np.concatenate(
            [np.asarray(inputs["bn_gamma"], np.float32).T,
             np.full((D1, 1), 4.0 * EPS, np.float32)], axis=1),
        "bnb": np.asarray(inputs["bn_beta"], np.float32).T.copy(),
        "lin1w": np.asarray(inputs["lin1_w"], np.float32),
        "lin1b": np.asarray(inputs["lin1_b"], np.float32).reshape(D2, 1),
        "fcw": np.transpose(np.asarray(inputs["fc_w"], np.float32), (1, 0, 2)).reshape(D2, FC * D2),
        "fcb": np.asarray(inputs["fc_b"], np.float32).T.copy(),
        "lin2w": np.asarray(inputs["lin2_w"], np.float32).reshape(D2, 1),
        "lin2b": np.asarray(inputs["lin2_b"], np.float32).reshape(1, 1),
        "iota128": np.broadcast_to(np.arange(128, dtype=np.float32)[None, :],
                                   (128, 128)).astype(ml_dtypes.bfloat16),
        "iota256": np.broadcast_to(np.arange(G, dtype=np.float32)[None, :],
                                   (128, G)).astype(ml_dtypes.bfloat16),
        "ident": np.eye(128, dtype=np.float32),
        "identb": np.eye(128, dtype=np.float32).astype(ml_dtypes.bfloat16),
    }
    in_maps = [dict(shared, **pc) for pc in per_core]
    return in_maps, cw


def kernel(**inputs):
    from concourse.bass_utils import run_bass_kernel_spmd

    in_maps, cw = _preprocess(inputs)
    key = ("nc", cw)
    if key not in _CACHE:
        _CACHE[key] = _build_nc(cw)
    nc = _CACHE[key]
    res = run_bass_kernel_spmd(nc, in_maps, core_ids=list(range(NCORES)))
    return res.results[0]["y"].reshape(G).astype(np.float32)


# revision 64
# speedup vs baseline: 1.0159x; 1.0007x over previous
"""CGCNN message-passing kernel for 8 Trainium2 NeuronCores (Bass/Tile), v9.

Data-parallel by dst shard; gather-based edge pipeline:
- Host: nodes are dealt into 240 global windows (8 cores x 30 windows x 128
  slots) in descending-degree snake order, equalizing per-window edge counts
  so the uniform chunks-per-window pad cw is minimal (16). Edges go to the
  core owning their dst, grouped by dst window, chunk-padded to cw.
- lin0 is computed for ALL nodes redundantly on every core (from a replicated
  full xT) into a DRAM fp8 table, so layer 0 needs no h AllGather; layers
  1..L-1 AllGather h in fp8 (staged at the previous layer's BN boundary so
  the collective launches as early as possible).
- Per layer, per core:
  * Qd table (own shard, SBUF bf16 [128, 30, 128]) = h_own @ Wdst.
  * Full Qs table = h_full @ Wsrc into DRAM [30720, 128] bf16 in
    partition-major row order (node g -> row (g%128)*240 + g//128); PSUM->SBUF
    staging copies round-robin over DVE/ACT to minimize build latency.
  * Per 1024-edge tile: one dma_gather pulls per-edge Qs rows (1024 x 256B
    descriptors; 1024 = SWDGE ring capacity); Qe = ea(fp8) @ Wea by matmul
    (edge attrs streamed fp8, 4 tiles per DMA); the dst contribution expands
    via a host-precomputed one-hot (fp8, SBUF-resident, layer-invariant)
    matmul against the SBUF Qd table. All three accumulate in PSUM per
    128-edge chunk.
  * Nonlinearity: joint exp u = [e^-a | e^b] (f-gate weights pre-negated),
    v = ln(1+u); 1/3 of tiles compute sigmoid(a) = e^-v_f on ACT, 2/3 as
    1/(1+u_f) on DVE (bf16, engine balance). m = 2*sigmoid(a)*softplus(b);
    the factor 2 is absorbed exactly by BatchNorm using 4*EPS.
  * Aggregation one-hots (is_equal(iota, dst) * 1/cnt, bf16 on DVE) are
    pre-built per tile, and the aggregation matmuls are deferred by one tile
    so the in-order PE queue never stalls on the ACT/DVE nonlinearity chain.
  * Segment-mean accumulates per dst window in PSUM (agg PSUM shares banks
    with the build-phase staging, freeing a third pre-PSUM buffer); BatchNorm
    batch stats via a tiny stats AllGather + local sum; the residual
    (scalar_tensor_tensor + relu) is computed in halves, with the fp8
    AllGather payload produced on ACT in parallel with the f32/bf16 h copies
    on DVE/Pool.
- Global mean pool via one-hot matmul, bf16 partials AllGathered and summed
  locally, head MLP computed redundantly on every core.
"""
import numpy as np
import ml_dtypes

N = 30000
E = 480000
NF = 92
EF = 50
D1 = 64
D2 = 64
L = 3
FC = 2
G = 256
EPS = 1e-5
NCORES = 8
SHARD = N // NCORES            # 3750
SHARD_P = 3840                 # padded shard (30 windows of 128)
NWIN = SHARD_P // 128          # 30
NWING = NCORES * NWIN          # 240 global windows
TBL = NCORES * SHARD_P         # 30720 table rows

_CACHE = {}



def _build_nc(cw):
    """Build the SPMD bass module. cw = chunks per dst window (uniform)."""
    import concourse.mybir as mybir
    from concourse import bacc
    from concourse.tile import TileContext

    f32 = mybir.dt.float32
    bf16 = mybir.dt.bfloat16
    f8 = mybir.dt.float8e4
    i16 = mybir.dt.int16
    AF = mybir.ActivationFunctionType
    OP = mybir.AluOpType

    nchunk = NWIN * cw                 # chunks per layer per core
    etot = nchunk * 128                # padded edges per core
    ntile = (nchunk + 7) // 8          # 8-chunk (1024-edge) PSUM tiles

    import concourse.hw_specs as _hw
    import concourse.bacc as _bacc_mod
    _real_tables = _hw.get_activation_tables("gen3")
    _combined = "natural_log_exp_and_others"
    if _combined in _real_tables:
        _patched = {
            k: (v if k == _combined else (v - {AF.Exp, AF.Ln}))
            for k, v in _real_tables.items()
        }
        _bacc_mod.get_activation_tables = lambda arch: _patched

    nc = bacc.Bacc(None, target_bir_lowering=False)

    # ---- inputs (per core) ----
    xTF = nc.dram_tensor("xTF", [NF, TBL], bf16, kind="ExternalInput")
    xT = nc.dram_tensor("xT", [NF, SHARD_P], bf16, kind="ExternalInput")
    eaT = nc.dram_tensor("eaT", [64, etot], f8, kind="ExternalInput")
    qs_idxD = nc.dram_tensor("qs_idxD", [128, etot // 16], i16, kind="ExternalInput")
    ohTD = nc.dram_tensor("ohTD", [128, etot], f8, kind="ExternalInput")
    dstloc_p = nc.dram_tensor("dstloc_p", [128, nchunk], f32, kind="ExternalInput")
    rc_p = nc.dram_tensor("rc_p", [128, nchunk], f32, kind="ExternalInput")
    batchloc = nc.dram_tensor("batchloc", [128, NWIN], f32, kind="ExternalInput")
    rgc_pn = nc.dram_tensor("rgc_pn", [128, NWIN], f32, kind="ExternalInput")
    # weights (replicated; f-gate halves pre-negated)
    lin0w = nc.dram_tensor("lin0w", [NF, D1], bf16, kind="ExternalInput")
    lin0b = nc.dram_tensor("lin0b", [D1, 1], f32, kind="ExternalInput")
    wdst = nc.dram_tensor("wdst", [D1, L * 128], bf16, kind="ExternalInput")
    wsrc = nc.dram_tensor("wsrc", [D1, L * 128], bf16, kind="ExternalInput")
    wea = nc.dram_tensor("wea", [64, L * 128], bf16, kind="ExternalInput")
    bng = nc.dram_tensor("bng", [D1, L + 1], f32, kind="ExternalInput")
    bnb = nc.dram_tensor("bnb", [D1, L], f32, kind="ExternalInput")
    lin1w = nc.dram_tensor("lin1w", [D1, D2], f32, kind="ExternalInput")
    lin1b = nc.dram_tensor("lin1b", [D2, 1], f32, kind="ExternalInput")
    fcw = nc.dram_tensor("fcw", [D2, FC * D2], f32, kind="ExternalInput")
    fcb = nc.dram_tensor("fcb", [D2, FC], f32, kind="ExternalInput")
    lin2w = nc.dram_tensor("lin2w", [D2, 1], f32, kind="ExternalInput")
    lin2b = nc.dram_tensor("lin2b", [1, 1], f32, kind="ExternalInput")
    iota128 = nc.dram_tensor("iota128", [128, 128], bf16, kind="ExternalInput")
    iota256 = nc.dram_tensor("iota256", [128, G], bf16, kind="ExternalInput")
    ident = nc.dram_tensor("ident", [128, 128], f32, kind="ExternalInput")
    identb = nc.dram_tensor("identb", [128, 128], bf16, kind="ExternalInput")

    yout = nc.dram_tensor("y", [1, G], f32, kind="ExternalOutput")

    # ---- DRAM scratch ----
    QsD = nc.dram_tensor("QsD", [TBL, 128], bf16)          # row p*NWING+W
    h1f8 = nc.dram_tensor("h1f8", [NCORES * D1, SHARD_P], f8)
    ag_in = nc.dram_tensor("ag_in", [D1, SHARD_P], f8)
    ag_out = nc.dram_tensor("ag_out", [NCORES * D1, SHARD_P], f8,
                            addr_space="Shared")
    ar_in = nc.dram_tensor("ar_in", [D1, 2], f32)
    ar_out = nc.dram_tensor("ar_out", [NCORES * D1, 2], f32, addr_space="Shared")
    pl_in = nc.dram_tensor("pl_in", [D1, G], bf16)
    pl_out = nc.dram_tensor("pl_out", [NCORES * D1, G], bf16, addr_space="Shared")

    rg = [list(range(NCORES))]
    QsD3 = QsD[:, :].rearrange("(p w) f -> p w f", p=128)   # [128, NWING, 128]

    from contextlib import ExitStack
    with TileContext(nc) as tc:
        with ExitStack() as _es:
            cp = _es.enter_context(tc.tile_pool(name="const", bufs=1))
            bigp = _es.enter_context(tc.tile_pool(name="big", bufs=1))
            wp = _es.enter_context(tc.tile_pool(name="work", bufs=3))
            tlp = _es.enter_context(tc.tile_pool(name="tail", bufs=1))
            gp = _es.enter_context(tc.tile_pool(name="gat", bufs=4))
            ep = _es.enter_context(tc.tile_pool(name="ea", bufs=3))
            nlp = _es.enter_context(tc.tile_pool(name="nl", bufs=3))
            ohp = _es.enter_context(tc.tile_pool(name="oh", bufs=13))
            ohgp = _es.enter_context(tc.tile_pool(name="ohg", bufs=5))
            stp = _es.enter_context(tc.tile_pool(name="st", bufs=2))
            sgp = _es.enter_context(tc.tile_pool(name="sgp", bufs=3))
            scp = _es.enter_context(tc.tile_pool(name="scr", bufs=1))
            pp = _es.enter_context(tc.tile_pool(name="pre", bufs=3, space="PSUM"))
            ppB = _es.enter_context(tc.tile_pool(name="psB", bufs=2, space="PSUM"))
            # ---------- constants ----------
            def load_const(t, dram, shape, dtype=f32):
                tt = cp.tile(shape, dtype, tag=t)
                nc.sync.dma_start(out=tt[:], in_=dram)
                return tt

            l0w = load_const("l0w", lin0w[:, :], [NF, D1], bf16)
            l0b = load_const("l0b", lin0b[:, :], [D1, 1])

            # ---------- resident state ----------
            hT_own = bigp.tile([D1, SHARD_P], f32, tag="hown")
            hb_own = bigp.tile([D1, SHARD_P], bf16, tag="hbown")
            aggr_sb = bigp.tile([D1, SHARD_P], bf16, tag="aggr")
            qd_sb = bigp.tile([128, NWIN, 128], bf16, tag="qdsb")
            asb = scp.tile([D1, SHARD_P], f32, tag="asb")

            # ---------- lin0 for ALL nodes (no AllGather for layer 0) ----
            # h1f8 holds relu(x @ lin0_w + b) for all 8 shards (global order),
            # computed redundantly on every core from the replicated xTF.
            HL = SHARD_P // 2
            for s_ in range(NCORES):
                h8s = stp.tile([D1, SHARD_P], f8, tag="h8")
                for hh in range(2):
                    xt = sgp.tile([NF, HL], bf16, tag="qsst")
                    o = s_ * SHARD_P + hh * HL
                    nc.sync.dma_start(out=xt[:], in_=xTF[:, o:o + HL])
                    for j in range(4):
                        sl = slice(j * 480, (j + 1) * 480)
                        ph = ppB.tile([D1, 512], f32, tag="bld")
                        nc.tensor.matmul(out=ph[:, :480], lhsT=l0w[:],
                                         rhs=xt[:, sl], start=True, stop=True)
                        osl = slice(hh * HL + j * 480, hh * HL + (j + 1) * 480)
                        if j % 2 == 0:
                            nc.scalar.activation(
                                out=h8s[:, osl],
                                in_=ph[:, :480], func=AF.Relu, bias=l0b[:],
                                scale=1.0)
                        else:
                            nc.vector.tensor_scalar(
                                out=h8s[:, osl], in0=ph[:, :480],
                                scalar1=l0b[:], scalar2=0.0,
                                op0=OP.add, op1=OP.max)
                nc.sync.dma_start(out=h1f8[s_ * D1:(s_ + 1) * D1, :], in_=h8s[:])

            # own-shard h in f32 from the per-core xT input
            for hh in range(2):
                xt0 = sgp.tile([NF, HL], bf16, tag="qsst")
                nc.sync.dma_start(out=xt0[:], in_=xT[:, hh * HL:(hh + 1) * HL])
                for j in range(4):
                    sl = slice(hh * HL + j * 480, hh * HL + (j + 1) * 480)
                    ph = ppB.tile([D1, 512], f32, tag="bld")
                    nc.tensor.matmul(out=ph[:, :480], lhsT=l0w[:],
                                     rhs=xt0[:, j * 480:(j + 1) * 480],
                                     start=True, stop=True)
                    nc.scalar.activation(out=hT_own[:, sl], in_=ph[:, :480],
                                         func=AF.Relu, bias=l0b[:], scale=1.0)
                    nc.vector.tensor_scalar(
                        out=hb_own[:, sl], in0=ph[:, :480],
                        scalar1=l0b[:], scalar2=0.0, op0=OP.add, op1=OP.max)

            # remaining constants: emitted after lin0 so their DMA (notably
            # the 60KB/partition one-hot + gather indices) doesn't serialize
            # ahead of the xTF streams in the SP/DMA queues
            ws = load_const("ws", wsrc[:, :], [D1, L * 128], bf16)
            wd = load_const("wd", wdst[:, :], [D1, L * 128], bf16)
            we = load_const("we", wea[:, :], [64, L * 128], bf16)
            io128 = load_const("io128", iota128[:, :], [128, 128], bf16)
            idnb = load_const("idnb", identb[:, :], [128, 128], bf16)
            dlp = load_const("dlp", dstloc_p[:, :], [128, nchunk])
            rcp = load_const("rcp", rc_p[:, :], [128, nchunk])
            gmt = load_const("gmt", bng[:, :], [D1, L + 1])
            bbt = load_const("bbt", bnb[:, :], [D1, L])
            io256 = load_const("io256", iota256[:, :], [128, G], bf16)
            idn = load_const("idn", ident[:, :], [128, 128])
            blc = load_const("blc", batchloc[:, :], [128, NWIN])
            rgp = load_const("rgp", rgc_pn[:, :], [128, NWIN])
            l1w = load_const("l1w", lin1w[:, :], [D1, D2])
            l1b = load_const("l1b", lin1b[:, :], [D2, 1])
            fw = load_const("fw", fcw[:, :], [D2, FC * D2])
            fb = load_const("fb", fcb[:, :], [D2, FC])
            l2w = load_const("l2w", lin2w[:, :], [D2, 1])
            l2b = load_const("l2b", lin2b[:, :], [1, 1])
            qsix = load_const("qsix", qs_idxD[:, :], [128, etot // 16], i16)
            ohT_res = cp.tile([128, nchunk, 128], f8, tag="ohres")
            nc.sync.dma_start(
                out=ohT_res[:].rearrange("p a b -> p (a b)"), in_=ohTD[:, :])

            # ---------- layers ----------
            for l in range(L):
                wd_l = wd[:, l * 128:(l + 1) * 128]
                ws_l = ws[:, l * 128:(l + 1) * 128]
                we_l = we[:, l * 128:(l + 1) * 128]

                if l == 0:
                    src_dram = h1f8
                else:
                    # --- AllGather h (fp8, staged into ag_in at layer end) ---
                    nc.gpsimd.collective_compute(
                        "AllGather", OP.bypass, replica_groups=rg,
                        ins=[ag_in.ap().opt()], outs=[ag_out.ap().opt()])
                    src_dram = ag_out

                # --- Qd table build (own shard) ---
                for w0 in range(0, NWIN, 4):
                    kk = min(4, NWIN - w0)
                    qp = ppB.tile([128, 512], f32, tag="bld")
                    for k in range(kk):
                        w = w0 + k
                        nc.tensor.matmul(
                            out=qp[:, k * 128:(k + 1) * 128],
                            lhsT=hb_own[:, w * 128:(w + 1) * 128],
                            rhs=wd_l, start=True, stop=True)
                    nc.vector.tensor_copy(
                        out=qd_sb[:, w0:w0 + kk, :].rearrange("p a b -> p (a b)"),
                        in_=qp[:, :kk * 128])

                # --- Qs table build (all nodes, per gathered shard) -> QsD ---
                ws8 = stp.tile([D1, 128], f8, tag="ws8")
                nc.scalar.activation(out=ws8[:], in_=ws_l,
                                     func=AF.Identity, scale=1.0)
                ncopy = 0
                for s_ in range(NCORES):
                    hb_sh = stp.tile([D1, SHARD_P], f8, tag="h8")
                    nc.sync.dma_start(out=hb_sh[:],
                                      in_=src_dram[s_ * D1:(s_ + 1) * D1, :])
                    for wB in range(0, NWIN, 16):
                        kB = min(16, NWIN - wB)
                        sg_t = sgp.tile([128, 16, 128], bf16, tag="qsst")
                        for w0 in range(wB, wB + kB, 4):
                            kk = min(4, wB + kB - w0)
                            qp = ppB.tile([128, 512], f32, tag="bld")
                            for k in range(kk):
                                w = w0 + k
                                nc.tensor.matmul(
                                    out=qp[:, k * 128:(k + 1) * 128],
                                    lhsT=hb_sh[:, w * 128:(w + 1) * 128],
                                    rhs=ws8[:], start=True, stop=True)
                            dst_ap = sg_t[:, w0 - wB:w0 - wB + kk, :] \
                                .rearrange("p a b -> p (a b)")
                            eng = ncopy % 2
                            ncopy += 1
                            if eng in (0,):
                                nc.vector.tensor_copy(
                                    out=dst_ap, in_=qp[:, :kk * 128])
                            else:
                                nc.scalar.activation(
                                    out=dst_ap, in_=qp[:, :kk * 128],
                                    func=AF.Identity, scale=1.0)
                        W0 = s_ * NWIN + wB
                        nc.sync.dma_start(out=QsD3[:, W0:W0 + kB, :],
                                          in_=sg_t[:, :kB, :])

                # --- edge pipeline ---
                st1g = wp.tile([D1, 8], f32, tag="st1g")
                st2g = wp.tile([D1, 8], f32, tag="st2g")
                agg = None
                qs_g = None
                aggst = {"agg": None}

                def emit_agg(m, ohs_t, t, te):
                    # aggregation for tile t, deferred one tile so the PE
                    # queue never stalls waiting for m
                    for c in range(te):
                        gc = t * 8 + c
                        w = gc // cw
                        if gc % (4 * cw) == 0:
                            agg_new = ppB.tile([D1, 512], f32, tag="bld")
                            aggst["agg"] = agg_new
                        agg = aggst["agg"]
                        nc.tensor.matmul(
                            out=agg[:, (w % 4) * 128:(w % 4 + 1) * 128],
                            lhsT=m[:, c, :], rhs=ohs_t[c][:],
                            start=(gc % cw == 0), stop=(gc % cw == cw - 1))
                        if gc % (4 * cw) == 4 * cw - 1 or gc == nchunk - 1:
                            grp = w // 4
                            lo = grp * 512
                            hi = min(lo + 512, SHARD_P)
                            nc.scalar.activation(
                                out=aggr_sb[:, lo:hi], in_=agg[:, :hi - lo],
                                func=AF.Identity, scale=1.0)
                            nc.vector.reduce_sum(
                                out=st1g[:, grp:grp + 1],
                                in_=aggr_sb[:, lo:hi],
                                axis=mybir.AxisListType.X)
                            sqg = nlp.tile([D1, 512], bf16, tag="sqg")
                            nc.vector.tensor_tensor(
                                out=sqg[:, :hi - lo], in0=aggr_sb[:, lo:hi],
                                in1=aggr_sb[:, lo:hi], op=OP.mult)
                            nc.vector.reduce_sum(
                                out=st2g[:, grp:grp + 1],
                                in_=sqg[:, :hi - lo],
                                axis=mybir.AxisListType.X)

                pend = []
                for t in range(ntile):
                    te = min(8, nchunk - t * 8)          # chunks this tile
                    ne = te * 128                        # edges this tile
                    if t % 4 == 0:
                        tc32 = min(32, nchunk - t * 8)
                        et = ep.tile([64, 4096], f8, tag="et")
                        nc.sync.dma_start(
                            out=et[:, :tc32 * 128],
                            in_=eaT[:, t * 1024: t * 1024 + tc32 * 128])
                    qs_g = gp.tile([128, 8, 128], bf16, tag="qsg")
                    nc.gpsimd.dma_gather(
                        qs_g[:, :te, :], QsD[:, :],
                        qsix[:, t * 64: t * 64 + te * 8],
                        te * 128, te * 128, 128)
                    half = 0
                    qhalf = (t % 4) * 8

                    # one-hot aggregation matrices: const-only deps, built
                    # ahead so the agg matmuls never wait on DVE
                    ohs_t = []
                    for c in range(te):
                        gc = t * 8 + c
                        oh_ = ohp.tile([128, 128], bf16, tag="ohS")
                        nc.vector.tensor_scalar(
                            out=oh_[:], in0=io128[:],
                            scalar1=dlp[:, gc:gc + 1], scalar2=rcp[:, gc:gc + 1],
                            op0=OP.is_equal, op1=OP.mult)
                        ohs_t.append(oh_)

                    pre = pp.tile([128, 1024], f32, tag="pre")
                    qs_f = qs_g[:].rearrange("p a b -> p (a b)")
                    for c in range(te):
                        gc = t * 8 + c
                        w = gc // cw
                        csl = slice(c * 128, (c + 1) * 128)
                        csl2 = slice((half + c) * 128, (half + c + 1) * 128)
                        csl4 = slice((qhalf + c) * 128, (qhalf + c + 1) * 128)
                        nc.tensor.matmul(out=pre[:, csl], lhsT=et[:, csl4],
                                         rhs=we_l, start=True, stop=False)
                        nc.tensor.matmul(out=pre[:, csl], lhsT=idnb[:],
                                         rhs=qs_f[:, csl2], start=False, stop=False)
                        nc.tensor.matmul(out=pre[:, csl], lhsT=ohT_res[:, gc, :],
                                         rhs=qd_sb[:, w, :], start=False, stop=True)

                    # nonlinearity: m = (1+tanh(a/2)) * softplus(b)
                    #             = 2*sigmoid(a)*softplus(b)  (2 absorbed by BN)
                    # nonlinearity: u = [e^-a | e^b], v = ln(1+u) = [sp(-a)|sp(b)]
                    # even tiles (ACT): sigma = e^-sp(-a); odd tiles (DVE):
                    # sigma = 1/(1+e^-a). m = 2*sigma*sp(b) (2 absorbed by BN
                    # via 4*EPS).
                    uf = nlp.tile([128, 8, 128], bf16, tag="uf")
                    nc.scalar.activation(
                        out=uf[:, :te, :].rearrange("p a b -> p (a b)"),
                        in_=pre[:, :ne], func=AF.Exp, scale=1.0)
                    m = nlp.tile([128, 8, 64], bf16, tag="m")
                    if t % 3 == 0:
                        vf = nlp.tile([128, 8, 128], bf16, tag="vf")
                        nc.scalar.activation(
                            out=vf[:, :te, :].rearrange("p a b -> p (a b)"),
                            in_=uf[:, :te, :].rearrange("p a b -> p (a b)"),
                            func=AF.Ln, bias=1.0, scale=1.0)
                        sg = nlp.tile([128, 8, 64], bf16, tag="sg")
                        nc.scalar.activation(out=sg[:, :te, :],
                                             in_=vf[:, :te, 0:64],
                                             func=AF.Exp, scale=-1.0)
                        nc.vector.scalar_tensor_tensor(
                            out=m[:, :te, :], in0=sg[:, :te, :], scalar=2.0,
                            in1=vf[:, :te, 64:128], op0=OP.mult, op1=OP.mult)
                    else:
                        vs = nlp.tile([128, 8, 64], bf16, tag="vs")
                        nc.scalar.activation(out=vs[:, :te, :],
                                             in_=uf[:, :te, 64:128],
                                             func=AF.Ln, bias=1.0, scale=1.0)
                        w1 = nlp.tile([128, 8, 64], bf16, tag="sg")
                        with nc.allow_low_precision(reason="sigmoid in bf16"):
                            nc.vector.tensor_scalar(out=w1[:, :te, :],
                                                    in0=uf[:, :te, 0:64],
                                                    scalar1=1.0, scalar2=None,
                                                    op0=OP.add)
                            nc.vector.reciprocal(out=w1[:, :te, :],
                                                 in_=w1[:, :te, :])
                        nc.vector.scalar_tensor_tensor(
                            out=m[:, :te, :], in0=w1[:, :te, :], scalar=2.0,
                            in1=vs[:, :te, :], op0=OP.mult, op1=OP.mult)

                    pend.append((m, ohs_t, t, te))
                    if len(pend) > 1:
                        emit_agg(*pend.pop(0))
                while pend:
                    emit_agg(*pend.pop(0))

                # --- BN stats + AllReduce ---
                st = wp.tile([D1, 2], f32, tag="st")
                nc.vector.reduce_sum(out=st[:, 0:1], in_=st1g[:],
                                     axis=mybir.AxisListType.X)
                nc.vector.reduce_sum(out=st[:, 1:2], in_=st2g[:],
                                     axis=mybir.AxisListType.X)
                nc.sync.dma_start(out=ar_in[:, :], in_=st[:])
                nc.gpsimd.collective_compute(
                    "AllGather", OP.bypass, replica_groups=rg,
                    ins=[ar_in.ap().opt()], outs=[ar_out.ap().opt()])
                stga = wp.tile([D1, 2, NCORES], f32, tag="stga")
                nc.sync.dma_start(
                    out=stga[:],
                    in_=ar_out[:, :].rearrange("(c p) s -> p s c", p=D1))
                stg = wp.tile([D1, 2], f32, tag="stg")
                nc.vector.reduce_sum(
                    out=stg[:].rearrange("p (s o) -> p s o", o=1),
                    in_=stga[:], axis=mybir.AxisListType.X)
                mu = wp.tile([D1, 1], f32, tag="mu")
                nc.vector.tensor_scalar(out=mu[:], in0=stg[:, 0:1],
                                        scalar1=1.0 / N, scalar2=None, op0=OP.mult)
                ex2 = wp.tile([D1, 1], f32, tag="ex2")
                nc.vector.tensor_scalar(out=ex2[:], in0=stg[:, 1:2],
                                        scalar1=1.0 / N, scalar2=None, op0=OP.mult)
                nvar = wp.tile([D1, 1], f32, tag="var")
                nc.vector.scalar_tensor_tensor(
                    out=nvar[:], in0=mu[:], scalar=mu[:], in1=ex2[:],
                    op0=OP.mult, op1=OP.subtract)
                lv = wp.tile([D1, 1], f32, tag="lv")
                # m carries a factor 2 -> aggr/mu scale by 2, var by 4; 4*EPS
                # rides as an extra bng column; nvar = mu^2-ex2 = -var, so the
                # Ln input is nvar*(-1) + 4*EPS.
                nc.scalar.activation(out=lv[:], in_=nvar[:], func=AF.Ln,
                                     bias=gmt[:, L:L + 1], scale=-1.0)
                isd = wp.tile([D1, 1], f32, tag="isd")
                nc.scalar.activation(out=isd[:], in_=lv[:], func=AF.Exp, scale=-0.5)
                scale = wp.tile([D1, 1], f32, tag="scale")
                nc.vector.tensor_tensor(out=scale[:], in0=isd[:],
                                        in1=gmt[:, l:l + 1], op=OP.mult)
                mshift = wp.tile([D1, 1], f32, tag="mshift")
                nc.vector.tensor_tensor(out=mshift[:], in0=mu[:], in1=scale[:],
                                        op=OP.mult)
                shift = wp.tile([D1, 1], f32, tag="shift")
                nc.vector.tensor_tensor(out=shift[:], in0=bbt[:, l:l + 1],
                                        in1=mshift[:], op=OP.subtract)
                # h = relu((aggr*scale + h) + shift); the three consumers
                # (f32 residual, f8 AllGather payload, bf16 matmul copy) are
                # produced from asb concurrently on DVE/ACT/Pool
                HB = SHARD_P // 2
                for hh in range(2):
                    hsl = slice(hh * HB, (hh + 1) * HB)
                    nc.vector.scalar_tensor_tensor(
                        out=asb[:, hsl], in0=aggr_sb[:, hsl], scalar=scale[:],
                        in1=hT_own[:, hsl], op0=OP.mult, op1=OP.add)
                if l + 1 < L:
                    h8n = stp.tile([D1, SHARD_P], f8, tag="h8")
                    for hh in range(2):
                        hsl = slice(hh * HB, (hh + 1) * HB)
                        nc.scalar.activation(out=h8n[:, hsl], in_=asb[:, hsl],
                                             func=AF.Relu, bias=shift[:],
                                             scale=1.0)
                        nc.sync.dma_start(out=ag_in[:, hsl], in_=h8n[:, hsl])
                if l + 1 < L:
                    nc.vector.tensor_scalar(out=hT_own[:], in0=asb[:],
                                            scalar1=shift[:], scalar2=0.0,
                                            op0=OP.add, op1=OP.max)
                    nc.gpsimd.tensor_copy(out=hb_own[:], in_=hT_own[:])
                else:
                    # last layer: no AllGather payload to produce on ACT and
                    # no next Qd build needing hb_own; do the relu on ACT in
                    # halves (pipelined behind the stt halves on DVE) so the
                    # pool phase starts sooner
                    for hh in range(2):
                        hsl = slice(hh * HB, (hh + 1) * HB)
                        nc.scalar.activation(out=hT_own[:, hsl],
                                             in_=asb[:, hsl], func=AF.Relu,
                                             bias=shift[:], scale=1.0)

            # ---------- global mean pool ----------
            pool_ps = pp.tile([D1, G], f32, tag="pre")
            for w in range(NWIN):
                tp = ppB.tile([128, D1], f32, tag="bld")
                nc.tensor.transpose(out=tp[:], in_=hT_own[:, w * 128:(w + 1) * 128],
                                    identity=idn[0:D1, 0:D1])
                rows = wp.tile([128, D1], bf16, tag="rows")
                nc.vector.tensor_copy(out=rows[:], in_=tp[:])
                ohg = ohgp.tile([128, G], bf16, tag="ohg")
                nc.vector.tensor_scalar(
                    out=ohg[:], in0=io256[:],
                    scalar1=blc[:, w:w + 1], scalar2=rgp[:, w:w + 1],
                    op0=OP.is_equal, op1=OP.mult)
                nc.tensor.matmul(out=pool_ps[:], lhsT=rows[:], rhs=ohg[:],
                                 start=(w == 0), stop=(w == NWIN - 1))
            poolT = tlp.tile([D1, G], bf16, tag="poolT")
            nc.vector.tensor_copy(out=poolT[:], in_=pool_ps[:])
            nc.sync.dma_start(out=pl_in[:, :], in_=poolT[:])
            nc.gpsimd.collective_compute(
                "AllGather", OP.bypass, replica_groups=rg,
                ins=[pl_in.ap().opt()], outs=[pl_out.ap().opt()])
            pga = tlp.tile([D1, NCORES, G], bf16, tag="pga")
            nc.sync.dma_start(
                out=pga[:],
                in_=pl_out[:, :].rearrange("(c p) g -> p c g", p=D1))
            pg = tlp.tile([D1, G], f32, tag="pg")
            nc.vector.reduce_sum(
                out=pg[:].rearrange("p (g o) -> p g o", o=1),
                in_=pga[:].rearrange("p c g -> p g c"),
                axis=mybir.AxisListType.X)

            # ---------- head ----------
            a = pg
            hw_ = [(l1w[:], l1b[:]), (fw[:, 0:D2], fb[:, 0:1]), (fw[:, D2:2 * D2], fb[:, 1:2])]
            for (wt, bt) in hw_:
                ps = ppB.tile([D2, G], f32, tag="bld")
                nc.tensor.matmul(out=ps[:, 0:G], lhsT=wt, rhs=a[:], start=True, stop=True)
                an = tlp.tile([D2, G], f32, tag="an")
                nc.scalar.activation(out=an[:], in_=ps[:, 0:G], func=AF.Relu,
                                     bias=bt, scale=1.0)
                a = an
            ps = ppB.tile([1, G], f32, tag="bld")
            nc.tensor.matmul(out=ps[:, 0:G], lhsT=l2w[:], rhs=a[:], start=True, stop=True)
            yt = tlp.tile([1, G], f32, tag="yt")
            nc.scalar.activation(out=yt[:], in_=ps[:, 0:G], func=AF.Identity,
                                 bias=l2b[:], scale=1.0)
            nc.sync.dma_start(out=yout[:, :], in_=yt[:])

    nc.compile()
    return nc


def _wrap16(idx):
    """Flat idx list -> [128, n/16] int16: slot i at [i%16, i//16], replicated
    across the 8 Q7 cores."""
    a = idx.reshape(-1, 16).T.astype(np.int16)
    return np.tile(a, (8, 1))


def _preprocess(inputs):
    x = np.asarray(inputs["x"], np.float32)
    ea = np.asarray(inputs["edge_attr"], np.float32)
    ei = np.asarray(inputs["edge_index"]).astype(np.int64)
    batch = np.asarray(inputs["batch"]).astype(np.int64)
    src, dst = ei[0], ei[1]

    cnt = np.bincount(dst, minlength=N).astype(np.float32)
    rc_node = 1.0 / np.maximum(cnt, 1.0)
    gcnt = np.bincount(batch, minlength=G).astype(np.float32)
    rgc = 1.0 / np.maximum(gcnt, 1.0)

    # Degree-balanced node -> (window, slot) assignment: snake-deal nodes in
    # descending-degree order across the 240 global windows, minimizing the
    # max per-window edge count (which sets the uniform chunk pad cw).
    deg_order = np.argsort(-cnt, kind="stable")       # node ids, deg desc
    nwin_g = NCORES * NWIN                            # 240
    perm_loc = np.empty(N, np.int64)                  # node -> global padded id
    for i0 in range(0, N, nwin_g):
        blk = deg_order[i0:i0 + nwin_g]
        j = i0 // nwin_g
        wins = np.arange(len(blk)) if j % 2 == 0 else (len(blk) - 1 - np.arange(len(blk)))
        w_ids = wins
        perm_loc[blk] = (w_ids // NWIN) * SHARD_P + (w_ids % NWIN) * 128 + j
    gperm = perm_loc
    srcg = gperm[src]
    dstg = gperm[dst]
    order = np.argsort(dstg, kind="stable")
    srcg_s, dstg_s, ea_idx = srcg[order], dstg[order], order

    bounds = []
    for c in range(NCORES):
        for w in range(NWIN):
            bounds.append(c * SHARD_P + w * 128)
    bounds.append(NCORES * SHARD_P)
    bpos = np.searchsorted(dstg_s, np.asarray(bounds), side="left")
    percw = {}
    maxcnt = 0
    k = 0
    for c in range(NCORES):
        for w in range(NWIN):
            lo, hi = bpos[k], bpos[k + 1]
            percw[(c, w)] = np.arange(lo, hi)
            maxcnt = max(maxcnt, hi - lo)
            k += 1
    cw = max(1, (maxcnt + 127) // 128)
    etot = NWIN * cw * 128

    # full padded x, rotated per core so block 0 is the own shard
    xfull = np.zeros((NF, NCORES * SHARD_P), np.float32)
    xfull[:, gperm] = x.T
    xfull = xfull.astype(ml_dtypes.bfloat16)

    per_core = []
    for c in range(NCORES):
        qs_idx = np.zeros(etot, np.int64)
        dl = np.full(etot, -1.0, np.float32)
        rc_e = np.ones(etot, np.float32)
        ea_e = np.zeros((etot, EF), np.float32)
        for w in range(NWIN):
            idxs = percw[(c, w)]
            o = w * cw * 128
            k = len(idxs)
            g = srcg_s[idxs]                           # padded global id
            qs_idx[o:o + k] = (g % 128) * NWING + (g // 128)
            loc = dstg_s[idxs] - c * SHARD_P           # 0..3839
            dl[o:o + k] = (loc - w * 128).astype(np.float32)
            rc_e[o:o + k] = rc_node[dst[ea_idx[idxs]]]
            ea_e[o:o + k] = ea[ea_idx[idxs]]
        eaT = np.zeros((64, etot), np.float32)
        eaT[1:EF + 1] = ea_e.T
        eaT[EF + 1] = 1.0
        eaT[EF + 1, dl < 0] = 0.0
        nch = etot // 128
        ohT = np.zeros((128, etot), np.float32)
        vv = dl >= 0
        ohT[dl[vv].astype(np.int64), np.nonzero(vv)[0]] = 1.0
        d = {
            "qs_idxD": _wrap16(qs_idx),
            "ohTD": ohT.astype(ml_dtypes.float8_e4m3),
            "dstloc_p": dl.reshape(nch, 128).T.copy(),
            "rc_p": rc_e.reshape(nch, 128).T.copy(),
            "eaT": eaT.astype(ml_dtypes.float8_e4m3),
        }
        d["xTF"] = xfull
        d["xT"] = xfull[:, c * SHARD_P:(c + 1) * SHARD_P].copy()
        nodes_c = np.nonzero((gperm // SHARD_P) == c)[0]
        locs_c = gperm[nodes_c] - c * SHARD_P
        bl = np.full(SHARD_P, -1.0, np.float32)
        bl[locs_c] = batch[nodes_c].astype(np.float32)
        rg_n = np.zeros(SHARD_P, np.float32)
        rg_n[locs_c] = rgc[batch[nodes_c]]
        d["batchloc"] = bl.reshape(NWIN, 128).T.copy()
        d["rgc_pn"] = rg_n.reshape(NWIN, 128).T.copy()
        per_core.append(d)

    # replicated weights; f-gate halves pre-negated
    wf = np.asarray(inputs["conv_wf"], np.float32)
    wsv = np.asarray(inputs["conv_ws"], np.float32)
    bf = np.asarray(inputs["conv_bf"], np.float32)
    bs = np.asarray(inputs["conv_bs"], np.float32)
    wdst = np.concatenate([-wf[:, 0:D1, :], wsv[:, 0:D1, :]], axis=2)
    wsrc = np.concatenate([-wf[:, D1:2 * D1, :], wsv[:, D1:2 * D1, :]], axis=2)
    wea = np.concatenate([-wf[:, 2 * D1:, :], wsv[:, 2 * D1:, :]], axis=2)
    bias = np.concatenate([-bf, bs], axis=1)[:, None, :]
    wea = np.concatenate([wea, bias], axis=1)
    shared = {
        "lin0w": np.asarray(inputs["lin0_w"], np.float32).astype(ml_dtypes.bfloat16),
        "lin0b": np.asarray(inputs["lin0_b"], np.float32).reshape(D1, 1),
        "wdst": np.transpose(wdst, (1, 0, 2)).reshape(D1, L * 128).astype(ml_dtypes.bfloat16),
        "wsrc": np.transpose(wsrc, (1, 0, 2)).reshape(D1, L * 128).astype(ml_dtypes.bfloat16),
        "wea": np.concatenate([
            np.zeros((1, L * 128), np.float32),
            np.transpose(wea, (1, 0, 2)).reshape(EF + 1, L * 128),
            np.zeros((64 - EF - 2, L * 128), np.float32),
        ], axis=0).astype(ml_dtypes.bfloat16),
        "bng": np.concatenate(
            [np.asarray(inputs["bn_gamma"], np.float32).T,
             np.full((D1, 1), 4.0 * EPS, np.float32)], axis=1),
        "bnb": np.asarray(inputs["bn_beta"], np.float32).T.copy(),
        "lin1w": np.asarray(inputs["lin1_w"], np.float32),
        "lin1b": np.asarray(inputs["lin1_b"], np.float32).reshape(D2, 1),
        "fcw": np.transpose(np.asarray(inputs["fc_w"], np.float32), (1, 0, 2)).reshape(D2, FC * D2),
        "fcb": np.asarray(inputs["fc_b"], np.float32).T.copy(),
        "lin2w": np.asarray(inputs["lin2_w"], np.float32).reshape(D2, 1),
        "lin2b": np.asarray(inputs["lin2_b"], np.float32).reshape(1, 1),
        "iota128": np.broadcast_to(np.arange(128, dtype=np.float32)[None, :],
                                   (128, 128)).astype(ml_dtypes.bfloat16),
        "iota256": np.broadcast_to(np.arange(G, dtype=np.float32)[None, :],
                                   (128, G)).astype(ml_dtypes.bfloat16),
        "ident": np.eye(128, dtype=np.float32),
        "identb": np.eye(128, dtype=np.float32).astype(ml_dtypes.bfloat16),
    }
    in_maps = [dict(shared, **pc) for pc in per_core]
    return in_maps, cw


def kernel(**inputs):
    from concourse.bass_utils import run_bass_kernel_spmd

    in_maps, cw = _preprocess(inputs)
    key = ("nc", cw)
    if key not in _CACHE:
        _CACHE[key] = _build_nc(cw)
    nc = _CACHE[key]
    res = run_bass_kernel_spmd(nc, in_maps, core_ids=list(range(NCORES)))
    return res.results[0]["y"].reshape(G).astype(np.float32)


# revision 65
# speedup vs baseline: 1.0220x; 1.0060x over previous
"""CGCNN message-passing kernel for 8 Trainium2 NeuronCores (Bass/Tile), v9.

Data-parallel by dst shard; gather-based edge pipeline:
- Host: nodes are dealt into 240 global windows (8 cores x 30 windows x 128
  slots) in descending-degree snake order, equalizing per-window edge counts
  so the uniform chunks-per-window pad cw is minimal (16). Edges go to the
  core owning their dst, grouped by dst window, chunk-padded to cw.
- lin0 is computed for ALL nodes redundantly on every core (from a replicated
  full xT) into a DRAM fp8 table, so layer 0 needs no h AllGather; layers
  1..L-1 AllGather h in fp8 (staged at the previous layer's BN boundary so
  the collective launches as early as possible).
- Per layer, per core:
  * Qd table (own shard, SBUF bf16 [128, 30, 128]) = h_own @ Wdst.
  * Full Qs table = h_full @ Wsrc into DRAM [30720, 128] bf16 in
    partition-major row order (node g -> row (g%128)*240 + g//128); PSUM->SBUF
    staging copies round-robin over DVE/ACT to minimize build latency.
  * Per 1024-edge tile: one dma_gather pulls per-edge Qs rows (1024 x 256B
    descriptors; 1024 = SWDGE ring capacity); Qe = ea(fp8) @ Wea by matmul
    (edge attrs streamed fp8, 4 tiles per DMA); the dst contribution expands
    via a host-precomputed one-hot (fp8, SBUF-resident, layer-invariant)
    matmul against the SBUF Qd table. All three accumulate in PSUM per
    128-edge chunk.
  * Nonlinearity: joint exp u = [e^-a | e^b] (f-gate weights pre-negated),
    v = ln(1+u); 1/3 of tiles compute sigmoid(a) = e^-v_f on ACT, 2/3 as
    1/(1+u_f) on DVE (bf16, engine balance). m = 2*sigmoid(a)*softplus(b);
    the factor 2 is absorbed exactly by BatchNorm using 4*EPS.
  * Aggregation one-hots (is_equal(iota, dst) * 1/cnt, bf16 on DVE) are
    pre-built per tile, and the aggregation matmuls are deferred by one tile
    so the in-order PE queue never stalls on the ACT/DVE nonlinearity chain.
  * Segment-mean accumulates per dst window in PSUM (agg PSUM shares banks
    with the build-phase staging, freeing a third pre-PSUM buffer); BatchNorm
    batch stats via a tiny stats AllGather + local sum; the residual
    (scalar_tensor_tensor + relu) is computed in halves, with the fp8
    AllGather payload produced on ACT in parallel with the f32/bf16 h copies
    on DVE/Pool.
- Global mean pool via one-hot matmul, bf16 partials AllGathered and summed
  locally, head MLP computed redundantly on every core.
"""
import numpy as np
import ml_dtypes

N = 30000
E = 480000
NF = 92
EF = 50
D1 = 64
D2 = 64
L = 3
FC = 2
G = 256
EPS = 1e-5
NCORES = 8
SHARD = N // NCORES            # 3750
SHARD_P = 3840                 # padded shard (30 windows of 128)
NWIN = SHARD_P // 128          # 30
NWING = NCORES * NWIN          # 240 global windows
TBL = NCORES * SHARD_P         # 30720 table rows

_CACHE = {}



def _build_nc(cw):
    """Build the SPMD bass module. cw = chunks per dst window (uniform)."""
    import concourse.mybir as mybir
    from concourse import bacc
    from concourse.tile import TileContext

    f32 = mybir.dt.float32
    bf16 = mybir.dt.bfloat16
    f8 = mybir.dt.float8e4
    i16 = mybir.dt.int16
    AF = mybir.ActivationFunctionType
    OP = mybir.AluOpType

    nchunk = NWIN * cw                 # chunks per layer per core
    etot = nchunk * 128                # padded edges per core
    ntile = (nchunk + 7) // 8          # 8-chunk (1024-edge) PSUM tiles

    import concourse.hw_specs as _hw
    import concourse.bacc as _bacc_mod
    _real_tables = _hw.get_activation_tables("gen3")
    _combined = "natural_log_exp_and_others"
    if _combined in _real_tables:
        _patched = {
            k: (v if k == _combined else (v - {AF.Exp, AF.Ln}))
            for k, v in _real_tables.items()
        }
        _bacc_mod.get_activation_tables = lambda arch: _patched

    nc = bacc.Bacc(None, target_bir_lowering=False)

    # ---- inputs (per core) ----
    xTF = nc.dram_tensor("xTF", [NF, TBL], bf16, kind="ExternalInput")
    xT = nc.dram_tensor("xT", [NF, SHARD_P], bf16, kind="ExternalInput")
    eaT = nc.dram_tensor("eaT", [64, etot], f8, kind="ExternalInput")
    qs_idxD = nc.dram_tensor("qs_idxD", [128, etot // 16], i16, kind="ExternalInput")
    ohTD = nc.dram_tensor("ohTD", [128, etot], f8, kind="ExternalInput")
    dstloc_p = nc.dram_tensor("dstloc_p", [128, nchunk], f32, kind="ExternalInput")
    rc_p = nc.dram_tensor("rc_p", [128, nchunk], f32, kind="ExternalInput")
    batchloc = nc.dram_tensor("batchloc", [128, NWIN], f32, kind="ExternalInput")
    rgc_pn = nc.dram_tensor("rgc_pn", [128, NWIN], f32, kind="ExternalInput")
    # weights (replicated; f-gate halves pre-negated)
    lin0w = nc.dram_tensor("lin0w", [NF, D1], bf16, kind="ExternalInput")
    lin0b = nc.dram_tensor("lin0b", [D1, 1], f32, kind="ExternalInput")
    wdst = nc.dram_tensor("wdst", [D1, L * 128], bf16, kind="ExternalInput")
    wsrc = nc.dram_tensor("wsrc", [D1, L * 128], bf16, kind="ExternalInput")
    wea = nc.dram_tensor("wea", [64, L * 128], bf16, kind="ExternalInput")
    bng = nc.dram_tensor("bng", [D1, L + 1], f32, kind="ExternalInput")
    bnb = nc.dram_tensor("bnb", [D1, L], f32, kind="ExternalInput")
    lin1w = nc.dram_tensor("lin1w", [D1, D2], f32, kind="ExternalInput")
    lin1b = nc.dram_tensor("lin1b", [D2, 1], f32, kind="ExternalInput")
    fcw = nc.dram_tensor("fcw", [D2, FC * D2], f32, kind="ExternalInput")
    fcb = nc.dram_tensor("fcb", [D2, FC], f32, kind="ExternalInput")
    lin2w = nc.dram_tensor("lin2w", [D2, 1], f32, kind="ExternalInput")
    lin2b = nc.dram_tensor("lin2b", [1, 1], f32, kind="ExternalInput")
    iota128 = nc.dram_tensor("iota128", [128, 128], bf16, kind="ExternalInput")
    iota256 = nc.dram_tensor("iota256", [128, G], bf16, kind="ExternalInput")
    ident = nc.dram_tensor("ident", [128, 128], f32, kind="ExternalInput")
    identb = nc.dram_tensor("identb", [128, 128], bf16, kind="ExternalInput")

    yout = nc.dram_tensor("y", [1, G], f32, kind="ExternalOutput")

    # ---- DRAM scratch ----
    QsD = nc.dram_tensor("QsD", [TBL, 128], bf16)          # row p*NWING+W
    h1f8 = nc.dram_tensor("h1f8", [NCORES * D1, SHARD_P], f8)
    ag_in = nc.dram_tensor("ag_in", [D1, SHARD_P], f8)
    ag_out = nc.dram_tensor("ag_out", [NCORES * D1, SHARD_P], f8,
                            addr_space="Shared")
    ar_in = nc.dram_tensor("ar_in", [D1, 2], f32)
    ar_out = nc.dram_tensor("ar_out", [NCORES * D1, 2], f32, addr_space="Shared")
    pl_in = nc.dram_tensor("pl_in", [D1, G], bf16)
    pl_out = nc.dram_tensor("pl_out", [NCORES * D1, G], bf16, addr_space="Shared")

    rg = [list(range(NCORES))]
    QsD3 = QsD[:, :].rearrange("(p w) f -> p w f", p=128)   # [128, NWING, 128]

    from contextlib import ExitStack
    with TileContext(nc) as tc:
        with ExitStack() as _es:
            cp = _es.enter_context(tc.tile_pool(name="const", bufs=1))
            bigp = _es.enter_context(tc.tile_pool(name="big", bufs=1))
            wp = _es.enter_context(tc.tile_pool(name="work", bufs=3))
            tlp = _es.enter_context(tc.tile_pool(name="tail", bufs=1))
            gp = _es.enter_context(tc.tile_pool(name="gat", bufs=4))
            ep = _es.enter_context(tc.tile_pool(name="ea", bufs=3))
            nlp = _es.enter_context(tc.tile_pool(name="nl", bufs=3))
            ohp = _es.enter_context(tc.tile_pool(name="oh", bufs=16))
            ohgp = _es.enter_context(tc.tile_pool(name="ohg", bufs=3))
            stp = _es.enter_context(tc.tile_pool(name="st", bufs=2))
            sgp = _es.enter_context(tc.tile_pool(name="sgp", bufs=3))
            scp = _es.enter_context(tc.tile_pool(name="scr", bufs=1))
            pp = _es.enter_context(tc.tile_pool(name="pre", bufs=3, space="PSUM"))
            ppB = _es.enter_context(tc.tile_pool(name="psB", bufs=2, space="PSUM"))
            # ---------- constants ----------
            def load_const(t, dram, shape, dtype=f32):
                tt = cp.tile(shape, dtype, tag=t)
                nc.sync.dma_start(out=tt[:], in_=dram)
                return tt

            l0w = load_const("l0w", lin0w[:, :], [NF, D1], bf16)
            l0b = load_const("l0b", lin0b[:, :], [D1, 1])

            # ---------- resident state ----------
            hT_own = bigp.tile([D1, SHARD_P], f32, tag="hown")
            hb_own = bigp.tile([D1, SHARD_P], bf16, tag="hbown")
            aggr_sb = bigp.tile([D1, SHARD_P], bf16, tag="aggr")
            qd_sb = bigp.tile([128, NWIN, 128], bf16, tag="qdsb")
            asb = scp.tile([D1, SHARD_P], f32, tag="asb")

            # ---------- lin0 for ALL nodes (no AllGather for layer 0) ----
            # h1f8 holds relu(x @ lin0_w + b) for all 8 shards (global order),
            # computed redundantly on every core from the replicated xTF.
            HL = SHARD_P // 2
            for s_ in range(NCORES):
                h8s = stp.tile([D1, SHARD_P], f8, tag="h8")
                for hh in range(2):
                    xt = sgp.tile([NF, HL], bf16, tag="qsst")
                    o = s_ * SHARD_P + hh * HL
                    nc.sync.dma_start(out=xt[:], in_=xTF[:, o:o + HL])
                    for j in range(4):
                        sl = slice(j * 480, (j + 1) * 480)
                        ph = ppB.tile([D1, 512], f32, tag="bld")
                        nc.tensor.matmul(out=ph[:, :480], lhsT=l0w[:],
                                         rhs=xt[:, sl], start=True, stop=True)
                        osl = slice(hh * HL + j * 480, hh * HL + (j + 1) * 480)
                        if j % 2 == 0:
                            nc.scalar.activation(
                                out=h8s[:, osl],
                                in_=ph[:, :480], func=AF.Relu, bias=l0b[:],
                                scale=1.0)
                        else:
                            nc.vector.tensor_scalar(
                                out=h8s[:, osl], in0=ph[:, :480],
                                scalar1=l0b[:], scalar2=0.0,
                                op0=OP.add, op1=OP.max)
                nc.sync.dma_start(out=h1f8[s_ * D1:(s_ + 1) * D1, :], in_=h8s[:])

            # own-shard h in f32 from the per-core xT input
            for hh in range(2):
                xt0 = sgp.tile([NF, HL], bf16, tag="qsst")
                nc.sync.dma_start(out=xt0[:], in_=xT[:, hh * HL:(hh + 1) * HL])
                for j in range(4):
                    sl = slice(hh * HL + j * 480, hh * HL + (j + 1) * 480)
                    ph = ppB.tile([D1, 512], f32, tag="bld")
                    nc.tensor.matmul(out=ph[:, :480], lhsT=l0w[:],
                                     rhs=xt0[:, j * 480:(j + 1) * 480],
                                     start=True, stop=True)
                    nc.scalar.activation(out=hT_own[:, sl], in_=ph[:, :480],
                                         func=AF.Relu, bias=l0b[:], scale=1.0)
                    nc.vector.tensor_scalar(
                        out=hb_own[:, sl], in0=ph[:, :480],
                        scalar1=l0b[:], scalar2=0.0, op0=OP.add, op1=OP.max)

            # remaining constants: emitted after lin0 so their DMA (notably
            # the 60KB/partition one-hot + gather indices) doesn't serialize
            # ahead of the xTF streams in the SP/DMA queues
            ws = load_const("ws", wsrc[:, :], [D1, L * 128], bf16)
            wd = load_const("wd", wdst[:, :], [D1, L * 128], bf16)
            we = load_const("we", wea[:, :], [64, L * 128], bf16)
            io128 = load_const("io128", iota128[:, :], [128, 128], bf16)
            idnb = load_const("idnb", identb[:, :], [128, 128], bf16)
            dlp = load_const("dlp", dstloc_p[:, :], [128, nchunk])
            rcp = load_const("rcp", rc_p[:, :], [128, nchunk])
            gmt = load_const("gmt", bng[:, :], [D1, L + 1])
            bbt = load_const("bbt", bnb[:, :], [D1, L])
            io256 = load_const("io256", iota256[:, :], [128, G], bf16)
            idn = load_const("idn", ident[:, :], [128, 128])
            blc = load_const("blc", batchloc[:, :], [128, NWIN])
            rgp = load_const("rgp", rgc_pn[:, :], [128, NWIN])
            l1w = load_const("l1w", lin1w[:, :], [D1, D2])
            l1b = load_const("l1b", lin1b[:, :], [D2, 1])
            fw = load_const("fw", fcw[:, :], [D2, FC * D2])
            fb = load_const("fb", fcb[:, :], [D2, FC])
            l2w = load_const("l2w", lin2w[:, :], [D2, 1])
            l2b = load_const("l2b", lin2b[:, :], [1, 1])
            qsix = load_const("qsix", qs_idxD[:, :], [128, etot // 16], i16)
            ohT_res = cp.tile([128, nchunk, 128], f8, tag="ohres")
            nc.sync.dma_start(
                out=ohT_res[:].rearrange("p a b -> p (a b)"), in_=ohTD[:, :])

            # ---------- layers ----------
            for l in range(L):
                wd_l = wd[:, l * 128:(l + 1) * 128]
                ws_l = ws[:, l * 128:(l + 1) * 128]
                we_l = we[:, l * 128:(l + 1) * 128]

                if l == 0:
                    src_dram = h1f8
                else:
                    # --- AllGather h (fp8, staged into ag_in at layer end) ---
                    nc.gpsimd.collective_compute(
                        "AllGather", OP.bypass, replica_groups=rg,
                        ins=[ag_in.ap().opt()], outs=[ag_out.ap().opt()])
                    src_dram = ag_out

                # --- Qd table build (own shard) ---
                for w0 in range(0, NWIN, 4):
                    kk = min(4, NWIN - w0)
                    qp = ppB.tile([128, 512], f32, tag="bld")
                    for k in range(kk):
                        w = w0 + k
                        nc.tensor.matmul(
                            out=qp[:, k * 128:(k + 1) * 128],
                            lhsT=hb_own[:, w * 128:(w + 1) * 128],
                            rhs=wd_l, start=True, stop=True)
                    nc.vector.tensor_copy(
                        out=qd_sb[:, w0:w0 + kk, :].rearrange("p a b -> p (a b)"),
                        in_=qp[:, :kk * 128])

                # --- Qs table build (all nodes, per gathered shard) -> QsD ---
                ws8 = stp.tile([D1, 128], f8, tag="ws8")
                nc.scalar.activation(out=ws8[:], in_=ws_l,
                                     func=AF.Identity, scale=1.0)
                ncopy = 0
                for s_ in range(NCORES):
                    hb_sh = stp.tile([D1, SHARD_P], f8, tag="h8")
                    nc.sync.dma_start(out=hb_sh[:],
                                      in_=src_dram[s_ * D1:(s_ + 1) * D1, :])
                    for wB in range(0, NWIN, 16):
                        kB = min(16, NWIN - wB)
                        sg_t = sgp.tile([128, 16, 128], bf16, tag="qsst")
                        for w0 in range(wB, wB + kB, 4):
                            kk = min(4, wB + kB - w0)
                            qp = ppB.tile([128, 512], f32, tag="bld")
                            for k in range(kk):
                                w = w0 + k
                                nc.tensor.matmul(
                                    out=qp[:, k * 128:(k + 1) * 128],
                                    lhsT=hb_sh[:, w * 128:(w + 1) * 128],
                                    rhs=ws8[:], start=True, stop=True)
                            dst_ap = sg_t[:, w0 - wB:w0 - wB + kk, :] \
                                .rearrange("p a b -> p (a b)")
                            eng = ncopy % 2
                            ncopy += 1
                            if eng in (0,):
                                nc.vector.tensor_copy(
                                    out=dst_ap, in_=qp[:, :kk * 128])
                            else:
                                nc.scalar.activation(
                                    out=dst_ap, in_=qp[:, :kk * 128],
                                    func=AF.Identity, scale=1.0)
                        W0 = s_ * NWIN + wB
                        nc.sync.dma_start(out=QsD3[:, W0:W0 + kB, :],
                                          in_=sg_t[:, :kB, :])

                # --- edge pipeline ---
                st1g = wp.tile([D1, 8], f32, tag="st1g")
                st2g = wp.tile([D1, 8], f32, tag="st2g")
                agg = None
                qs_g = None
                aggst = {"agg": None}

                def emit_agg(m, ohs_t, t, te):
                    # aggregation for tile t, deferred one tile so the PE
                    # queue never stalls waiting for m
                    for c in range(te):
                        gc = t * 8 + c
                        w = gc // cw
                        if gc % (4 * cw) == 0:
                            agg_new = ppB.tile([D1, 512], f32, tag="bld")
                            aggst["agg"] = agg_new
                        agg = aggst["agg"]
                        nc.tensor.matmul(
                            out=agg[:, (w % 4) * 128:(w % 4 + 1) * 128],
                            lhsT=m[:, c, :], rhs=ohs_t[c][:],
                            start=(gc % cw == 0), stop=(gc % cw == cw - 1))
                        if gc % (4 * cw) == 4 * cw - 1 or gc == nchunk - 1:
                            grp = w // 4
                            lo = grp * 512
                            hi = min(lo + 512, SHARD_P)
                            nc.scalar.activation(
                                out=aggr_sb[:, lo:hi], in_=agg[:, :hi - lo],
                                func=AF.Identity, scale=1.0)
                            nc.vector.reduce_sum(
                                out=st1g[:, grp:grp + 1],
                                in_=aggr_sb[:, lo:hi],
                                axis=mybir.AxisListType.X)
                            sqg = nlp.tile([D1, 512], bf16, tag="sqg")
                            nc.vector.tensor_tensor(
                                out=sqg[:, :hi - lo], in0=aggr_sb[:, lo:hi],
                                in1=aggr_sb[:, lo:hi], op=OP.mult)
                            nc.vector.reduce_sum(
                                out=st2g[:, grp:grp + 1],
                                in_=sqg[:, :hi - lo],
                                axis=mybir.AxisListType.X)

                pend = []
                for t in range(ntile):
                    te = min(8, nchunk - t * 8)          # chunks this tile
                    ne = te * 128                        # edges this tile
                    if t % 4 == 0:
                        tc32 = min(32, nchunk - t * 8)
                        et = ep.tile([64, 4096], f8, tag="et")
                        nc.sync.dma_start(
                            out=et[:, :tc32 * 128],
                            in_=eaT[:, t * 1024: t * 1024 + tc32 * 128])
                    qs_g = gp.tile([128, 8, 128], bf16, tag="qsg")
                    nc.gpsimd.dma_gather(
                        qs_g[:, :te, :], QsD[:, :],
                        qsix[:, t * 64: t * 64 + te * 8],
                        te * 128, te * 128, 128)
                    half = 0
                    qhalf = (t % 4) * 8

                    # one-hot aggregation matrices: const-only deps, built
                    # ahead so the agg matmuls never wait on DVE
                    ohs_t = []
                    for c in range(te):
                        gc = t * 8 + c
                        oh_ = ohp.tile([128, 128], bf16, tag="ohS")
                        nc.vector.tensor_scalar(
                            out=oh_[:], in0=io128[:],
                            scalar1=dlp[:, gc:gc + 1], scalar2=rcp[:, gc:gc + 1],
                            op0=OP.is_equal, op1=OP.mult)
                        ohs_t.append(oh_)

                    pre = pp.tile([128, 1024], f32, tag="pre")
                    qs_f = qs_g[:].rearrange("p a b -> p (a b)")
                    for c in range(te):
                        gc = t * 8 + c
                        w = gc // cw
                        csl = slice(c * 128, (c + 1) * 128)
                        csl2 = slice((half + c) * 128, (half + c + 1) * 128)
                        csl4 = slice((qhalf + c) * 128, (qhalf + c + 1) * 128)
                        nc.tensor.matmul(out=pre[:, csl], lhsT=et[:, csl4],
                                         rhs=we_l, start=True, stop=False)
                        nc.tensor.matmul(out=pre[:, csl], lhsT=idnb[:],
                                         rhs=qs_f[:, csl2], start=False, stop=False)
                        nc.tensor.matmul(out=pre[:, csl], lhsT=ohT_res[:, gc, :],
                                         rhs=qd_sb[:, w, :], start=False, stop=True)

                    # nonlinearity: m = (1+tanh(a/2)) * softplus(b)
                    #             = 2*sigmoid(a)*softplus(b)  (2 absorbed by BN)
                    # nonlinearity: u = [e^-a | e^b], v = ln(1+u) = [sp(-a)|sp(b)]
                    # even tiles (ACT): sigma = e^-sp(-a); odd tiles (DVE):
                    # sigma = 1/(1+e^-a). m = 2*sigma*sp(b) (2 absorbed by BN
                    # via 4*EPS).
                    uf = nlp.tile([128, 8, 128], bf16, tag="uf")
                    nc.scalar.activation(
                        out=uf[:, :te, :].rearrange("p a b -> p (a b)"),
                        in_=pre[:, :ne], func=AF.Exp, scale=1.0)
                    m = nlp.tile([128, 8, 64], bf16, tag="m")
                    if t % 3 == 0:
                        vf = nlp.tile([128, 8, 128], bf16, tag="vf")
                        nc.scalar.activation(
                            out=vf[:, :te, :].rearrange("p a b -> p (a b)"),
                            in_=uf[:, :te, :].rearrange("p a b -> p (a b)"),
                            func=AF.Ln, bias=1.0, scale=1.0)
                        sg = nlp.tile([128, 8, 64], bf16, tag="sg")
                        nc.scalar.activation(out=sg[:, :te, :],
                                             in_=vf[:, :te, 0:64],
                                             func=AF.Exp, scale=-1.0)
                        nc.vector.scalar_tensor_tensor(
                            out=m[:, :te, :], in0=sg[:, :te, :], scalar=2.0,
                            in1=vf[:, :te, 64:128], op0=OP.mult, op1=OP.mult)
                    else:
                        vs = nlp.tile([128, 8, 64], bf16, tag="vs")
                        nc.scalar.activation(out=vs[:, :te, :],
                                             in_=uf[:, :te, 64:128],
                                             func=AF.Ln, bias=1.0, scale=1.0)
                        w1 = nlp.tile([128, 8, 64], bf16, tag="sg")
                        with nc.allow_low_precision(reason="sigmoid in bf16"):
                            nc.vector.tensor_scalar(out=w1[:, :te, :],
                                                    in0=uf[:, :te, 0:64],
                                                    scalar1=1.0, scalar2=None,
                                                    op0=OP.add)
                            nc.vector.reciprocal(out=w1[:, :te, :],
                                                 in_=w1[:, :te, :])
                        nc.vector.scalar_tensor_tensor(
                            out=m[:, :te, :], in0=w1[:, :te, :], scalar=2.0,
                            in1=vs[:, :te, :], op0=OP.mult, op1=OP.mult)

                    pend.append((m, ohs_t, t, te))
                    if len(pend) > 1:
                        emit_agg(*pend.pop(0))
                while pend:
                    emit_agg(*pend.pop(0))

                # --- BN stats + AllReduce ---
                st = wp.tile([D1, 2], f32, tag="st")
                nc.vector.reduce_sum(out=st[:, 0:1], in_=st1g[:],
                                     axis=mybir.AxisListType.X)
                nc.vector.reduce_sum(out=st[:, 1:2], in_=st2g[:],
                                     axis=mybir.AxisListType.X)
                nc.sync.dma_start(out=ar_in[:, :], in_=st[:])
                nc.gpsimd.collective_compute(
                    "AllGather", OP.bypass, replica_groups=rg,
                    ins=[ar_in.ap().opt()], outs=[ar_out.ap().opt()])
                stga = wp.tile([D1, 2, NCORES], f32, tag="stga")
                nc.sync.dma_start(
                    out=stga[:],
                    in_=ar_out[:, :].rearrange("(c p) s -> p s c", p=D1))
                stg = wp.tile([D1, 2], f32, tag="stg")
                nc.vector.reduce_sum(
                    out=stg[:].rearrange("p (s o) -> p s o", o=1),
                    in_=stga[:], axis=mybir.AxisListType.X)
                mu = wp.tile([D1, 1], f32, tag="mu")
                nc.vector.tensor_scalar(out=mu[:], in0=stg[:, 0:1],
                                        scalar1=1.0 / N, scalar2=None, op0=OP.mult)
                ex2 = wp.tile([D1, 1], f32, tag="ex2")
                nc.vector.tensor_scalar(out=ex2[:], in0=stg[:, 1:2],
                                        scalar1=1.0 / N, scalar2=None, op0=OP.mult)
                nvar = wp.tile([D1, 1], f32, tag="var")
                nc.vector.scalar_tensor_tensor(
                    out=nvar[:], in0=mu[:], scalar=mu[:], in1=ex2[:],
                    op0=OP.mult, op1=OP.subtract)
                lv = wp.tile([D1, 1], f32, tag="lv")
                # m carries a factor 2 -> aggr/mu scale by 2, var by 4; 4*EPS
                # rides as an extra bng column; nvar = mu^2-ex2 = -var, so the
                # Ln input is nvar*(-1) + 4*EPS.
                nc.scalar.activation(out=lv[:], in_=nvar[:], func=AF.Ln,
                                     bias=gmt[:, L:L + 1], scale=-1.0)
                isd = wp.tile([D1, 1], f32, tag="isd")
                nc.scalar.activation(out=isd[:], in_=lv[:], func=AF.Exp, scale=-0.5)
                scale = wp.tile([D1, 1], f32, tag="scale")
                nc.vector.tensor_tensor(out=scale[:], in0=isd[:],
                                        in1=gmt[:, l:l + 1], op=OP.mult)
                mshift = wp.tile([D1, 1], f32, tag="mshift")
                nc.vector.tensor_tensor(out=mshift[:], in0=mu[:], in1=scale[:],
                                        op=OP.mult)
                shift = wp.tile([D1, 1], f32, tag="shift")
                nc.vector.tensor_tensor(out=shift[:], in0=bbt[:, l:l + 1],
                                        in1=mshift[:], op=OP.subtract)
                # h = relu((aggr*scale + h) + shift); the three consumers
                # (f32 residual, f8 AllGather payload, bf16 matmul copy) are
                # produced from asb concurrently on DVE/ACT/Pool
                HB = SHARD_P // 2
                for hh in range(2):
                    hsl = slice(hh * HB, (hh + 1) * HB)
                    nc.vector.scalar_tensor_tensor(
                        out=asb[:, hsl], in0=aggr_sb[:, hsl], scalar=scale[:],
                        in1=hT_own[:, hsl], op0=OP.mult, op1=OP.add)
                if l + 1 < L:
                    h8n = stp.tile([D1, SHARD_P], f8, tag="h8")
                    for hh in range(2):
                        hsl = slice(hh * HB, (hh + 1) * HB)
                        nc.scalar.activation(out=h8n[:, hsl], in_=asb[:, hsl],
                                             func=AF.Relu, bias=shift[:],
                                             scale=1.0)
                        nc.sync.dma_start(out=ag_in[:, hsl], in_=h8n[:, hsl])
                if l + 1 < L:
                    nc.vector.tensor_scalar(out=hT_own[:], in0=asb[:],
                                            scalar1=shift[:], scalar2=0.0,
                                            op0=OP.add, op1=OP.max)
                    nc.gpsimd.tensor_copy(out=hb_own[:], in_=hT_own[:])
                else:
                    # last layer: no AllGather payload to produce on ACT and
                    # no next Qd build needing hb_own; do the relu on ACT in
                    # halves (pipelined behind the stt halves on DVE) so the
                    # pool phase starts sooner
                    for hh in range(2):
                        hsl = slice(hh * HB, (hh + 1) * HB)
                        nc.scalar.activation(out=hT_own[:, hsl],
                                             in_=asb[:, hsl], func=AF.Relu,
                                             bias=shift[:], scale=1.0)

            # ---------- global mean pool ----------
            pool_ps = pp.tile([D1, G], f32, tag="pre")
            for w in range(NWIN):
                tp = ppB.tile([128, D1], f32, tag="bld")
                nc.tensor.transpose(out=tp[:], in_=hT_own[:, w * 128:(w + 1) * 128],
                                    identity=idn[0:D1, 0:D1])
                rows = wp.tile([128, D1], bf16, tag="rows")
                nc.vector.tensor_copy(out=rows[:], in_=tp[:])
                ohg = ohgp.tile([128, G], bf16, tag="ohg")
                nc.vector.tensor_scalar(
                    out=ohg[:], in0=io256[:],
                    scalar1=blc[:, w:w + 1], scalar2=rgp[:, w:w + 1],
                    op0=OP.is_equal, op1=OP.mult)
                nc.tensor.matmul(out=pool_ps[:], lhsT=rows[:], rhs=ohg[:],
                                 start=(w == 0), stop=(w == NWIN - 1))
            poolT = tlp.tile([D1, G], bf16, tag="poolT")
            nc.vector.tensor_copy(out=poolT[:], in_=pool_ps[:])
            nc.sync.dma_start(out=pl_in[:, :], in_=poolT[:])
            nc.gpsimd.collective_compute(
                "AllGather", OP.bypass, replica_groups=rg,
                ins=[pl_in.ap().opt()], outs=[pl_out.ap().opt()])
            pga = tlp.tile([D1, NCORES, G], bf16, tag="pga")
            nc.sync.dma_start(
                out=pga[:],
                in_=pl_out[:, :].rearrange("(c p) g -> p c g", p=D1))
            pg = tlp.tile([D1, G], f32, tag="pg")
            nc.vector.reduce_sum(
                out=pg[:].rearrange("p (g o) -> p g o", o=1),
                in_=pga[:].rearrange("p c g -> p g c"),
                axis=mybir.AxisListType.X)

            # ---------- head ----------
            a = pg
            hw_ = [(l1w[:], l1b[:]), (fw[:, 0:D2], fb[:, 0:1]), (fw[:, D2:2 * D2], fb[:, 1:2])]
            for (wt, bt) in hw_:
                ps = ppB.tile([D2, G], f32, tag="bld")
                nc.tensor.matmul(out=ps[:, 0:G], lhsT=wt, rhs=a[:], start=True, stop=True)
                an = tlp.tile([D2, G], f32, tag="an")
                nc.scalar.activation(out=an[:], in_=ps[:, 0:G], func=AF.Relu,
                                     bias=bt, scale=1.0)
                a = an
            ps = ppB.tile([1, G], f32, tag="bld")
            nc.tensor.matmul(out=ps[:, 0:G], lhsT=l2w[:], rhs=a[:], start=True, stop=True)
            yt = tlp.tile([1, G], f32, tag="yt")
            nc.scalar.activation(out=yt[:], in_=ps[:, 0:G], func=AF.Identity,
                                 bias=l2b[:], scale=1.0)
            nc.sync.dma_start(out=yout[:, :], in_=yt[:])

    nc.compile()
    return nc


def _wrap16(idx):
    """Flat idx list -> [128, n/16] int16: slot i at [i%16, i//16], replicated
    across the 8 Q7 cores."""
    a = idx.reshape(-1, 16).T.astype(np.int16)
    return np.tile(a, (8, 1))


def _preprocess(inputs):
    x = np.asarray(inputs["x"], np.float32)
    ea = np.asarray(inputs["edge_attr"], np.float32)
    ei = np.asarray(inputs["edge_index"]).astype(np.int64)
    batch = np.asarray(inputs["batch"]).astype(np.int64)
    src, dst = ei[0], ei[1]

    cnt = np.bincount(dst, minlength=N).astype(np.float32)
    rc_node = 1.0 / np.maximum(cnt, 1.0)
    gcnt = np.bincount(batch, minlength=G).astype(np.float32)
    rgc = 1.0 / np.maximum(gcnt, 1.0)

    # Degree-balanced node -> (window, slot) assignment: snake-deal nodes in
    # descending-degree order across the 240 global windows, minimizing the
    # max per-window edge count (which sets the uniform chunk pad cw).
    deg_order = np.argsort(-cnt, kind="stable")       # node ids, deg desc
    nwin_g = NCORES * NWIN                            # 240
    perm_loc = np.empty(N, np.int64)                  # node -> global padded id
    for i0 in range(0, N, nwin_g):
        blk = deg_order[i0:i0 + nwin_g]
        j = i0 // nwin_g
        wins = np.arange(len(blk)) if j % 2 == 0 else (len(blk) - 1 - np.arange(len(blk)))
        w_ids = wins
        perm_loc[blk] = (w_ids // NWIN) * SHARD_P + (w_ids % NWIN) * 128 + j
    gperm = perm_loc
    srcg = gperm[src]
    dstg = gperm[dst]
    order = np.argsort(dstg, kind="stable")
    srcg_s, dstg_s, ea_idx = srcg[order], dstg[order], order

    bounds = []
    for c in range(NCORES):
        for w in range(NWIN):
            bounds.append(c * SHARD_P + w * 128)
    bounds.append(NCORES * SHARD_P)
    bpos = np.searchsorted(dstg_s, np.asarray(bounds), side="left")
    percw = {}
    maxcnt = 0
    k = 0
    for c in range(NCORES):
        for w in range(NWIN):
            lo, hi = bpos[k], bpos[k + 1]
            percw[(c, w)] = np.arange(lo, hi)
            maxcnt = max(maxcnt, hi - lo)
            k += 1
    cw = max(1, (maxcnt + 127) // 128)
    etot = NWIN * cw * 128

    # full padded x, rotated per core so block 0 is the own shard
    xfull = np.zeros((NF, NCORES * SHARD_P), np.float32)
    xfull[:, gperm] = x.T
    xfull = xfull.astype(ml_dtypes.bfloat16)

    per_core = []
    for c in range(NCORES):
        qs_idx = np.zeros(etot, np.int64)
        dl = np.full(etot, -1.0, np.float32)
        rc_e = np.ones(etot, np.float32)
        ea_e = np.zeros((etot, EF), np.float32)
        for w in range(NWIN):
            idxs = percw[(c, w)]
            o = w * cw * 128
            k = len(idxs)
            g = srcg_s[idxs]                           # padded global id
            qs_idx[o:o + k] = (g % 128) * NWING + (g // 128)
            loc = dstg_s[idxs] - c * SHARD_P           # 0..3839
            dl[o:o + k] = (loc - w * 128).astype(np.float32)
            rc_e[o:o + k] = rc_node[dst[ea_idx[idxs]]]
            ea_e[o:o + k] = ea[ea_idx[idxs]]
        eaT = np.zeros((64, etot), np.float32)
        eaT[1:EF + 1] = ea_e.T
        eaT[EF + 1] = 1.0
        eaT[EF + 1, dl < 0] = 0.0
        nch = etot // 128
        ohT = np.zeros((128, etot), np.float32)
        vv = dl >= 0
        ohT[dl[vv].astype(np.int64), np.nonzero(vv)[0]] = 1.0
        d = {
            "qs_idxD": _wrap16(qs_idx),
            "ohTD": ohT.astype(ml_dtypes.float8_e4m3),
            "dstloc_p": dl.reshape(nch, 128).T.copy(),
            "rc_p": rc_e.reshape(nch, 128).T.copy(),
            "eaT": eaT.astype(ml_dtypes.float8_e4m3),
        }
        d["xTF"] = xfull
        d["xT"] = xfull[:, c * SHARD_P:(c + 1) * SHARD_P].copy()
        nodes_c = np.nonzero((gperm // SHARD_P) == c)[0]
        locs_c = gperm[nodes_c] - c * SHARD_P
        bl = np.full(SHARD_P, -1.0, np.float32)
        bl[locs_c] = batch[nodes_c].astype(np.float32)
        rg_n = np.zeros(SHARD_P, np.float32)
        rg_n[locs_c] = rgc[batch[nodes_c]]
        d["batchloc"] = bl.reshape(NWIN, 128).T.copy()
        d["rgc_pn"] = rg_n.reshape(NWIN, 128).T.copy()
        per_core.append(d)

    # replicated weights; f-gate halves pre-negated
    wf = np.asarray(inputs["conv_wf"], np.float32)
    wsv = np.asarray(inputs["conv_ws"], np.float32)
    bf = np.asarray(inputs["conv_bf"], np.float32)
    bs = np.asarray(inputs["conv_bs"], np.float32)
    wdst = np.concatenate([-wf[:, 0:D1, :], wsv[:, 0:D1, :]], axis=2)
    wsrc = np.concatenate([-wf[:, D1:2 * D1, :], wsv[:, D1:2 * D1, :]], axis=2)
    wea = np.concatenate([-wf[:, 2 * D1:, :], wsv[:, 2 * D1:, :]], axis=2)
    bias = np.concatenate([-bf, bs], axis=1)[:, None, :]
    wea = np.concatenate([wea, bias], axis=1)
    shared = {
        "lin0w": np.asarray(inputs["lin0_w"], np.float32).astype(ml_dtypes.bfloat16),
        "lin0b": np.asarray(inputs["lin0_b"], np.float32).reshape(D1, 1),
        "wdst": np.transpose(wdst, (1, 0, 2)).reshape(D1, L * 128).astype(ml_dtypes.bfloat16),
        "wsrc": np.transpose(wsrc, (1, 0, 2)).reshape(D1, L * 128).astype(ml_dtypes.bfloat16),
        "wea": np.concatenate([
            np.zeros((1, L * 128), np.float32),
            np.transpose(wea, (1, 0, 2)).reshape(EF + 1, L * 128),
            np.zeros((64 - EF - 2, L * 128), np.float32),
        ], axis=0).astype(ml_dtypes.bfloat16),
        "bng": np.concatenate(
            [np.asarray(inputs["bn_gamma"], np.float32).T,
             np.full((D1, 1), 4.0 * EPS, np.float32)], axis=1),
        "bnb": np.asarray(inputs["bn_beta"], np.float32).T.copy(),
        "lin1w": np.asarray(inputs["lin1_w"], np.float32),
        "lin1b": np.asarray(inputs["lin1_b"], np.float32).reshape(D2, 1),
        "fcw": np.transpose(np.asarray(inputs["fc_w"], np.float32), (1, 0, 2)).reshape(D2, FC * D2),
        "fcb": np.asarray(inputs["fc_b"], np.float32).T.copy(),
        "lin2w": np.asarray(inputs["lin2_w"], np.float32).reshape(D2, 1),
        "lin2b": np.asarray(inputs["lin2_b"], np.float32).reshape(1, 1),
        "iota128": np.broadcast_to(np.arange(128, dtype=np.float32)[None, :],
                                   (128, 128)).astype(ml_dtypes.bfloat16),
        "iota256": np.broadcast_to(np.arange(G, dtype=np.float32)[None, :],
                                   (128, G)).astype(ml_dtypes.bfloat16),
        "ident": np.eye(128, dtype=np.float32),
        "identb": np.eye(128, dtype=np.float32).astype(ml_dtypes.bfloat16),
    }
    in_maps = [dict(shared, **pc) for pc in per_core]
    return in_maps, cw


def kernel(**inputs):
    from concourse.bass_utils import run_bass_kernel_spmd

    in_maps, cw = _preprocess(inputs)
    key = ("nc", cw)
    if key not in _CACHE:
        _CACHE[key] = _build_nc(cw)
    nc = _CACHE[key]
    res = run_bass_kernel_spmd(nc, in_maps, core_ids=list(range(NCORES)))
    return res.results[0]["y"].reshape(G).astype(np.float32)


# revision 68
# speedup vs baseline: 1.0224x; 1.0004x over previous
"""CGCNN message-passing kernel for 8 Trainium2 NeuronCores (Bass/Tile), v9.

Data-parallel by dst shard; gather-based edge pipeline:
- Host: nodes are dealt into 240 global windows (8 cores x 30 windows x 128
  slots) in descending-degree snake order, equalizing per-window edge counts
  so the uniform chunks-per-window pad cw is minimal (16). Edges go to the
  core owning their dst, grouped by dst window, chunk-padded to cw.
- lin0 is computed for ALL nodes redundantly on every core (from a replicated
  full xT) into a DRAM fp8 table, so layer 0 needs no h AllGather; layers
  1..L-1 AllGather h in fp8 (staged at the previous layer's BN boundary so
  the collective launches as early as possible).
- Per layer, per core:
  * Qd table (own shard, SBUF bf16 [128, 30, 128]) = h_own @ Wdst.
  * Full Qs table = h_full @ Wsrc into DRAM [30720, 128] bf16 in
    partition-major row order (node g -> row (g%128)*240 + g//128); PSUM->SBUF
    staging copies round-robin over DVE/ACT to minimize build latency.
  * Per 1024-edge tile: one dma_gather pulls per-edge Qs rows (1024 x 256B
    descriptors; 1024 = SWDGE ring capacity); Qe = ea(fp8) @ Wea by matmul
    (edge attrs streamed fp8, 4 tiles per DMA); the dst contribution expands
    via a host-precomputed one-hot (fp8, SBUF-resident, layer-invariant)
    matmul against the SBUF Qd table. All three accumulate in PSUM per
    128-edge chunk.
  * Nonlinearity: joint exp u = [e^-a | e^b] (f-gate weights pre-negated),
    v = ln(1+u); 1/3 of tiles compute sigmoid(a) = e^-v_f on ACT, 2/3 as
    1/(1+u_f) on DVE (bf16, engine balance). m = 2*sigmoid(a)*softplus(b);
    the factor 2 is absorbed exactly by BatchNorm using 4*EPS.
  * Aggregation one-hots (is_equal(iota, dst) * 1/cnt, bf16 on DVE) are
    pre-built per tile, and the aggregation matmuls are deferred by one tile
    so the in-order PE queue never stalls on the ACT/DVE nonlinearity chain.
  * Segment-mean accumulates per dst window in PSUM (agg PSUM shares banks
    with the build-phase staging, freeing a third pre-PSUM buffer); BatchNorm
    batch stats via a tiny stats AllGather + local sum; the residual
    (scalar_tensor_tensor + relu) is computed in halves, with the fp8
    AllGather payload produced on ACT in parallel with the f32/bf16 h copies
    on DVE/Pool.
- Global mean pool via one-hot matmul, bf16 partials AllGathered and summed
  locally, head MLP computed redundantly on every core.
"""
import numpy as np
import ml_dtypes

N = 30000
E = 480000
NF = 92
EF = 50
D1 = 64
D2 = 64
L = 3
FC = 2
G = 256
EPS = 1e-5
NCORES = 8
SHARD = N // NCORES            # 3750
SHARD_P = 3840                 # padded shard (30 windows of 128)
NWIN = SHARD_P // 128          # 30
NWING = NCORES * NWIN          # 240 global windows
TBL = NCORES * SHARD_P         # 30720 table rows

_CACHE = {}



def _build_nc(cw):
    """Build the SPMD bass module. cw = chunks per dst window (uniform)."""
    import concourse.mybir as mybir
    from concourse import bacc
    from concourse.tile import TileContext

    f32 = mybir.dt.float32
    bf16 = mybir.dt.bfloat16
    f8 = mybir.dt.float8e4
    i16 = mybir.dt.int16
    AF = mybir.ActivationFunctionType
    OP = mybir.AluOpType

    nchunk = NWIN * cw                 # chunks per layer per core
    etot = nchunk * 128                # padded edges per core
    ntile = (nchunk + 7) // 8          # 8-chunk (1024-edge) PSUM tiles

    import concourse.hw_specs as _hw
    import concourse.bacc as _bacc_mod
    _real_tables = _hw.get_activation_tables("gen3")
    _combined = "natural_log_exp_and_others"
    if _combined in _real_tables:
        _patched = {
            k: (v if k == _combined else (v - {AF.Exp, AF.Ln}))
            for k, v in _real_tables.items()
        }
        _bacc_mod.get_activation_tables = lambda arch: _patched

    nc = bacc.Bacc(None, target_bir_lowering=False)

    # ---- inputs (per core) ----
    xTF = nc.dram_tensor("xTF", [NF, TBL], bf16, kind="ExternalInput")
    xT = nc.dram_tensor("xT", [NF, SHARD_P], bf16, kind="ExternalInput")
    eaT = nc.dram_tensor("eaT", [64, etot], f8, kind="ExternalInput")
    qs_idxD = nc.dram_tensor("qs_idxD", [128, etot // 16], i16, kind="ExternalInput")
    ohTD = nc.dram_tensor("ohTD", [128, etot], f8, kind="ExternalInput")
    dstloc_p = nc.dram_tensor("dstloc_p", [128, nchunk], f32, kind="ExternalInput")
    rc_p = nc.dram_tensor("rc_p", [128, nchunk], f32, kind="ExternalInput")
    batchloc = nc.dram_tensor("batchloc", [128, NWIN], f32, kind="ExternalInput")
    rgc_pn = nc.dram_tensor("rgc_pn", [128, NWIN], f32, kind="ExternalInput")
    # weights (replicated; f-gate halves pre-negated)
    lin0w = nc.dram_tensor("lin0w", [NF, D1], bf16, kind="ExternalInput")
    lin0b = nc.dram_tensor("lin0b", [D1, 1], f32, kind="ExternalInput")
    wdst = nc.dram_tensor("wdst", [D1, L * 128], bf16, kind="ExternalInput")
    wsrc = nc.dram_tensor("wsrc", [D1, L * 128], bf16, kind="ExternalInput")
    wea = nc.dram_tensor("wea", [64, L * 128], bf16, kind="ExternalInput")
    bng = nc.dram_tensor("bng", [D1, L + 1], f32, kind="ExternalInput")
    bnb = nc.dram_tensor("bnb", [D1, L], f32, kind="ExternalInput")
    lin1w = nc.dram_tensor("lin1w", [D1, D2], f32, kind="ExternalInput")
    lin1b = nc.dram_tensor("lin1b", [D2, 1], f32, kind="ExternalInput")
    fcw = nc.dram_tensor("fcw", [D2, FC * D2], f32, kind="ExternalInput")
    fcb = nc.dram_tensor("fcb", [D2, FC], f32, kind="ExternalInput")
    lin2w = nc.dram_tensor("lin2w", [D2, 1], f32, kind="ExternalInput")
    lin2b = nc.dram_tensor("lin2b", [1, 1], f32, kind="ExternalInput")
    iota128 = nc.dram_tensor("iota128", [128, 128], bf16, kind="ExternalInput")
    iota256 = nc.dram_tensor("iota256", [128, G], bf16, kind="ExternalInput")
    ident = nc.dram_tensor("ident", [128, 128], f32, kind="ExternalInput")
    identb = nc.dram_tensor("identb", [128, 128], bf16, kind="ExternalInput")

    yout = nc.dram_tensor("y", [1, G], f32, kind="ExternalOutput")

    # ---- DRAM scratch ----
    QsD = nc.dram_tensor("QsD", [TBL, 128], bf16)          # row p*NWING+W
    h1f8 = nc.dram_tensor("h1f8", [NCORES * D1, SHARD_P], f8)
    ag_in = nc.dram_tensor("ag_in", [D1, SHARD_P], f8)
    ag_out = nc.dram_tensor("ag_out", [NCORES * D1, SHARD_P], f8,
                            addr_space="Shared")
    ar_in = nc.dram_tensor("ar_in", [D1, 2], f32)
    ar_out = nc.dram_tensor("ar_out", [NCORES * D1, 2], f32, addr_space="Shared")
    pl_in = nc.dram_tensor("pl_in", [D1, G], bf16)
    pl_out = nc.dram_tensor("pl_out", [NCORES * D1, G], bf16, addr_space="Shared")

    rg = [list(range(NCORES))]
    QsD3 = QsD[:, :].rearrange("(p w) f -> p w f", p=128)   # [128, NWING, 128]

    from contextlib import ExitStack
    with TileContext(nc) as tc:
        with ExitStack() as _es:
            cp = _es.enter_context(tc.tile_pool(name="const", bufs=1))
            bigp = _es.enter_context(tc.tile_pool(name="big", bufs=1))
            wp = _es.enter_context(tc.tile_pool(name="work", bufs=3))
            tlp = _es.enter_context(tc.tile_pool(name="tail", bufs=1))
            gp = _es.enter_context(tc.tile_pool(name="gat", bufs=4))
            ep = _es.enter_context(tc.tile_pool(name="ea", bufs=3))
            nlp = _es.enter_context(tc.tile_pool(name="nl", bufs=3))
            ohp = _es.enter_context(tc.tile_pool(name="oh", bufs=17))
            ohgp = _es.enter_context(tc.tile_pool(name="ohg", bufs=3))
            stp = _es.enter_context(tc.tile_pool(name="st", bufs=2))
            sgp = _es.enter_context(tc.tile_pool(name="sgp", bufs=3))
            scp = _es.enter_context(tc.tile_pool(name="scr", bufs=1))
            pp = _es.enter_context(tc.tile_pool(name="pre", bufs=3, space="PSUM"))
            ppB = _es.enter_context(tc.tile_pool(name="psB", bufs=2, space="PSUM"))
            # ---------- constants ----------
            def load_const(t, dram, shape, dtype=f32):
                tt = cp.tile(shape, dtype, tag=t)
                nc.sync.dma_start(out=tt[:], in_=dram)
                return tt

            l0w = load_const("l0w", lin0w[:, :], [NF, D1], bf16)
            l0b = load_const("l0b", lin0b[:, :], [D1, 1])

            # ---------- resident state ----------
            hT_own = bigp.tile([D1, SHARD_P], f32, tag="hown")
            hb_own = bigp.tile([D1, SHARD_P], bf16, tag="hbown")
            aggr_sb = bigp.tile([D1, SHARD_P], bf16, tag="aggr")
            qd_sb = bigp.tile([128, NWIN, 128], bf16, tag="qdsb")
            asb = scp.tile([D1, SHARD_P], f32, tag="asb")

            # ---------- lin0 for ALL nodes (no AllGather for layer 0) ----
            # h1f8 holds relu(x @ lin0_w + b) for all 8 shards (global order),
            # computed redundantly on every core from the replicated xTF.
            HL = SHARD_P // 2
            for s_ in range(NCORES):
                h8s = stp.tile([D1, SHARD_P], f8, tag="h8")
                for hh in range(2):
                    xt = sgp.tile([NF, HL], bf16, tag="qsst")
                    o = s_ * SHARD_P + hh * HL
                    nc.sync.dma_start(out=xt[:], in_=xTF[:, o:o + HL])
                    for j in range(4):
                        sl = slice(j * 480, (j + 1) * 480)
                        ph = ppB.tile([D1, 512], f32, tag="bld")
                        nc.tensor.matmul(out=ph[:, :480], lhsT=l0w[:],
                                         rhs=xt[:, sl], start=True, stop=True)
                        osl = slice(hh * HL + j * 480, hh * HL + (j + 1) * 480)
                        if j % 2 == 0:
                            nc.scalar.activation(
                                out=h8s[:, osl],
                                in_=ph[:, :480], func=AF.Relu, bias=l0b[:],
                                scale=1.0)
                        else:
                            nc.vector.tensor_scalar(
                                out=h8s[:, osl], in0=ph[:, :480],
                                scalar1=l0b[:], scalar2=0.0,
                                op0=OP.add, op1=OP.max)
                nc.sync.dma_start(out=h1f8[s_ * D1:(s_ + 1) * D1, :], in_=h8s[:])

            # own-shard h in f32 from the per-core xT input
            for hh in range(2):
                xt0 = sgp.tile([NF, HL], bf16, tag="qsst")
                nc.sync.dma_start(out=xt0[:], in_=xT[:, hh * HL:(hh + 1) * HL])
                for j in range(4):
                    sl = slice(hh * HL + j * 480, hh * HL + (j + 1) * 480)
                    ph = ppB.tile([D1, 512], f32, tag="bld")
                    nc.tensor.matmul(out=ph[:, :480], lhsT=l0w[:],
                                     rhs=xt0[:, j * 480:(j + 1) * 480],
                                     start=True, stop=True)
                    nc.scalar.activation(out=hT_own[:, sl], in_=ph[:, :480],
                                         func=AF.Relu, bias=l0b[:], scale=1.0)
                    nc.vector.tensor_scalar(
                        out=hb_own[:, sl], in0=ph[:, :480],
                        scalar1=l0b[:], scalar2=0.0, op0=OP.add, op1=OP.max)

            # remaining constants: emitted after lin0 so their DMA (notably
            # the 60KB/partition one-hot + gather indices) doesn't serialize
            # ahead of the xTF streams in the SP/DMA queues
            ws = load_const("ws", wsrc[:, :], [D1, L * 128], bf16)
            wd = load_const("wd", wdst[:, :], [D1, L * 128], bf16)
            we = load_const("we", wea[:, :], [64, L * 128], bf16)
            io128 = load_const("io128", iota128[:, :], [128, 128], bf16)
            idnb = load_const("idnb", identb[:, :], [128, 128], bf16)
            dlp = load_const("dlp", dstloc_p[:, :], [128, nchunk])
            rcp = load_const("rcp", rc_p[:, :], [128, nchunk])
            gmt = load_const("gmt", bng[:, :], [D1, L + 1])
            bbt = load_const("bbt", bnb[:, :], [D1, L])
            io256 = load_const("io256", iota256[:, :], [128, G], bf16)
            idn = load_const("idn", ident[:, :], [128, 128])
            blc = load_const("blc", batchloc[:, :], [128, NWIN])
            rgp = load_const("rgp", rgc_pn[:, :], [128, NWIN])
            l1w = load_const("l1w", lin1w[:, :], [D1, D2])
            l1b = load_const("l1b", lin1b[:, :], [D2, 1])
            fw = load_const("fw", fcw[:, :], [D2, FC * D2])
            fb = load_const("fb", fcb[:, :], [D2, FC])
            l2w = load_const("l2w", lin2w[:, :], [D2, 1])
            l2b = load_const("l2b", lin2b[:, :], [1, 1])
            qsix = load_const("qsix", qs_idxD[:, :], [128, etot // 16], i16)
            ohT_res = cp.tile([128, nchunk, 128], f8, tag="ohres")
            nc.sync.dma_start(
                out=ohT_res[:].rearrange("p a b -> p (a b)"), in_=ohTD[:, :])

            # ---------- layers ----------
            for l in range(L):
                wd_l = wd[:, l * 128:(l + 1) * 128]
                ws_l = ws[:, l * 128:(l + 1) * 128]
                we_l = we[:, l * 128:(l + 1) * 128]

                if l == 0:
                    src_dram = h1f8
                else:
                    # --- AllGather h (fp8, staged into ag_in at layer end) ---
                    nc.gpsimd.collective_compute(
                        "AllGather", OP.bypass, replica_groups=rg,
                        ins=[ag_in.ap().opt()], outs=[ag_out.ap().opt()])
                    src_dram = ag_out

                # --- Qd table build (own shard) ---
                for w0 in range(0, NWIN, 4):
                    kk = min(4, NWIN - w0)
                    qp = ppB.tile([128, 512], f32, tag="bld")
                    for k in range(kk):
                        w = w0 + k
                        nc.tensor.matmul(
                            out=qp[:, k * 128:(k + 1) * 128],
                            lhsT=hb_own[:, w * 128:(w + 1) * 128],
                            rhs=wd_l, start=True, stop=True)
                    nc.vector.tensor_copy(
                        out=qd_sb[:, w0:w0 + kk, :].rearrange("p a b -> p (a b)"),
                        in_=qp[:, :kk * 128])

                # --- Qs table build (all nodes, per gathered shard) -> QsD ---
                ws8 = stp.tile([D1, 128], f8, tag="ws8")
                nc.scalar.activation(out=ws8[:], in_=ws_l,
                                     func=AF.Identity, scale=1.0)
                ncopy = 0
                for s_ in range(NCORES):
                    hb_sh = stp.tile([D1, SHARD_P], f8, tag="h8")
                    nc.sync.dma_start(out=hb_sh[:],
                                      in_=src_dram[s_ * D1:(s_ + 1) * D1, :])
                    for wB in range(0, NWIN, 16):
                        kB = min(16, NWIN - wB)
                        sg_t = sgp.tile([128, 16, 128], bf16, tag="qsst")
                        for w0 in range(wB, wB + kB, 4):
                            kk = min(4, wB + kB - w0)
                            qp = ppB.tile([128, 512], f32, tag="bld")
                            for k in range(kk):
                                w = w0 + k
                                nc.tensor.matmul(
                                    out=qp[:, k * 128:(k + 1) * 128],
                                    lhsT=hb_sh[:, w * 128:(w + 1) * 128],
                                    rhs=ws8[:], start=True, stop=True)
                            dst_ap = sg_t[:, w0 - wB:w0 - wB + kk, :] \
                                .rearrange("p a b -> p (a b)")
                            eng = ncopy % 2
                            ncopy += 1
                            if eng in (0,):
                                nc.vector.tensor_copy(
                                    out=dst_ap, in_=qp[:, :kk * 128])
                            else:
                                nc.scalar.activation(
                                    out=dst_ap, in_=qp[:, :kk * 128],
                                    func=AF.Identity, scale=1.0)
                        W0 = s_ * NWIN + wB
                        nc.sync.dma_start(out=QsD3[:, W0:W0 + kB, :],
                                          in_=sg_t[:, :kB, :])

                # --- edge pipeline ---
                st1g = wp.tile([D1, 8], f32, tag="st1g")
                st2g = wp.tile([D1, 8], f32, tag="st2g")
                agg = None
                qs_g = None
                aggst = {"agg": None}

                def emit_agg(m, ohs_t, t, te):
                    # aggregation for tile t, deferred one tile so the PE
                    # queue never stalls waiting for m
                    for c in range(te):
                        gc = t * 8 + c
                        w = gc // cw
                        if gc % (4 * cw) == 0:
                            agg_new = ppB.tile([D1, 512], f32, tag="bld")
                            aggst["agg"] = agg_new
                        agg = aggst["agg"]
                        nc.tensor.matmul(
                            out=agg[:, (w % 4) * 128:(w % 4 + 1) * 128],
                            lhsT=m[:, c, :], rhs=ohs_t[c][:],
                            start=(gc % cw == 0), stop=(gc % cw == cw - 1))
                        if gc % (4 * cw) == 4 * cw - 1 or gc == nchunk - 1:
                            grp = w // 4
                            lo = grp * 512
                            hi = min(lo + 512, SHARD_P)
                            nc.scalar.activation(
                                out=aggr_sb[:, lo:hi], in_=agg[:, :hi - lo],
                                func=AF.Identity, scale=1.0)
                            nc.vector.reduce_sum(
                                out=st1g[:, grp:grp + 1],
                                in_=aggr_sb[:, lo:hi],
                                axis=mybir.AxisListType.X)
                            sqg = nlp.tile([D1, 512], bf16, tag="sqg")
                            nc.vector.tensor_tensor(
                                out=sqg[:, :hi - lo], in0=aggr_sb[:, lo:hi],
                                in1=aggr_sb[:, lo:hi], op=OP.mult)
                            nc.vector.reduce_sum(
                                out=st2g[:, grp:grp + 1],
                                in_=sqg[:, :hi - lo],
                                axis=mybir.AxisListType.X)

                pend = []
                for t in range(ntile):
                    te = min(8, nchunk - t * 8)          # chunks this tile
                    ne = te * 128                        # edges this tile
                    if t % 4 == 0:
                        tc32 = min(32, nchunk - t * 8)
                        et = ep.tile([64, 4096], f8, tag="et")
                        nc.sync.dma_start(
                            out=et[:, :tc32 * 128],
                            in_=eaT[:, t * 1024: t * 1024 + tc32 * 128])
                    qs_g = gp.tile([128, 8, 128], bf16, tag="qsg")
                    nc.gpsimd.dma_gather(
                        qs_g[:, :te, :], QsD[:, :],
                        qsix[:, t * 64: t * 64 + te * 8],
                        te * 128, te * 128, 128)
                    half = 0
                    qhalf = (t % 4) * 8

                    # one-hot aggregation matrices: const-only deps, built
                    # ahead so the agg matmuls never wait on DVE
                    ohs_t = []
                    for c in range(te):
                        gc = t * 8 + c
                        oh_ = ohp.tile([128, 128], bf16, tag="ohS")
                        nc.vector.tensor_scalar(
                            out=oh_[:], in0=io128[:],
                            scalar1=dlp[:, gc:gc + 1], scalar2=rcp[:, gc:gc + 1],
                            op0=OP.is_equal, op1=OP.mult)
                        ohs_t.append(oh_)

                    pre = pp.tile([128, 1024], f32, tag="pre")
                    qs_f = qs_g[:].rearrange("p a b -> p (a b)")
                    for c in range(te):
                        gc = t * 8 + c
                        w = gc // cw
                        csl = slice(c * 128, (c + 1) * 128)
                        csl2 = slice((half + c) * 128, (half + c + 1) * 128)
                        csl4 = slice((qhalf + c) * 128, (qhalf + c + 1) * 128)
                        nc.tensor.matmul(out=pre[:, csl], lhsT=et[:, csl4],
                                         rhs=we_l, start=True, stop=False)
                        nc.tensor.matmul(out=pre[:, csl], lhsT=idnb[:],
                                         rhs=qs_f[:, csl2], start=False, stop=False)
                        nc.tensor.matmul(out=pre[:, csl], lhsT=ohT_res[:, gc, :],
                                         rhs=qd_sb[:, w, :], start=False, stop=True)

                    # nonlinearity: m = (1+tanh(a/2)) * softplus(b)
                    #             = 2*sigmoid(a)*softplus(b)  (2 absorbed by BN)
                    # nonlinearity: u = [e^-a | e^b], v = ln(1+u) = [sp(-a)|sp(b)]
                    # even tiles (ACT): sigma = e^-sp(-a); odd tiles (DVE):
                    # sigma = 1/(1+e^-a). m = 2*sigma*sp(b) (2 absorbed by BN
                    # via 4*EPS).
                    uf = nlp.tile([128, 8, 128], bf16, tag="uf")
                    nc.scalar.activation(
                        out=uf[:, :te, :].rearrange("p a b -> p (a b)"),
                        in_=pre[:, :ne], func=AF.Exp, scale=1.0)
                    m = nlp.tile([128, 8, 64], bf16, tag="m")
                    if t % 3 == 0:
                        vf = nlp.tile([128, 8, 128], bf16, tag="vf")
                        nc.scalar.activation(
                            out=vf[:, :te, :].rearrange("p a b -> p (a b)"),
                            in_=uf[:, :te, :].rearrange("p a b -> p (a b)"),
                            func=AF.Ln, bias=1.0, scale=1.0)
                        sg = nlp.tile([128, 8, 64], bf16, tag="sg")
                        nc.scalar.activation(out=sg[:, :te, :],
                                             in_=vf[:, :te, 0:64],
                                             func=AF.Exp, scale=-1.0)
                        nc.vector.scalar_tensor_tensor(
                            out=m[:, :te, :], in0=sg[:, :te, :], scalar=2.0,
                            in1=vf[:, :te, 64:128], op0=OP.mult, op1=OP.mult)
                    else:
                        vs = nlp.tile([128, 8, 64], bf16, tag="vs")
                        nc.scalar.activation(out=vs[:, :te, :],
                                             in_=uf[:, :te, 64:128],
                                             func=AF.Ln, bias=1.0, scale=1.0)
                        w1 = nlp.tile([128, 8, 64], bf16, tag="sg")
                        with nc.allow_low_precision(reason="sigmoid in bf16"):
                            nc.vector.tensor_scalar(out=w1[:, :te, :],
                                                    in0=uf[:, :te, 0:64],
                                                    scalar1=1.0, scalar2=None,
                                                    op0=OP.add)
                            nc.vector.reciprocal(out=w1[:, :te, :],
                                                 in_=w1[:, :te, :])
                        nc.vector.scalar_tensor_tensor(
                            out=m[:, :te, :], in0=w1[:, :te, :], scalar=2.0,
                            in1=vs[:, :te, :], op0=OP.mult, op1=OP.mult)

                    pend.append((m, ohs_t, t, te))
                    if len(pend) > 1:
                        emit_agg(*pend.pop(0))
                while pend:
                    emit_agg(*pend.pop(0))

                # --- BN stats + AllReduce ---
                st = wp.tile([D1, 2], f32, tag="st")
                nc.vector.reduce_sum(out=st[:, 0:1], in_=st1g[:],
                                     axis=mybir.AxisListType.X)
                nc.vector.reduce_sum(out=st[:, 1:2], in_=st2g[:],
                                     axis=mybir.AxisListType.X)
                nc.sync.dma_start(out=ar_in[:, :], in_=st[:])
                nc.gpsimd.collective_compute(
                    "AllGather", OP.bypass, replica_groups=rg,
                    ins=[ar_in.ap().opt()], outs=[ar_out.ap().opt()])
                stga = wp.tile([D1, 2, NCORES], f32, tag="stga")
                nc.sync.dma_start(
                    out=stga[:],
                    in_=ar_out[:, :].rearrange("(c p) s -> p s c", p=D1))
                stg = wp.tile([D1, 2], f32, tag="stg")
                nc.vector.reduce_sum(
                    out=stg[:].rearrange("p (s o) -> p s o", o=1),
                    in_=stga[:], axis=mybir.AxisListType.X)
                mu = wp.tile([D1, 1], f32, tag="mu")
                nc.vector.tensor_scalar(out=mu[:], in0=stg[:, 0:1],
                                        scalar1=1.0 / N, scalar2=None, op0=OP.mult)
                ex2 = wp.tile([D1, 1], f32, tag="ex2")
                nc.vector.tensor_scalar(out=ex2[:], in0=stg[:, 1:2],
                                        scalar1=1.0 / N, scalar2=None, op0=OP.mult)
                nvar = wp.tile([D1, 1], f32, tag="var")
                nc.vector.scalar_tensor_tensor(
                    out=nvar[:], in0=mu[:], scalar=mu[:], in1=ex2[:],
                    op0=OP.mult, op1=OP.subtract)
                lv = wp.tile([D1, 1], f32, tag="lv")
                # m carries a factor 2 -> aggr/mu scale by 2, var by 4; 4*EPS
                # rides as an extra bng column; nvar = mu^2-ex2 = -var, so the
                # Ln input is nvar*(-1) + 4*EPS.
                nc.scalar.activation(out=lv[:], in_=nvar[:], func=AF.Ln,
                                     bias=gmt[:, L:L + 1], scale=-1.0)
                isd = wp.tile([D1, 1], f32, tag="isd")
                nc.scalar.activation(out=isd[:], in_=lv[:], func=AF.Exp, scale=-0.5)
                scale = wp.tile([D1, 1], f32, tag="scale")
                nc.vector.tensor_tensor(out=scale[:], in0=isd[:],
                                        in1=gmt[:, l:l + 1], op=OP.mult)
                mshift = wp.tile([D1, 1], f32, tag="mshift")
                nc.vector.tensor_tensor(out=mshift[:], in0=mu[:], in1=scale[:],
                                        op=OP.mult)
                shift = wp.tile([D1, 1], f32, tag="shift")
                nc.vector.tensor_tensor(out=shift[:], in0=bbt[:, l:l + 1],
                                        in1=mshift[:], op=OP.subtract)
                # h = relu((aggr*scale + h) + shift); the three consumers
                # (f32 residual, f8 AllGather payload, bf16 matmul copy) are
                # produced from asb concurrently on DVE/ACT/Pool
                HB = SHARD_P // 2
                for hh in range(2):
                    hsl = slice(hh * HB, (hh + 1) * HB)
                    nc.vector.scalar_tensor_tensor(
                        out=asb[:, hsl], in0=aggr_sb[:, hsl], scalar=scale[:],
                        in1=hT_own[:, hsl], op0=OP.mult, op1=OP.add)
                if l + 1 < L:
                    h8n = stp.tile([D1, SHARD_P], f8, tag="h8")
                    for hh in range(2):
                        hsl = slice(hh * HB, (hh + 1) * HB)
                        nc.scalar.activation(out=h8n[:, hsl], in_=asb[:, hsl],
                                             func=AF.Relu, bias=shift[:],
                                             scale=1.0)
                        nc.sync.dma_start(out=ag_in[:, hsl], in_=h8n[:, hsl])
                if l + 1 < L:
                    nc.vector.tensor_scalar(out=hT_own[:], in0=asb[:],
                                            scalar1=shift[:], scalar2=0.0,
                                            op0=OP.add, op1=OP.max)
                    nc.gpsimd.tensor_copy(out=hb_own[:], in_=hT_own[:])
                else:
                    # last layer: no AllGather payload to produce on ACT and
                    # no next Qd build needing hb_own; do the relu on ACT in
                    # halves (pipelined behind the stt halves on DVE) so the
                    # pool phase starts sooner
                    for hh in range(2):
                        hsl = slice(hh * HB, (hh + 1) * HB)
                        nc.scalar.activation(out=hT_own[:, hsl],
                                             in_=asb[:, hsl], func=AF.Relu,
                                             bias=shift[:], scale=1.0)

            # ---------- global mean pool ----------
            pool_ps = pp.tile([D1, G], f32, tag="pre")
            for w in range(NWIN):
                tp = ppB.tile([128, D1], f32, tag="bld")
                nc.tensor.transpose(out=tp[:], in_=hT_own[:, w * 128:(w + 1) * 128],
                                    identity=idn[0:D1, 0:D1])
                rows = wp.tile([128, D1], bf16, tag="rows")
                nc.vector.tensor_copy(out=rows[:], in_=tp[:])
                ohg = ohgp.tile([128, G], bf16, tag="ohg")
                nc.vector.tensor_scalar(
                    out=ohg[:], in0=io256[:],
                    scalar1=blc[:, w:w + 1], scalar2=rgp[:, w:w + 1],
                    op0=OP.is_equal, op1=OP.mult)
                nc.tensor.matmul(out=pool_ps[:], lhsT=rows[:], rhs=ohg[:],
                                 start=(w == 0), stop=(w == NWIN - 1))
            poolT = tlp.tile([D1, G], bf16, tag="poolT")
            nc.vector.tensor_copy(out=poolT[:], in_=pool_ps[:])
            nc.sync.dma_start(out=pl_in[:, :], in_=poolT[:])
            nc.gpsimd.collective_compute(
                "AllGather", OP.bypass, replica_groups=rg,
                ins=[pl_in.ap().opt()], outs=[pl_out.ap().opt()])
            pga = tlp.tile([D1, NCORES, G], bf16, tag="pga")
            nc.sync.dma_start(
                out=pga[:],
                in_=pl_out[:, :].rearrange("(c p) g -> p c g", p=D1))
            pg = tlp.tile([D1, G], f32, tag="pg")
            nc.vector.reduce_sum(
                out=pg[:].rearrange("p (g o) -> p g o", o=1),
                in_=pga[:].rearrange("p c g -> p g c"),
                axis=mybir.AxisListType.X)

            # ---------- head ----------
            a = pg
            hw_ = [(l1w[:], l1b[:]), (fw[:, 0:D2], fb[:, 0:1]), (fw[:, D2:2 * D2], fb[:, 1:2])]
            for (wt, bt) in hw_:
                ps = ppB.tile([D2, G], f32, tag="bld")
                nc.tensor.matmul(out=ps[:, 0:G], lhsT=wt, rhs=a[:], start=True, stop=True)
                an = tlp.tile([D2, G], f32, tag="an")
                nc.scalar.activation(out=an[:], in_=ps[:, 0:G], func=AF.Relu,
                                     bias=bt, scale=1.0)
                a = an
            ps = ppB.tile([1, G], f32, tag="bld")
            nc.tensor.matmul(out=ps[:, 0:G], lhsT=l2w[:], rhs=a[:], start=True, stop=True)
            yt = tlp.tile([1, G], f32, tag="yt")
            nc.scalar.activation(out=yt[:], in_=ps[:, 0:G], func=AF.Identity,
                                 bias=l2b[:], scale=1.0)
            nc.sync.dma_start(out=yout[:, :], in_=yt[:])

    nc.compile()
    return nc


def _wrap16(idx):
    """Flat idx list -> [128, n/16] int16: slot i at [i%16, i//16], replicated
    across the 8 Q7 cores."""
    a = idx.reshape(-1, 16).T.astype(np.int16)
    return np.tile(a, (8, 1))


def _preprocess(inputs):
    x = np.asarray(inputs["x"], np.float32)
    ea = np.asarray(inputs["edge_attr"], np.float32)
    ei = np.asarray(inputs["edge_index"]).astype(np.int64)
    batch = np.asarray(inputs["batch"]).astype(np.int64)
    src, dst = ei[0], ei[1]

    cnt = np.bincount(dst, minlength=N).astype(np.float32)
    rc_node = 1.0 / np.maximum(cnt, 1.0)
    gcnt = np.bincount(batch, minlength=G).astype(np.float32)
    rgc = 1.0 / np.maximum(gcnt, 1.0)

    # Degree-balanced node -> (window, slot) assignment: snake-deal nodes in
    # descending-degree order across the 240 global windows, minimizing the
    # max per-window edge count (which sets the uniform chunk pad cw).
    deg_order = np.argsort(-cnt, kind="stable")       # node ids, deg desc
    nwin_g = NCORES * NWIN                            # 240
    perm_loc = np.empty(N, np.int64)                  # node -> global padded id
    for i0 in range(0, N, nwin_g):
        blk = deg_order[i0:i0 + nwin_g]
        j = i0 // nwin_g
        wins = np.arange(len(blk)) if j % 2 == 0 else (len(blk) - 1 - np.arange(len(blk)))
        w_ids = wins
        perm_loc[blk] = (w_ids // NWIN) * SHARD_P + (w_ids % NWIN) * 128 + j
    gperm = perm_loc
    srcg = gperm[src]
    dstg = gperm[dst]
    order = np.argsort(dstg, kind="stable")
    srcg_s, dstg_s, ea_idx = srcg[order], dstg[order], order

    bounds = []
    for c in range(NCORES):
        for w in range(NWIN):
            bounds.append(c * SHARD_P + w * 128)
    bounds.append(NCORES * SHARD_P)
    bpos = np.searchsorted(dstg_s, np.asarray(bounds), side="left")
    percw = {}
    maxcnt = 0
    k = 0
    for c in range(NCORES):
        for w in range(NWIN):
            lo, hi = bpos[k], bpos[k + 1]
            percw[(c, w)] = np.arange(lo, hi)
            maxcnt = max(maxcnt, hi - lo)
            k += 1
    cw = max(1, (maxcnt + 127) // 128)
    etot = NWIN * cw * 128

    # full padded x, rotated per core so block 0 is the own shard
    xfull = np.zeros((NF, NCORES * SHARD_P), np.float32)
    xfull[:, gperm] = x.T
    xfull = xfull.astype(ml_dtypes.bfloat16)

    per_core = []
    for c in range(NCORES):
        qs_idx = np.zeros(etot, np.int64)
        dl = np.full(etot, -1.0, np.float32)
        rc_e = np.ones(etot, np.float32)
        ea_e = np.zeros((etot, EF), np.float32)
        for w in range(NWIN):
            idxs = percw[(c, w)]
            o = w * cw * 128
            k = len(idxs)
            g = srcg_s[idxs]                           # padded global id
            qs_idx[o:o + k] = (g % 128) * NWING + (g // 128)
            loc = dstg_s[idxs] - c * SHARD_P           # 0..3839
            dl[o:o + k] = (loc - w * 128).astype(np.float32)
            rc_e[o:o + k] = rc_node[dst[ea_idx[idxs]]]
            ea_e[o:o + k] = ea[ea_idx[idxs]]
        eaT = np.zeros((64, etot), np.float32)
        eaT[1:EF + 1] = ea_e.T
        eaT[EF + 1] = 1.0
        eaT[EF + 1, dl < 0] = 0.0
        nch = etot // 128
        ohT = np.zeros((128, etot), np.float32)
        vv = dl >= 0
        ohT[dl[vv].astype(np.int64), np.nonzero(vv)[0]] = 1.0
        d = {
            "qs_idxD": _wrap16(qs_idx),
            "ohTD": ohT.astype(ml_dtypes.float8_e4m3),
            "dstloc_p": dl.reshape(nch, 128).T.copy(),
            "rc_p": rc_e.reshape(nch, 128).T.copy(),
            "eaT": eaT.astype(ml_dtypes.float8_e4m3),
        }
        d["xTF"] = xfull
        d["xT"] = xfull[:, c * SHARD_P:(c + 1) * SHARD_P].copy()
        nodes_c = np.nonzero((gperm // SHARD_P) == c)[0]
        locs_c = gperm[nodes_c] - c * SHARD_P
        bl = np.full(SHARD_P, -1.0, np.float32)
        bl[locs_c] = batch[nodes_c].astype(np.float32)
        rg_n = np.zeros(SHARD_P, np.float32)
        rg_n[locs_c] = rgc[batch[nodes_c]]
        d["batchloc"] = bl.reshape(NWIN, 128).T.copy()
        d["rgc_pn"] = rg_n.reshape(NWIN, 128).T.copy()
        per_core.append(d)

    # replicated weights; f-gate halves pre-negated
    wf = np.asarray(inputs["conv_wf"], np.float32)
    wsv = np.asarray(inputs["conv_ws"], np.float32)
    bf = np.asarray(inputs["conv_bf"], np.float32)
    bs = np.asarray(inputs["conv_bs"], np.float32)
    wdst = np.concatenate([-wf[:, 0:D1, :], wsv[:, 0:D1, :]], axis=2)
    wsrc = np.concatenate([-wf[:, D1:2 * D1, :], wsv[:, D1:2 * D1, :]], axis=2)
    wea = np.concatenate([-wf[:, 2 * D1:, :], wsv[:, 2 * D1:, :]], axis=2)
    bias = np.concatenate([-bf, bs], axis=1)[:, None, :]
    wea = np.concatenate([wea, bias], axis=1)
    shared = {
        "lin0w": np.asarray(inputs["lin0_w"], np.float32).astype(ml_dtypes.bfloat16),
        "lin0b": np.asarray(inputs["lin0_b"], np.float32).reshape(D1, 1),
        "wdst": np.transpose(wdst, (1, 0, 2)).reshape(D1, L * 128).astype(ml_dtypes.bfloat16),
        "wsrc": np.transpose(wsrc, (1, 0, 2)).reshape(D1, L * 128).astype(ml_dtypes.bfloat16),
        "wea": np.concatenate([
            np.zeros((1, L * 128), np.float32),
            np.transpose(wea, (1, 0, 2)).reshape(EF + 1, L * 128),
            np.zeros((64 - EF - 2, L * 128), np.float32),
        ], axis=0).astype(ml_dtypes.bfloat16),
        "bng": np.concatenate(
            [np.asarray(inputs["bn_gamma"], np.float32).T,
             np.full((D1, 1), 4.0 * EPS, np.float32)], axis=1),
        "bnb": np.asarray(inputs["bn_beta"], np.float32).T.copy(),
        "lin1w": np.asarray(inputs["lin1_w"], np.float32),
        "lin1b": np.asarray(inputs["lin1_b"], np.float32).reshape(D2, 1),
        "fcw": np.transpose(np.asarray(inputs["fc_w"], np.float32), (1, 0, 2)).reshape(D2, FC * D2),
        "fcb": np.asarray(inputs["fc_b"], np.float32).T.copy(),
        "lin2w": np.asarray(inputs["lin2_w"], np.float32).reshape(D2, 1),
        "lin2b": np.asarray(inputs["lin2_b"], np.float32).reshape(1, 1),
        "iota128": np.broadcast_to(np.arange(128, dtype=np.float32)[None, :],
                                   (128, 128)).astype(ml_dtypes.bfloat16),
        "iota256": np.broadcast_to(np.arange(G, dtype=np.float32)[None, :],
                                   (128, G)).astype(ml_dtypes.bfloat16),
        "ident": np.eye(128, dtype=np.float32),
        "identb": np.eye(128, dtype=np.float32).astype(ml_dtypes.bfloat16),
    }
    in_maps = [dict(shared, **pc) for pc in per_core]
    return in_maps, cw


def kernel(**inputs):
    from concourse.bass_utils import run_bass_kernel_spmd

    in_maps, cw = _preprocess(inputs)
    key = ("nc", cw)
    if key not in _CACHE:
        _CACHE[key] = _build_nc(cw)
    nc = _CACHE[key]
    res = run_bass_kernel_spmd(nc, in_maps, core_ids=list(range(NCORES)))
    return res.results[0]["y"].reshape(G).astype(np.float32)
